# revision 1
# baseline (speedup 1.0000x reference)
"""Trainium2 Bass kernel for nn_CELossWithSVLS_VE (SVLS cross-entropy loss).

Math (derived + numerically validated vs reference):
  For the 26 non-center offsets n, with per-voxel
    u_n = exp(-0.5*(maxdiff_n^2 + r_n^2)),
    maxdiff_n(v) = max_c(img_c(v+n) - img_c(v))   (replicate-padded),
  the SVLS label weights reduce EXACTLY to w_center = 1/2, w_n = u_n/(2S),
  S = sum_n u_n.  Then
    loss(v) = lse(v) - 0.5*x_{l(v)}(v) - (1/(2S)) * sum_n u_n * x_{l(v+n)}(v)
  and the output is mean_v loss(v).

Engine plan (vs the 93.7us baseline):
  * u_n via ONE ScalarE activation: Derivative_Erf(m/sqrt2) = c*exp(-m^2/2);
    c cancels in T/S, and exp(-r2/2) moves into r2-scaled identity stationary
    matrices used by the PE accumulation matmuls (no bias/second activation).
  * most 4-channel stencil subtractions run on the PE as shift-matrix matmul
    pairs into PSUM; ScalarE copies PSUM->SBUF bf16 (the only engine that can
    get PSUM data back cheaply); DVE does only max/min trees + mask products.
  * loss folds into 3 per-partition accumulators (p_lse, p_yx, p_w) via
    accum_out side outputs; host combines  sum = p_lse - 0.5*p_yx + p_w.
    The T-dot reads accP straight out of PSUM (single-PSUM-operand TT).

Sharding: 8 cores, core k takes d-slab [8k, 8k+8) of both batches.
On-core layout: partition p = b*64 + h (128), free = (c?, d, w) with d,w
halos in SBUF.  h+-1 stencil shifts: PE shift-matrix matmuls (edge clamp
baked in) or partition-shifted SBUF DMA copies for the DVE-path frames.
"""
import sys
from contextlib import ExitStack

import numpy as np

if "/opt/trn_rl_repo" not in sys.path:
    sys.path.insert(0, "/opt/trn_rl_repo")

B, C, D, H, W = 2, 4, 64, 64, 64
NCORES = 8
DL = D // NCORES          # 8 local d-planes
DE, WE = DL + 2, W + 2    # 10, 66 (d/w halos)
P = 128                   # partitions = (b, h)
NVOX = B * D * H * W      # 524288

# 13 positive offsets; r2 = i*i+j*j+k*k.
PAIRS = [
    (1, 0, 0), (0, 0, 1), (1, 0, 1), (1, 0, -1),
    (0, 1, 0), (1, 1, 0), (1, -1, 0), (0, 1, 1), (0, 1, -1),
    (1, 1, 1), (1, 1, -1), (1, -1, 1), (1, -1, -1),
]

# ---- schedule config ----
USE_DERF = True
# j!=0 pairs whose min-frame runs on PE (rest: DVE sub via h-shifted copies)
MIN_ON_PE = {10, 11, 12, 4, 5, 6, 7}
# emission order (j0 pairs interleaved between PE-heavy pairs)
PAIR_ORDER = [0, 4, 5, 1, 10, 11, 2, 12, 6, 3, 7, 8, 9]
CPHASE_AT = 5   # slot at which the dx/label-gather DVE chain is emitted
CACT_AT = 3     # slot at which the exp/p_x0 Act work is emitted
LSE_EARLY = False
EXP_LATE = False
USE_XB = False
USE_RSB = False
TTR_YM = False    # tensor_tensor_reduce compiles but faults at runtime
TTR_TAIL = False
USE_POOLOPS = True
CB_FULL = False
B_LAG = 2       # DErf stage lags the sub/tree stage by 2 pairs
C_LAG = 3       # prods/accumulate stage lags by 3 pairs

_CACHED = {}

SQ2I = 0.7071067811865476  # 1/sqrt(2)


def _build_nc():
    import concourse.bacc as bacc
    import concourse.mybir as mybir
    import concourse.tile as tile

    AF = mybir.ActivationFunctionType
    ALU = mybir.AluOpType
    dt = mybir.dt

    nc = bacc.Bacc("TRN2", target_bir_lowering=False, debug=False,
                   num_devices=NCORES)
    img_d = nc.dram_tensor("img", [P, C * DE * WE], dt.bfloat16,
                           kind="ExternalInput")
    lab_d = nc.dram_tensor("lab", [P, DE * WE], dt.bfloat16,
                           kind="ExternalInput")
    logit_d = nc.dram_tensor("logits", [P, C * DL * W], dt.bfloat16,
                             kind="ExternalInput")
    # mats: [-I, Sh(+1), Sh(-1), I*e^-.5, I*e^-1, I*e^-1.5,
    eye_d = nc.dram_tensor("eye", [P, 6 * P], dt.bfloat16,
                           kind="ExternalInput")
    out_d = nc.dram_tensor("partials", [P, 4], dt.float32,
                           kind="ExternalOutput")

    import concourse.bass as bass_mod

    with tile.TileContext(nc) as tc, ExitStack() as ctx:
        persist = ctx.enter_context(tc.tile_pool(name="persist", bufs=1))
        cpool = ctx.enter_context(tc.tile_pool(name="cpool", bufs=1))
        trans = ctx.enter_context(tc.tile_pool(name="trans", bufs=3))
        upool = ctx.enter_context(
            tc.tile_pool(name="upool", bufs=max(3, B_LAG + 1, C_LAG - B_LAG + 2)))
        psum = ctx.enter_context(
            tc.tile_pool(name="psum", bufs=1, space=bass_mod.MemorySpace.PSUM))
        psum2 = ctx.enter_context(
            tc.tile_pool(name="psum2", bufs=(1 if CB_FULL else 2),
                         space=bass_mod.MemorySpace.PSUM))

        f32, bf16 = dt.float32, dt.bfloat16
        TT = nc.vector.tensor_tensor

        # ---- loads (images/labels arrive pre-cast to bf16 from host) ----
        # The DMA engines serialize transfers, so order by first use: mats
        # (PE idles until it lands), labf (masks), imgb per-channel, then the
        # big f32 logits tensor (only needed once the Act exp work starts).
        mats = persist.tile([P, 6, P], bf16, tag="mats")
        nc.sync.dma_start(mats[:], eye_d[:, :])
        labf = persist.tile([P, DE, WE], bf16, tag="labf")
        imgb = persist.tile([P, C, DE, WE], bf16, tag="imgb")
        for c in range(C):
            nc.sync.dma_start(imgb[:, c],
                              img_d[:, c * DE * WE:(c + 1) * DE * WE])
        nc.sync.dma_start(labf[:], lab_d[:, :])
        x = persist.tile([P, C, DL, W], bf16, tag="x")
        nc.sync.dma_start(x[:], logit_d[:, :])

        negI = mats[:, 0]
        shm = {1: mats[:, 1], -1: mats[:, 2]}
        ir2 = {1.0: mats[:, 3], 2.0: mats[:, 4], 3.0: mats[:, 5]}

        masks = persist.tile([P, 3, DE, WE], bf16, tag="masks")

        def emit_masks():
            eng = nc.gpsimd if USE_POOLOPS else nc.vector
            for ci, cval in enumerate((1.0, 2.0, 3.0)):
                eng.tensor_scalar(masks[:, ci], labf[:], cval, None,
                                  ALU.is_equal)

        # ---- h-shifted copies (partition shift via SBUF->SBUF DMA).
        def hshift_copies(dst_p, dst_m, src, eng):
            eng.dma_start(dst_p[0:63], src[1:64])
            eng.dma_start(dst_p[64:127], src[65:128])
            eng.dma_start(dst_p[63:64], src[63:64])
            eng.dma_start(dst_p[127:128], src[127:128])
            eng.dma_start(dst_m[1:64], src[0:63])
            eng.dma_start(dst_m[65:128], src[64:127])
            eng.dma_start(dst_m[0:1], src[0:1])
            eng.dma_start(dst_m[64:65], src[64:65])

        # masks_h before imgb_h: first mask-product use is much earlier than
        # the first DVE-path min-frame. SP ring so Act SEQ never blocks.
        masks_hp = persist.tile([P, 3, DE, WE], bf16, tag="masks_hp")
        masks_hm = persist.tile([P, 3, DE, WE], bf16, tag="masks_hm")
        msk_h = {1: masks_hp, 0: masks, -1: masks_hm}
        need_imgb_h = len(MIN_ON_PE) < 9
        if need_imgb_h:
            imgb_hp = persist.tile([P, C, DE, WE], bf16, tag="imgb_hp")
            imgb_hm = persist.tile([P, C, DE, WE], bf16, tag="imgb_hm")
            img_h = {1: imgb_hp, 0: imgb, -1: imgb_hm}

        def emit_hshifts():
            hshift_copies(masks_hp, masks_hm, masks, nc.sync)
            if need_imgb_h:
                hshift_copies(imgb_hp, imgb_hm, imgb, nc.sync)

        def cv(tile_, i, k):
            """center view shifted by (i, ., k) of a [..., DE, WE] tile."""
            return tile_[:, :, 1 + i:1 + i + DL, 1 + k:1 + k + W]

        # ---- PSUM accumulators; PE accumulates via r2-scaled identities ----
        accP = psum.tile([P, 3, DL, W], f32, tag="accP")
        SP = psum.tile([P, DL, W], f32, tag="SP")

        pl = cpool.tile([P, 4], f32, tag="pl")
        scr1 = cpool.tile([P, DL, W], f32, tag="scr1")
        scr2 = cpool.tile([P, DL, W], f32, tag="scr2")
        scr3 = cpool.tile([P, DL, W], f32, tag="scr3")

        cph = {}

        def emit_cphase_act():
            # exp-set work up front while PE/DVE wind up; p_x0 on the side
            if USE_XB:
                xb = cpool.tile([P, C, DL, W], bf16, tag="xb")
                nc.scalar.activation(xb[:], x[:], AF.Copy, scale=-0.5)
                cph.update(xb=xb)
            expx = cpool.tile([P, C, DL, W], bf16, tag="expx")
            nc.scalar.activation(expx[:], x[:], AF.Exp)
            nc.scalar.activation(scr1[:], x[:, 0], AF.Copy,
                                 accum_out=pl[:, 3:4])
            cph.update(expx=expx)

        def emit_expsum():
            expx = cph['expx']
            e2 = cpool.tile([P, 2, DL, W], bf16, tag="e2")
            TT(e2[:], expx[:, 0:2], expx[:, 2:4], ALU.add)
            esum = cpool.tile([P, DL, W], bf16, tag="esum")
            TT(esum[:], e2[:, 0], e2[:, 1], ALU.add)
            cph.update(esum=esum)

        def emit_cphase_front():
            # DVE part: dxb; fused p_ym = sum(m_c * dx_c)
            if not EXP_LATE:
                emit_expsum()
            xs = cph['xb'] if USE_XB else x
            dxb = cpool.tile([P, 3, DL, W], bf16, tag="dxb")
            TT(dxb[:], xs[:, 1:4], xs[:, 0:1].broadcast_to((P, 3, DL, W)),
               ALU.subtract)
            # p_ym = sum over (c,d,w) of m_c*dx_c: one stt dot with accum
            # (needs a contiguous mask-center copy; Pool makes it for free)
            mc = cpool.tile([P, 3, DL, W], bf16, tag="mc")
            nc.gpsimd.tensor_copy(mc[:], cv(masks, 0, 0))
            ym = cpool.tile([P, 3, DL, W], bf16, tag="ym")
            nc.vector.scalar_tensor_tensor(ym[:], mc[:], 1.0, dxb[:],
                                           ALU.mult, ALU.mult,
                                           accum_out=pl[:, 1:2])
            cph.update(dxb=dxb)
            if LSE_EARLY:
                emit_cphase_back()

        def emit_cphase_back():
            # p_lse: one act-table switch back to the ln/exp set
            if EXP_LATE:
                emit_cphase_act()
                emit_expsum()
            nc.scalar.activation(scr2[:], cph['esum'], AF.Ln,
                                 accum_out=pl[:, 0:1])

        def pe_frame(jj, ii, kk, mop, m1p_slot):
            """d4 = Sh_jj.T@view(ii,kk) - center on PE; Act copyback halves;
            DVE tree into m1p_slot."""
            cb = trans.tile([P, C, DL, W], bf16, tag="cb")
            if CB_FULL:
                d4p = psum2.tile([P, C, DL, W], f32, tag="d4p")
                for c in range(C):
                    nc.tensor.matmul(d4p[:, c], shm[jj],
                                     imgb[:, c, 1 + ii:1 + ii + DL,
                                          1 + kk:1 + kk + W],
                                     start=True, stop=False)
                    nc.tensor.matmul(d4p[:, c], negI,
                                     imgb[:, c, 1:1 + DL, 1:1 + W],
                                     start=False, stop=True)
                nc.scalar.copy(cb[:], d4p[:])
            else:
                for half in range(2):
                    d4p = psum2.tile([P, 2, DL, W], f32, tag="d4p")
                    for cc in range(2):
                        c = 2 * half + cc
                        nc.tensor.matmul(d4p[:, cc], shm[jj],
                                         imgb[:, c, 1 + ii:1 + ii + DL,
                                              1 + kk:1 + kk + W],
                                         start=True, stop=False)
                        nc.tensor.matmul(d4p[:, cc], negI,
                                         imgb[:, c, 1:1 + DL, 1:1 + W],
                                         start=False, stop=True)
                    nc.scalar.copy(cb[:, 2 * half:2 * half + 2], d4p[:])
            m2 = trans.tile([P, 2, DL, W], bf16, tag="m2")
            TT(m2[:], cb[:, 0:2], cb[:, 2:4], mop)
            TT(m1p_slot, m2[:, 0], m2[:, 1], mop)

        # ---- software-pipelined main loop over offset pairs ----
        # slot p: [C] prods+acc for pair p-2, [B] DErf for pair p-1,
        #         [A] subs/copybacks/trees for pair p.
        m1p_t, up_t = {}, {}

        def stage_A(pi):
            i, j, k = PAIRS[pi]
            m1p = upool.tile([P, 2, DL, W], bf16, tag="m1p")
            m1p_t[pi] = m1p
            if j == 0:
                # single sub on an extended box serves both frames as views
                nd, nw = (9 if i else 8), (65 if k else 64)
                d0, w0 = (0 if i == 1 else 1), (0 if k == 1 else 1)
                dpe = trans.tile([P, C, nd, nw], bf16, tag="dpe")
                if pi == PAIR_ORDER[0]:
                    for ch in range(0, C, 2):
                        TT(dpe[:, ch:ch + 2],
                           imgb[:, ch:ch + 2, d0 + i:d0 + i + nd,
                                w0 + k:w0 + k + nw],
                           imgb[:, ch:ch + 2, d0:d0 + nd, w0:w0 + nw],
                           ALU.subtract)
                else:
                    TT(dpe[:],
                       imgb[:, :, d0 + i:d0 + i + nd, w0 + k:w0 + k + nw],
                       imgb[:, :, d0:d0 + nd, w0:w0 + nw], ALU.subtract)
                for fr in range(2):
                    ds = 1 - d0 - (i if fr else 0)
                    ws = 1 - w0 - (k if fr else 0)
                    mop = ALU.max if fr == 0 else ALU.min
                    dv = dpe[:, :, ds:ds + DL, ws:ws + W]
                    m2 = trans.tile([P, 2, DL, W], bf16, tag="m2")
                    TT(m2[:], dv[:, 0:2], dv[:, 2:4], mop)
                    TT(m1p[:, fr], m2[:, 0], m2[:, 1], mop)
            else:
                pe_frame(j, i, k, ALU.max, m1p[:, 0])
                if pi in MIN_ON_PE:
                    # sign-free: min tree of Sh_-j view(-i,-k) - center
                    pe_frame(-j, -i, -k, ALU.min, m1p[:, 1])
                else:
                    d4 = trans.tile([P, C, DL, W], bf16, tag="d4")
                    TT(d4[:], cv(imgb, 0, 0), cv(img_h[-j], -i, -k),
                       ALU.subtract)
                    m2n = trans.tile([P, 2, DL, W], bf16, tag="m2n")
                    TT(m2n[:], d4[:, 0:2], d4[:, 2:4], ALU.min)
                    TT(m1p[:, 1], m2n[:, 0], m2n[:, 1], ALU.min)

        def stage_B(pi):
            # u for both frames in one activation (const 2/sqrt(pi) cancels;
            # exp(-r2/2) lives in the scaled identity used by the acc matmuls)
            up = upool.tile([P, 2, DL, W], bf16, tag="up")
            up_t[pi] = up
            m1p = m1p_t.pop(pi)
            if USE_DERF:
                nc.scalar.activation(up[:], m1p[:], AF.Derivative_Erf,
                                     scale=SQ2I)
            else:
                sqp = trans.tile([P, 2, DL, W], bf16, tag="sqp")
                nc.scalar.activation(sqp[:], m1p[:], AF.Square)
                nc.scalar.activation(up[:], sqp[:], AF.Exp, scale=-0.5)

        def stage_C(pi):
            i, j, k = PAIRS[pi]
            r2 = float(i * i + j * j + k * k)
            st, sp = (pi == PAIR_ORDER[0]), (pi == PAIR_ORDER[-1])
            up = up_t.pop(pi)
            for fr, sgn in ((0, 1), (1, -1)):
                si, sj, sk = sgn * i, sgn * j, sgn * k
                mview = cv(msk_h[sj], si, sk)
                ub = up[:, fr:fr + 1].broadcast_to((P, 3, DL, W))
                prods = trans.tile([P, 3, DL, W], bf16, tag="prods")
                TT(prods[:], ub, mview, ALU.mult)
                nc.tensor.matmul(SP[:], ir2[r2], up[:, fr],
                                 start=(st and fr == 0), stop=(sp and fr == 1))
                for ci in range(3):
                    nc.tensor.matmul(accP[:, ci], ir2[r2], prods[:, ci],
                                     start=(st and fr == 0),
                                     stop=(sp and fr == 1))

        NP = len(PAIR_ORDER)
        for idx in range(NP + max(B_LAG, C_LAG)):
            if idx == CPHASE_AT:
                emit_cphase_front()
            if idx == 0:
                emit_masks()
            if idx < NP:
                stage_A(PAIR_ORDER[idx])
            if idx == 0:
                emit_hshifts()
            if B_LAG <= idx < NP + B_LAG:
                stage_B(PAIR_ORDER[idx - B_LAG])
            if C_LAG <= idx < NP + C_LAG:
                stage_C(PAIR_ORDER[idx - C_LAG])
            if idx == CACT_AT and not EXP_LATE:
                emit_cphase_act()
        if CPHASE_AT >= NP:
            emit_cphase_front()
        if not LSE_EARLY:
            emit_cphase_back()

        # ---- tail: p_w = sum(-0.5/S * sum_c dx_c*acc_c) ----
        dxb = cph['dxb']
        rS = cpool.tile([P, DL, W], f32, tag="rS")
        nc.vector.reciprocal_approx_fast(rS[:], SP[:])
        tp = cpool.tile([P, 3, DL, W], bf16, tag="tp")
        TT(tp[:], accP[:], dxb[:], ALU.mult)
        t1 = cpool.tile([P, DL, W], bf16, tag="t1")
        TT(t1[:], tp[:, 0], tp[:, 1], ALU.add)
        t2 = cpool.tile([P, DL, W], bf16, tag="t2")
        TT(t2[:], t1[:], tp[:, 2], ALU.add)
        nc.vector.scalar_tensor_tensor(scr3[:], t2[:], -0.5, rS[:],
                                       ALU.mult, ALU.mult,
                                       accum_out=pl[:, 2:3])
        nc.sync.dma_start(out_d[:, :], pl[:])

    nc.compile()
    return nc


def _get_nc():
    if "nc" not in _CACHED:
        _CACHED["nc"] = _build_nc()
    return _CACHED["nc"]


def make_in_maps(inputs, labels, images):
    """Host-side shard: full inputs -> per-core input dicts (layout prep:
    (b,h)->partition transpose, d/w halo padding, bf16 pre-cast)."""
    import ml_dtypes

    bf = ml_dtypes.bfloat16
    img = np.asarray(images, np.float32).astype(bf)
    lab = np.asarray(labels).astype(bf)  # values 0..3, exact in bf16
    lgt = np.ascontiguousarray(np.asarray(inputs, np.float32))

    img_p = np.pad(img, ((0, 0), (0, 0), (1, 1), (0, 0), (1, 1)), mode="edge")
    lab_p = np.pad(lab, ((0, 0), (1, 1), (0, 0), (1, 1)), mode="edge")

    in_maps = []
    for k in range(NCORES):
        d0 = k * DL
        ic = img_p[:, :, d0:d0 + DE]          # [2,4,10,64,66]
        lc = lab_p[:, d0:d0 + DE]             # [2,10,64,66]
        xc = lgt[:, :, d0:d0 + DL]            # [2,4,8,64,64]
        im = np.ascontiguousarray(ic.transpose(0, 3, 1, 2, 4)).reshape(P, -1)
        lm = np.ascontiguousarray(lc.transpose(0, 2, 1, 3)).reshape(P, -1)
        xm = np.ascontiguousarray(
            xc.transpose(0, 3, 1, 2, 4)).reshape(P, -1).astype(bf)
        in_maps.append({"img": im, "lab": lm, "logits": xm, "eye": _mats()})
    return in_maps


def _mats():
    """[-I, Sh(+1), Sh(-1), e^-.5 I, e^-1 I, e^-1.5 I] as one [P, 6P] bf16
    array. Sh(j)[k, m] = 1 iff k = b(m)*64 + clamp(h(m)+j, 0, 63)."""
    import ml_dtypes

    eye = np.eye(P, dtype=np.float32)
    sh = {}
    for jj in (1, -1):
        M = np.zeros((P, P), np.float32)
        for m in range(P):
            b, h = divmod(m, 64)
            M[b * 64 + min(max(h + jj, 0), 63), m] = 1.0
        sh[jj] = M
    blocks = [-eye, sh[1], sh[-1],
              np.exp(-0.5) * eye, np.exp(-1.0) * eye, np.exp(-1.5) * eye]
    out = np.concatenate(blocks, axis=1)
    return np.ascontiguousarray(out).astype(ml_dtypes.bfloat16)


def kernel(inputs, labels, images):
    from concourse.bass_utils import run_bass_kernel_spmd

    nc = _get_nc()
    in_maps = make_in_maps(inputs, labels, images)
    res = run_bass_kernel_spmd(nc, in_maps, core_ids=list(range(NCORES)))
    total = 0.0
    for k in range(NCORES):
        pl = res.results[k]["partials"].astype(np.float64)
        ym_scale = -1.0 if USE_XB else 0.5
        total += (pl[:, 0] - ym_scale * pl[:, 1] - pl[:, 3] + pl[:, 2]).sum()
    return np.float32(total / NVOX)



# revision 2
# speedup vs baseline: 1.0008x; 1.0008x over previous
"""Trainium2 Bass kernel for nn_CELossWithSVLS_VE (SVLS cross-entropy loss).

Math (derived + numerically validated vs reference):
  For the 26 non-center offsets n, with per-voxel
    u_n = exp(-0.5*(maxdiff_n^2 + r_n^2)),
    maxdiff_n(v) = max_c(img_c(v+n) - img_c(v))   (replicate-padded),
  the SVLS label weights reduce EXACTLY to w_center = 1/2, w_n = u_n/(2S),
  S = sum_n u_n.  Then
    loss(v) = lse(v) - 0.5*x_{l(v)}(v) - (1/(2S)) * sum_n u_n * x_{l(v+n)}(v)
  and the output is mean_v loss(v).

Engine plan (vs the 93.7us baseline):
  * u_n via ONE ScalarE activation: Derivative_Erf(m/sqrt2) = c*exp(-m^2/2);
    c cancels in T/S, and exp(-r2/2) moves into r2-scaled identity stationary
    matrices used by the PE accumulation matmuls (no bias/second activation).
  * most 4-channel stencil subtractions run on the PE as shift-matrix matmul
    pairs into PSUM; ScalarE copies PSUM->SBUF bf16 (the only engine that can
    get PSUM data back cheaply); DVE does only max/min trees + mask products.
  * loss folds into 3 per-partition accumulators (p_lse, p_yx, p_w) via
    accum_out side outputs; host combines  sum = p_lse - 0.5*p_yx + p_w.
    The T-dot reads accP straight out of PSUM (single-PSUM-operand TT).

Sharding: 8 cores, core k takes d-slab [8k, 8k+8) of both batches.
On-core layout: partition p = b*64 + h (128), free = (c?, d, w) with d,w
halos in SBUF.  h+-1 stencil shifts: PE shift-matrix matmuls (edge clamp
baked in) or partition-shifted SBUF DMA copies for the DVE-path frames.
"""
import sys
from contextlib import ExitStack

import numpy as np

if "/opt/trn_rl_repo" not in sys.path:
    sys.path.insert(0, "/opt/trn_rl_repo")

B, C, D, H, W = 2, 4, 64, 64, 64
NCORES = 8
DL = D // NCORES          # 8 local d-planes
DE, WE = DL + 2, W + 2    # 10, 66 (d/w halos)
P = 128                   # partitions = (b, h)
NVOX = B * D * H * W      # 524288

# 13 positive offsets; r2 = i*i+j*j+k*k.
PAIRS = [
    (1, 0, 0), (0, 0, 1), (1, 0, 1), (1, 0, -1),
    (0, 1, 0), (1, 1, 0), (1, -1, 0), (0, 1, 1), (0, 1, -1),
    (1, 1, 1), (1, 1, -1), (1, -1, 1), (1, -1, -1),
]

import os as _os, json as _json
_ov = _json.loads(_os.environ.get("KCONF", "{}"))
T_J0POOL = set(_ov.get("j0pool", []))     # j0 pairs: m2 on Pool
T_MINPOOL = set(_ov.get("minpool", []))   # DVE-min-path pairs: m2n on Pool
T_MASKS_DVE = _ov.get("masks_dve", 0)
T_EXPSUM_POOL = _ov.get("expsum_pool", 0)
T_PRODS_POOL = {tuple(t) for t in _ov.get("prods_pool",
                                           [[4, 1], [10, 1]])}
T_PSPLIT = _ov.get("psplit", 1)           # prods stage one slot early
T_CLAG = _ov.get("clag", 3)
T_BLAG = _ov.get("blag", 2)
# ---- schedule config ----
USE_DERF = True
# j!=0 pairs whose min-frame runs on PE (rest: DVE sub via h-shifted copies)
MIN_ON_PE = {10, 11, 12, 4, 5, 6, 7}
# emission order (j0 pairs interleaved between PE-heavy pairs)
PAIR_ORDER = [0, 4, 5, 1, 10, 11, 2, 12, 6, 3, 7, 8, 9]
CPHASE_AT = 5   # slot at which the dx/label-gather DVE chain is emitted
CACT_AT = 3     # slot at which the exp/p_x0 Act work is emitted
LSE_EARLY = False
EXP_LATE = False
USE_XB = False
USE_RSB = False
TTR_YM = False    # tensor_tensor_reduce compiles but faults at runtime
TTR_TAIL = False
USE_POOLOPS = True
CB_FULL = False
B_LAG = _ov.get("blag", 2)
C_LAG = _ov.get("clag", 3)

_CACHED = {}

SQ2I = 0.7071067811865476  # 1/sqrt(2)


def _build_nc():
    import concourse.bacc as bacc
    import concourse.mybir as mybir
    import concourse.tile as tile

    AF = mybir.ActivationFunctionType
    ALU = mybir.AluOpType
    dt = mybir.dt

    nc = bacc.Bacc("TRN2", target_bir_lowering=False, debug=False,
                   num_devices=NCORES)
    img_d = nc.dram_tensor("img", [P, C * DE * WE], dt.bfloat16,
                           kind="ExternalInput")
    lab_d = nc.dram_tensor("lab", [P, DE * WE], dt.bfloat16,
                           kind="ExternalInput")
    logit_d = nc.dram_tensor("logits", [P, C * DL * W], dt.bfloat16,
                             kind="ExternalInput")
    # mats: [-I, Sh(+1), Sh(-1), I*e^-.5, I*e^-1, I*e^-1.5,
    eye_d = nc.dram_tensor("eye", [P, 6 * P], dt.bfloat16,
                           kind="ExternalInput")
    out_d = nc.dram_tensor("partials", [P, 4], dt.float32,
                           kind="ExternalOutput")

    import concourse.bass as bass_mod

    with tile.TileContext(nc) as tc, ExitStack() as ctx:
        persist = ctx.enter_context(tc.tile_pool(name="persist", bufs=1))
        cpool = ctx.enter_context(tc.tile_pool(name="cpool", bufs=1))
        trans = ctx.enter_context(tc.tile_pool(name="trans", bufs=3))
        upool = ctx.enter_context(
            tc.tile_pool(name="upool", bufs=max(3, B_LAG + 1, C_LAG - B_LAG + 2)))
        psum = ctx.enter_context(
            tc.tile_pool(name="psum", bufs=1, space=bass_mod.MemorySpace.PSUM))
        psum2 = ctx.enter_context(
            tc.tile_pool(name="psum2", bufs=(1 if CB_FULL else 2),
                         space=bass_mod.MemorySpace.PSUM))

        f32, bf16 = dt.float32, dt.bfloat16
        TT = nc.vector.tensor_tensor

        # ---- loads (images/labels arrive pre-cast to bf16 from host) ----
        # The DMA engines serialize transfers, so order by first use: mats
        # (PE idles until it lands), labf (masks), imgb per-channel, then the
        # big f32 logits tensor (only needed once the Act exp work starts).
        mats = persist.tile([P, 6, P], bf16, tag="mats")
        nc.sync.dma_start(mats[:], eye_d[:, :])
        labf = persist.tile([P, DE, WE], bf16, tag="labf")
        imgb = persist.tile([P, C, DE, WE], bf16, tag="imgb")
        for c in range(C):
            nc.sync.dma_start(imgb[:, c],
                              img_d[:, c * DE * WE:(c + 1) * DE * WE])
        nc.sync.dma_start(labf[:], lab_d[:, :])
        x = persist.tile([P, C, DL, W], bf16, tag="x")
        nc.sync.dma_start(x[:], logit_d[:, :])

        negI = mats[:, 0]
        shm = {1: mats[:, 1], -1: mats[:, 2]}
        ir2 = {1.0: mats[:, 3], 2.0: mats[:, 4], 3.0: mats[:, 5]}

        masks = persist.tile([P, 3, DE, WE], bf16, tag="masks")

        def emit_masks():
            eng = nc.vector if T_MASKS_DVE else (
                nc.gpsimd if USE_POOLOPS else nc.vector)
            for ci, cval in enumerate((1.0, 2.0, 3.0)):
                eng.tensor_scalar(masks[:, ci], labf[:], cval, None,
                                  ALU.is_equal)

        # ---- h-shifted copies (partition shift via SBUF->SBUF DMA).
        def hshift_copies(dst_p, dst_m, src, eng):
            eng.dma_start(dst_p[0:63], src[1:64])
            eng.dma_start(dst_p[64:127], src[65:128])
            eng.dma_start(dst_p[63:64], src[63:64])
            eng.dma_start(dst_p[127:128], src[127:128])
            eng.dma_start(dst_m[1:64], src[0:63])
            eng.dma_start(dst_m[65:128], src[64:127])
            eng.dma_start(dst_m[0:1], src[0:1])
            eng.dma_start(dst_m[64:65], src[64:65])

        # masks_h before imgb_h: first mask-product use is much earlier than
        # the first DVE-path min-frame. SP ring so Act SEQ never blocks.
        masks_hp = persist.tile([P, 3, DE, WE], bf16, tag="masks_hp")
        masks_hm = persist.tile([P, 3, DE, WE], bf16, tag="masks_hm")
        msk_h = {1: masks_hp, 0: masks, -1: masks_hm}
        need_imgb_h = len(MIN_ON_PE) < 9
        if need_imgb_h:
            imgb_hp = persist.tile([P, C, DE, WE], bf16, tag="imgb_hp")
            imgb_hm = persist.tile([P, C, DE, WE], bf16, tag="imgb_hm")
            img_h = {1: imgb_hp, 0: imgb, -1: imgb_hm}

        def emit_hshifts():
            hshift_copies(masks_hp, masks_hm, masks, nc.sync)
            if need_imgb_h:
                hshift_copies(imgb_hp, imgb_hm, imgb, nc.sync)

        def cv(tile_, i, k):
            """center view shifted by (i, ., k) of a [..., DE, WE] tile."""
            return tile_[:, :, 1 + i:1 + i + DL, 1 + k:1 + k + W]

        # ---- PSUM accumulators; PE accumulates via r2-scaled identities ----
        accP = psum.tile([P, 3, DL, W], f32, tag="accP")
        SP = psum.tile([P, DL, W], f32, tag="SP")

        pl = cpool.tile([P, 4], f32, tag="pl")
        scr1 = cpool.tile([P, DL, W], f32, tag="scr1")
        scr2 = cpool.tile([P, DL, W], f32, tag="scr2")
        scr3 = cpool.tile([P, DL, W], f32, tag="scr3")

        cph = {}

        def emit_cphase_act():
            # exp-set work up front while PE/DVE wind up; p_x0 on the side
            if USE_XB:
                xb = cpool.tile([P, C, DL, W], bf16, tag="xb")
                nc.scalar.activation(xb[:], x[:], AF.Copy, scale=-0.5)
                cph.update(xb=xb)
            expx = cpool.tile([P, C, DL, W], bf16, tag="expx")
            nc.scalar.activation(expx[:], x[:], AF.Exp)
            nc.scalar.activation(scr1[:], x[:, 0], AF.Copy,
                                 accum_out=pl[:, 3:4])
            cph.update(expx=expx)

        def emit_expsum():
            expx = cph['expx']
            e2 = cpool.tile([P, 2, DL, W], bf16, tag="e2")
            ene = nc.gpsimd if T_EXPSUM_POOL else nc.vector
            ene.tensor_tensor(e2[:], expx[:, 0:2], expx[:, 2:4], ALU.add)
            esum = cpool.tile([P, DL, W], bf16, tag="esum")
            ene.tensor_tensor(esum[:], e2[:, 0], e2[:, 1], ALU.add)
            cph.update(esum=esum)

        def emit_cphase_front():
            # DVE part: dxb; fused p_ym = sum(m_c * dx_c)
            if not EXP_LATE:
                emit_expsum()
            xs = cph['xb'] if USE_XB else x
            dxb = cpool.tile([P, 3, DL, W], bf16, tag="dxb")
            TT(dxb[:], xs[:, 1:4], xs[:, 0:1].broadcast_to((P, 3, DL, W)),
               ALU.subtract)
            # p_ym = sum over (c,d,w) of m_c*dx_c: one stt dot with accum
            # (needs a contiguous mask-center copy; Pool makes it for free)
            mc = cpool.tile([P, 3, DL, W], bf16, tag="mc")
            nc.gpsimd.tensor_copy(mc[:], cv(masks, 0, 0))
            ym = cpool.tile([P, 3, DL, W], bf16, tag="ym")
            nc.vector.scalar_tensor_tensor(ym[:], mc[:], 1.0, dxb[:],
                                           ALU.mult, ALU.mult,
                                           accum_out=pl[:, 1:2])
            cph.update(dxb=dxb)
            if LSE_EARLY:
                emit_cphase_back()

        def emit_cphase_back():
            # p_lse: one act-table switch back to the ln/exp set
            if EXP_LATE:
                emit_cphase_act()
                emit_expsum()
            nc.scalar.activation(scr2[:], cph['esum'], AF.Ln,
                                 accum_out=pl[:, 0:1])

        def pe_frame(jj, ii, kk, mop, m1p_slot):
            """d4 = Sh_jj.T@view(ii,kk) - center on PE; Act copyback halves;
            DVE tree into m1p_slot."""
            cb = trans.tile([P, C, DL, W], bf16, tag="cb")
            if CB_FULL:
                d4p = psum2.tile([P, C, DL, W], f32, tag="d4p")
                for c in range(C):
                    nc.tensor.matmul(d4p[:, c], shm[jj],
                                     imgb[:, c, 1 + ii:1 + ii + DL,
                                          1 + kk:1 + kk + W],
                                     start=True, stop=False)
                    nc.tensor.matmul(d4p[:, c], negI,
                                     imgb[:, c, 1:1 + DL, 1:1 + W],
                                     start=False, stop=True)
                nc.scalar.copy(cb[:], d4p[:])
            else:
                for half in range(2):
                    d4p = psum2.tile([P, 2, DL, W], f32, tag="d4p")
                    for cc in range(2):
                        c = 2 * half + cc
                        nc.tensor.matmul(d4p[:, cc], shm[jj],
                                         imgb[:, c, 1 + ii:1 + ii + DL,
                                              1 + kk:1 + kk + W],
                                         start=True, stop=False)
                        nc.tensor.matmul(d4p[:, cc], negI,
                                         imgb[:, c, 1:1 + DL, 1:1 + W],
                                         start=False, stop=True)
                    nc.scalar.copy(cb[:, 2 * half:2 * half + 2], d4p[:])
            m2 = trans.tile([P, 2, DL, W], bf16, tag="m2")
            TT(m2[:], cb[:, 0:2], cb[:, 2:4], mop)
            TT(m1p_slot, m2[:, 0], m2[:, 1], mop)

        # ---- software-pipelined main loop over offset pairs ----
        m1p_t, up_t, prods_t = {}, {}, {}
        P_LAG = max(B_LAG, C_LAG - T_PSPLIT)

        def stage_A(pi):
            i, j, k = PAIRS[pi]
            m1p = upool.tile([P, 2, DL, W], bf16, tag="m1p")
            m1p_t[pi] = m1p
            if j == 0:
                # single sub on an extended box serves both frames as views
                nd, nw = (9 if i else 8), (65 if k else 64)
                d0, w0 = (0 if i == 1 else 1), (0 if k == 1 else 1)
                dpe = trans.tile([P, C, nd, nw], bf16, tag="dpe")
                if pi == PAIR_ORDER[0]:
                    for ch in range(0, C, 2):
                        TT(dpe[:, ch:ch + 2],
                           imgb[:, ch:ch + 2, d0 + i:d0 + i + nd,
                                w0 + k:w0 + k + nw],
                           imgb[:, ch:ch + 2, d0:d0 + nd, w0:w0 + nw],
                           ALU.subtract)
                else:
                    TT(dpe[:],
                       imgb[:, :, d0 + i:d0 + i + nd, w0 + k:w0 + k + nw],
                       imgb[:, :, d0:d0 + nd, w0:w0 + nw], ALU.subtract)
                for fr in range(2):
                    ds = 1 - d0 - (i if fr else 0)
                    ws = 1 - w0 - (k if fr else 0)
                    mop = ALU.max if fr == 0 else ALU.min
                    dv = dpe[:, :, ds:ds + DL, ws:ws + W]
                    m2 = trans.tile([P, 2, DL, W], bf16, tag="m2")
                    eng = nc.gpsimd if pi in T_J0POOL else nc.vector
                    eng.tensor_tensor(m2[:], dv[:, 0:2], dv[:, 2:4], mop)
                    TT(m1p[:, fr], m2[:, 0], m2[:, 1], mop)
            else:
                pe_frame(j, i, k, ALU.max, m1p[:, 0])
                if pi in MIN_ON_PE:
                    # sign-free: min tree of Sh_-j view(-i,-k) - center
                    pe_frame(-j, -i, -k, ALU.min, m1p[:, 1])
                else:
                    d4 = trans.tile([P, C, DL, W], bf16, tag="d4")
                    TT(d4[:], cv(imgb, 0, 0), cv(img_h[-j], -i, -k),
                       ALU.subtract)
                    m2n = trans.tile([P, 2, DL, W], bf16, tag="m2n")
                    enm = nc.gpsimd if pi in T_MINPOOL else nc.vector
                    enm.tensor_tensor(m2n[:], d4[:, 0:2], d4[:, 2:4],
                                      ALU.min)
                    TT(m1p[:, 1], m2n[:, 0], m2n[:, 1], ALU.min)

        def stage_B(pi):
            # u for both frames in one activation (const 2/sqrt(pi) cancels;
            # exp(-r2/2) lives in the scaled identity used by the acc matmuls)
            up = upool.tile([P, 2, DL, W], bf16, tag="up")
            up_t[pi] = up
            m1p = m1p_t.pop(pi)
            if USE_DERF:
                nc.scalar.activation(up[:], m1p[:], AF.Derivative_Erf,
                                     scale=SQ2I)
            else:
                sqp = trans.tile([P, 2, DL, W], bf16, tag="sqp")
                nc.scalar.activation(sqp[:], m1p[:], AF.Square)
                nc.scalar.activation(up[:], sqp[:], AF.Exp, scale=-0.5)

        def stage_P(pi):
            i, j, k = PAIRS[pi]
            up = up_t[pi]
            pr2 = trans.tile([P, 2, 3, DL, W], bf16, tag="prods",
                             name="pr2", bufs=C_LAG + 2)
            prods_t[pi] = pr2
            for fr, sgn in ((0, 1), (1, -1)):
                si, sj, sk = sgn * i, sgn * j, sgn * k
                mview = cv(msk_h[sj], si, sk)
                ub = up[:, fr:fr + 1].broadcast_to((P, 3, DL, W))
                eng = nc.gpsimd if (pi, fr) in T_PRODS_POOL else nc.vector
                eng.tensor_tensor(pr2[:, fr], ub, mview, ALU.mult)

        def stage_C(pi):
            i, j, k = PAIRS[pi]
            r2 = float(i * i + j * j + k * k)
            st, sp = (pi == PAIR_ORDER[0]), (pi == PAIR_ORDER[-1])
            up = up_t.pop(pi)
            pr2 = prods_t.pop(pi)
            for fr in range(2):
                nc.tensor.matmul(SP[:], ir2[r2], up[:, fr],
                                 start=(st and fr == 0), stop=(sp and fr == 1))
                for ci in range(3):
                    nc.tensor.matmul(accP[:, ci], ir2[r2], pr2[:, fr, ci],
                                     start=(st and fr == 0),
                                     stop=(sp and fr == 1))

        NP = len(PAIR_ORDER)
        for idx in range(NP + max(B_LAG, C_LAG)):
            if idx == CPHASE_AT:
                emit_cphase_front()
            if idx == 0:
                emit_masks()
            if idx < NP:
                stage_A(PAIR_ORDER[idx])
            if idx == 0:
                emit_hshifts()
            if B_LAG <= idx < NP + B_LAG:
                stage_B(PAIR_ORDER[idx - B_LAG])
            if P_LAG <= idx < NP + P_LAG:
                stage_P(PAIR_ORDER[idx - P_LAG])
            if C_LAG <= idx < NP + C_LAG:
                stage_C(PAIR_ORDER[idx - C_LAG])
            if idx == CACT_AT and not EXP_LATE:
                emit_cphase_act()
        if CPHASE_AT >= NP:
            emit_cphase_front()
        if not LSE_EARLY:
            emit_cphase_back()

        # ---- tail: p_w = sum(-0.5/S * sum_c dx_c*acc_c) ----
        dxb = cph['dxb']
        rS = cpool.tile([P, DL, W], f32, tag="rS")
        nc.vector.reciprocal_approx_fast(rS[:], SP[:])
        tp = cpool.tile([P, 3, DL, W], bf16, tag="tp")
        TT(tp[:], accP[:], dxb[:], ALU.mult)
        t1 = cpool.tile([P, DL, W], bf16, tag="t1")
        TT(t1[:], tp[:, 0], tp[:, 1], ALU.add)
        t2 = cpool.tile([P, DL, W], bf16, tag="t2")
        TT(t2[:], t1[:], tp[:, 2], ALU.add)
        nc.vector.scalar_tensor_tensor(scr3[:], t2[:], -0.5, rS[:],
                                       ALU.mult, ALU.mult,
                                       accum_out=pl[:, 2:3])
        nc.sync.dma_start(out_d[:, :], pl[:])

    nc.compile()
    return nc


def _get_nc():
    if "nc" not in _CACHED:
        _CACHED["nc"] = _build_nc()
    return _CACHED["nc"]


def make_in_maps(inputs, labels, images):
    """Host-side shard: full inputs -> per-core input dicts (layout prep:
    (b,h)->partition transpose, d/w halo padding, bf16 pre-cast)."""
    import ml_dtypes

    bf = ml_dtypes.bfloat16
    img = np.asarray(images, np.float32).astype(bf)
    lab = np.asarray(labels).astype(bf)  # values 0..3, exact in bf16
    lgt = np.ascontiguousarray(np.asarray(inputs, np.float32))

    img_p = np.pad(img, ((0, 0), (0, 0), (1, 1), (0, 0), (1, 1)), mode="edge")
    lab_p = np.pad(lab, ((0, 0), (1, 1), (0, 0), (1, 1)), mode="edge")

    in_maps = []
    for k in range(NCORES):
        d0 = k * DL
        ic = img_p[:, :, d0:d0 + DE]          # [2,4,10,64,66]
        lc = lab_p[:, d0:d0 + DE]             # [2,10,64,66]
        xc = lgt[:, :, d0:d0 + DL]            # [2,4,8,64,64]
        im = np.ascontiguousarray(ic.transpose(0, 3, 1, 2, 4)).reshape(P, -1)
        lm = np.ascontiguousarray(lc.transpose(0, 2, 1, 3)).reshape(P, -1)
        xm = np.ascontiguousarray(
            xc.transpose(0, 3, 1, 2, 4)).reshape(P, -1).astype(bf)
        in_maps.append({"img": im, "lab": lm, "logits": xm, "eye": _mats()})
    return in_maps


def _mats():
    """[-I, Sh(+1), Sh(-1), e^-.5 I, e^-1 I, e^-1.5 I] as one [P, 6P] bf16
    array. Sh(j)[k, m] = 1 iff k = b(m)*64 + clamp(h(m)+j, 0, 63)."""
    import ml_dtypes

    eye = np.eye(P, dtype=np.float32)
    sh = {}
    for jj in (1, -1):
        M = np.zeros((P, P), np.float32)
        for m in range(P):
            b, h = divmod(m, 64)
            M[b * 64 + min(max(h + jj, 0), 63), m] = 1.0
        sh[jj] = M
    blocks = [-eye, sh[1], sh[-1],
              np.exp(-0.5) * eye, np.exp(-1.0) * eye, np.exp(-1.5) * eye]
    out = np.concatenate(blocks, axis=1)
    return np.ascontiguousarray(out).astype(ml_dtypes.bfloat16)


def kernel(inputs, labels, images):
    from concourse.bass_utils import run_bass_kernel_spmd

    nc = _get_nc()
    in_maps = make_in_maps(inputs, labels, images)
    res = run_bass_kernel_spmd(nc, in_maps, core_ids=list(range(NCORES)))
    total = 0.0
    for k in range(NCORES):
        pl = res.results[k]["partials"].astype(np.float64)
        ym_scale = -1.0 if USE_XB else 0.5
        total += (pl[:, 0] - ym_scale * pl[:, 1] - pl[:, 3] + pl[:, 2]).sum()
    return np.float32(total / NVOX)



# revision 3
# speedup vs baseline: 1.0382x; 1.0373x over previous
"""Trainium2 Bass kernel for nn_CELossWithSVLS_VE (SVLS cross-entropy loss).

Math (derived + numerically validated vs reference):
  For the 26 non-center offsets n, with per-voxel
    u_n = exp(-0.5*(maxdiff_n^2 + r_n^2)),
    maxdiff_n(v) = max_c(img_c(v+n) - img_c(v))   (replicate-padded),
  the SVLS label weights reduce EXACTLY to w_center = 1/2, w_n = u_n/(2S),
  S = sum_n u_n.  Then
    loss(v) = lse(v) - 0.5*x_{l(v)}(v) - (1/(2S)) * sum_n u_n * x_{l(v+n)}(v)
  and the output is mean_v loss(v).

Engine plan (70.2us; vs the 93.7us original and 70.5us prior best):
  * prods are emitted one pipeline slot ahead of their accumulation
    matmuls (stage_P/stage_C split) so the in-order PE stream never
    head-blocks on a late product; two min-frame products run on the
    otherwise-idle Pool engine.
  * B_LAG=3: the DErf stage trails the sub/tree stage by 3 pairs.
  (Prior plan below.)
  * u_n via ONE ScalarE activation: Derivative_Erf(m/sqrt2) = c*exp(-m^2/2);
    c cancels in T/S, and exp(-r2/2) moves into r2-scaled identity stationary
    matrices used by the PE accumulation matmuls (no bias/second activation).
  * most 4-channel stencil subtractions run on the PE as shift-matrix matmul
    pairs into PSUM; ScalarE copies PSUM->SBUF bf16 (the only engine that can
    get PSUM data back cheaply); DVE does only max/min trees + mask products.
  * loss folds into 3 per-partition accumulators (p_lse, p_yx, p_w) via
    accum_out side outputs; host combines  sum = p_lse - 0.5*p_yx + p_w.
    The T-dot reads accP straight out of PSUM (single-PSUM-operand TT).

Sharding: 8 cores, core k takes d-slab [8k, 8k+8) of both batches.
On-core layout: partition p = b*64 + h (128), free = (c?, d, w) with d,w
halos in SBUF.  h+-1 stencil shifts: PE shift-matrix matmuls (edge clamp
baked in) or partition-shifted SBUF DMA copies for the DVE-path frames.
"""
import sys
from contextlib import ExitStack

import numpy as np

if "/opt/trn_rl_repo" not in sys.path:
    sys.path.insert(0, "/opt/trn_rl_repo")

B, C, D, H, W = 2, 4, 64, 64, 64
NCORES = 8
DL = D // NCORES          # 8 local d-planes
DE, WE = DL + 2, W + 2    # 10, 66 (d/w halos)
P = 128                   # partitions = (b, h)
NVOX = B * D * H * W      # 524288

# 13 positive offsets; r2 = i*i+j*j+k*k.
PAIRS = [
    (1, 0, 0), (0, 0, 1), (1, 0, 1), (1, 0, -1),
    (0, 1, 0), (1, 1, 0), (1, -1, 0), (0, 1, 1), (0, 1, -1),
    (1, 1, 1), (1, 1, -1), (1, -1, 1), (1, -1, -1),
]

import os as _os, json as _json
_ov = _json.loads(_os.environ.get("KCONF", "{}"))
T_J0POOL = set(_ov.get("j0pool", []))     # j0 pairs: m2 on Pool
T_MINPOOL = set(_ov.get("minpool", []))   # DVE-min-path pairs: m2n on Pool
T_MASKS_DVE = _ov.get("masks_dve", 0)
T_EXPSUM_POOL = _ov.get("expsum_pool", 0)
T_PRODS_POOL = {tuple(t) for t in _ov.get("prods_pool",
                                           [[4, 1], [10, 1]])}
T_PSPLIT = _ov.get("psplit", 1)           # prods stage one slot early
T_CLAG = _ov.get("clag", 3)
T_BLAG = _ov.get("blag", 2)
# ---- schedule config ----
USE_DERF = True
# j!=0 pairs whose min-frame runs on PE (rest: DVE sub via h-shifted copies)
MIN_ON_PE = {10, 11, 12, 4, 5, 6, 7}
# emission order (j0 pairs interleaved between PE-heavy pairs)
PAIR_ORDER = _ov.get("order", [0, 4, 5, 1, 10, 11, 2, 12, 6, 3, 7, 8, 9])
CPHASE_AT = _ov.get("cphase", 5)
CACT_AT = _ov.get("cact", 3)
LSE_EARLY = False
EXP_LATE = False
USE_XB = False
USE_RSB = False
TTR_YM = False    # tensor_tensor_reduce compiles but faults at runtime
TTR_TAIL = False
USE_POOLOPS = True
CB_FULL = False
B_LAG = _ov.get("blag", 3)
C_LAG = _ov.get("clag", 3)

_CACHED = {}

SQ2I = 0.7071067811865476  # 1/sqrt(2)


def _build_nc():
    import concourse.bacc as bacc
    import concourse.mybir as mybir
    import concourse.tile as tile

    AF = mybir.ActivationFunctionType
    ALU = mybir.AluOpType
    dt = mybir.dt

    nc = bacc.Bacc("TRN2", target_bir_lowering=False, debug=False,
                   num_devices=NCORES)
    img_d = nc.dram_tensor("img", [P, C * DE * WE], dt.bfloat16,
                           kind="ExternalInput")
    lab_d = nc.dram_tensor("lab", [P, DE * WE], dt.bfloat16,
                           kind="ExternalInput")
    logit_d = nc.dram_tensor("logits", [P, C * DL * W], dt.bfloat16,
                             kind="ExternalInput")
    # mats: [-I, Sh(+1), Sh(-1), I*e^-.5, I*e^-1, I*e^-1.5,
    eye_d = nc.dram_tensor("eye", [P, 6 * P], dt.bfloat16,
                           kind="ExternalInput")
    out_d = nc.dram_tensor("partials", [P, 4], dt.float32,
                           kind="ExternalOutput")

    import concourse.bass as bass_mod

    with tile.TileContext(nc) as tc, ExitStack() as ctx:
        persist = ctx.enter_context(tc.tile_pool(name="persist", bufs=1))
        cpool = ctx.enter_context(tc.tile_pool(name="cpool", bufs=1))
        trans = ctx.enter_context(tc.tile_pool(name="trans", bufs=3))
        upool = ctx.enter_context(
            tc.tile_pool(name="upool", bufs=max(3, B_LAG + 1, C_LAG - B_LAG + 2)))
        psum = ctx.enter_context(
            tc.tile_pool(name="psum", bufs=1, space=bass_mod.MemorySpace.PSUM))
        psum2 = ctx.enter_context(
            tc.tile_pool(name="psum2", bufs=(1 if CB_FULL else 2),
                         space=bass_mod.MemorySpace.PSUM))

        f32, bf16 = dt.float32, dt.bfloat16
        TT = nc.vector.tensor_tensor

        # ---- loads (images/labels arrive pre-cast to bf16 from host) ----
        # The DMA engines serialize transfers, so order by first use: mats
        # (PE idles until it lands), labf (masks), imgb per-channel, then the
        # big f32 logits tensor (only needed once the Act exp work starts).
        mats = persist.tile([P, 6, P], bf16, tag="mats")
        nc.sync.dma_start(mats[:], eye_d[:, :])
        labf = persist.tile([P, DE, WE], bf16, tag="labf")
        imgb = persist.tile([P, C, DE, WE], bf16, tag="imgb")
        for c in range(C):
            nc.sync.dma_start(imgb[:, c],
                              img_d[:, c * DE * WE:(c + 1) * DE * WE])
        nc.sync.dma_start(labf[:], lab_d[:, :])
        x = persist.tile([P, C, DL, W], bf16, tag="x")
        nc.sync.dma_start(x[:], logit_d[:, :])

        negI = mats[:, 0]
        shm = {1: mats[:, 1], -1: mats[:, 2]}
        ir2 = {1.0: mats[:, 3], 2.0: mats[:, 4], 3.0: mats[:, 5]}

        masks = persist.tile([P, 3, DE, WE], bf16, tag="masks")

        def emit_masks():
            eng = nc.vector if T_MASKS_DVE else (
                nc.gpsimd if USE_POOLOPS else nc.vector)
            for ci, cval in enumerate((1.0, 2.0, 3.0)):
                eng.tensor_scalar(masks[:, ci], labf[:], cval, None,
                                  ALU.is_equal)

        # ---- h-shifted copies (partition shift via SBUF->SBUF DMA).
        def hshift_copies(dst_p, dst_m, src, eng):
            eng.dma_start(dst_p[0:63], src[1:64])
            eng.dma_start(dst_p[64:127], src[65:128])
            eng.dma_start(dst_p[63:64], src[63:64])
            eng.dma_start(dst_p[127:128], src[127:128])
            eng.dma_start(dst_m[1:64], src[0:63])
            eng.dma_start(dst_m[65:128], src[64:127])
            eng.dma_start(dst_m[0:1], src[0:1])
            eng.dma_start(dst_m[64:65], src[64:65])

        # masks_h before imgb_h: first mask-product use is much earlier than
        # the first DVE-path min-frame. SP ring so Act SEQ never blocks.
        masks_hp = persist.tile([P, 3, DE, WE], bf16, tag="masks_hp")
        masks_hm = persist.tile([P, 3, DE, WE], bf16, tag="masks_hm")
        msk_h = {1: masks_hp, 0: masks, -1: masks_hm}
        need_imgb_h = len(MIN_ON_PE) < 9
        if need_imgb_h:
            imgb_hp = persist.tile([P, C, DE, WE], bf16, tag="imgb_hp")
            imgb_hm = persist.tile([P, C, DE, WE], bf16, tag="imgb_hm")
            img_h = {1: imgb_hp, 0: imgb, -1: imgb_hm}

        def emit_hshifts():
            hshift_copies(masks_hp, masks_hm, masks, nc.sync)
            if need_imgb_h:
                hshift_copies(imgb_hp, imgb_hm, imgb, nc.sync)

        def cv(tile_, i, k):
            """center view shifted by (i, ., k) of a [..., DE, WE] tile."""
            return tile_[:, :, 1 + i:1 + i + DL, 1 + k:1 + k + W]

        # ---- PSUM accumulators; PE accumulates via r2-scaled identities ----
        accP = psum.tile([P, 3, DL, W], f32, tag="accP")
        SP = psum.tile([P, DL, W], f32, tag="SP")

        pl = cpool.tile([P, 4], f32, tag="pl")
        scr1 = cpool.tile([P, DL, W], f32, tag="scr1")
        scr2 = cpool.tile([P, DL, W], f32, tag="scr2")
        scr3 = cpool.tile([P, DL, W], f32, tag="scr3")

        cph = {}

        def emit_cphase_act():
            # exp-set work up front while PE/DVE wind up; p_x0 on the side
            if USE_XB:
                xb = cpool.tile([P, C, DL, W], bf16, tag="xb")
                nc.scalar.activation(xb[:], x[:], AF.Copy, scale=-0.5)
                cph.update(xb=xb)
            expx = cpool.tile([P, C, DL, W], bf16, tag="expx")
            nc.scalar.activation(expx[:], x[:], AF.Exp)
            nc.scalar.activation(scr1[:], x[:, 0], AF.Copy,
                                 accum_out=pl[:, 3:4])
            cph.update(expx=expx)

        def emit_expsum():
            expx = cph['expx']
            e2 = cpool.tile([P, 2, DL, W], bf16, tag="e2")
            ene = nc.gpsimd if T_EXPSUM_POOL else nc.vector
            ene.tensor_tensor(e2[:], expx[:, 0:2], expx[:, 2:4], ALU.add)
            esum = cpool.tile([P, DL, W], bf16, tag="esum")
            ene.tensor_tensor(esum[:], e2[:, 0], e2[:, 1], ALU.add)
            cph.update(esum=esum)

        def emit_cphase_front():
            # DVE part: dxb; fused p_ym = sum(m_c * dx_c)
            if not EXP_LATE:
                emit_expsum()
            xs = cph['xb'] if USE_XB else x
            dxb = cpool.tile([P, 3, DL, W], bf16, tag="dxb")
            TT(dxb[:], xs[:, 1:4], xs[:, 0:1].broadcast_to((P, 3, DL, W)),
               ALU.subtract)
            # p_ym = sum over (c,d,w) of m_c*dx_c: one stt dot with accum
            # (needs a contiguous mask-center copy; Pool makes it for free)
            mc = cpool.tile([P, 3, DL, W], bf16, tag="mc")
            nc.gpsimd.tensor_copy(mc[:], cv(masks, 0, 0))
            ym = cpool.tile([P, 3, DL, W], bf16, tag="ym")
            nc.vector.scalar_tensor_tensor(ym[:], mc[:], 1.0, dxb[:],
                                           ALU.mult, ALU.mult,
                                           accum_out=pl[:, 1:2])
            cph.update(dxb=dxb)
            if LSE_EARLY:
                emit_cphase_back()

        def emit_cphase_back():
            # p_lse: one act-table switch back to the ln/exp set
            if EXP_LATE:
                emit_cphase_act()
                emit_expsum()
            nc.scalar.activation(scr2[:], cph['esum'], AF.Ln,
                                 accum_out=pl[:, 0:1])

        def pe_frame(jj, ii, kk, mop, m1p_slot):
            """d4 = Sh_jj.T@view(ii,kk) - center on PE; Act copyback halves;
            DVE tree into m1p_slot."""
            cb = trans.tile([P, C, DL, W], bf16, tag="cb")
            if CB_FULL:
                d4p = psum2.tile([P, C, DL, W], f32, tag="d4p")
                for c in range(C):
                    nc.tensor.matmul(d4p[:, c], shm[jj],
                                     imgb[:, c, 1 + ii:1 + ii + DL,
                                          1 + kk:1 + kk + W],
                                     start=True, stop=False)
                    nc.tensor.matmul(d4p[:, c], negI,
                                     imgb[:, c, 1:1 + DL, 1:1 + W],
                                     start=False, stop=True)
                nc.scalar.copy(cb[:], d4p[:])
            else:
                for half in range(2):
                    d4p = psum2.tile([P, 2, DL, W], f32, tag="d4p")
                    for cc in range(2):
                        c = 2 * half + cc
                        nc.tensor.matmul(d4p[:, cc], shm[jj],
                                         imgb[:, c, 1 + ii:1 + ii + DL,
                                              1 + kk:1 + kk + W],
                                         start=True, stop=False)
                        nc.tensor.matmul(d4p[:, cc], negI,
                                         imgb[:, c, 1:1 + DL, 1:1 + W],
                                         start=False, stop=True)
                    nc.scalar.copy(cb[:, 2 * half:2 * half + 2], d4p[:])
            m2 = trans.tile([P, 2, DL, W], bf16, tag="m2")
            TT(m2[:], cb[:, 0:2], cb[:, 2:4], mop)
            TT(m1p_slot, m2[:, 0], m2[:, 1], mop)

        # ---- software-pipelined main loop over offset pairs ----
        m1p_t, up_t, prods_t = {}, {}, {}
        P_LAG = max(B_LAG, C_LAG - T_PSPLIT)

        def stage_A(pi):
            i, j, k = PAIRS[pi]
            m1p = upool.tile([P, 2, DL, W], bf16, tag="m1p")
            m1p_t[pi] = m1p
            if j == 0:
                # single sub on an extended box serves both frames as views
                nd, nw = (9 if i else 8), (65 if k else 64)
                d0, w0 = (0 if i == 1 else 1), (0 if k == 1 else 1)
                dpe = trans.tile([P, C, nd, nw], bf16, tag="dpe")
                if pi == PAIR_ORDER[0]:
                    for ch in range(0, C, 2):
                        TT(dpe[:, ch:ch + 2],
                           imgb[:, ch:ch + 2, d0 + i:d0 + i + nd,
                                w0 + k:w0 + k + nw],
                           imgb[:, ch:ch + 2, d0:d0 + nd, w0:w0 + nw],
                           ALU.subtract)
                else:
                    TT(dpe[:],
                       imgb[:, :, d0 + i:d0 + i + nd, w0 + k:w0 + k + nw],
                       imgb[:, :, d0:d0 + nd, w0:w0 + nw], ALU.subtract)
                for fr in range(2):
                    ds = 1 - d0 - (i if fr else 0)
                    ws = 1 - w0 - (k if fr else 0)
                    mop = ALU.max if fr == 0 else ALU.min
                    dv = dpe[:, :, ds:ds + DL, ws:ws + W]
                    m2 = trans.tile([P, 2, DL, W], bf16, tag="m2")
                    eng = nc.gpsimd if pi in T_J0POOL else nc.vector
                    eng.tensor_tensor(m2[:], dv[:, 0:2], dv[:, 2:4], mop)
                    TT(m1p[:, fr], m2[:, 0], m2[:, 1], mop)
            else:
                pe_frame(j, i, k, ALU.max, m1p[:, 0])
                if pi in MIN_ON_PE:
                    # sign-free: min tree of Sh_-j view(-i,-k) - center
                    pe_frame(-j, -i, -k, ALU.min, m1p[:, 1])
                else:
                    d4 = trans.tile([P, C, DL, W], bf16, tag="d4")
                    TT(d4[:], cv(imgb, 0, 0), cv(img_h[-j], -i, -k),
                       ALU.subtract)
                    m2n = trans.tile([P, 2, DL, W], bf16, tag="m2n")
                    enm = nc.gpsimd if pi in T_MINPOOL else nc.vector
                    enm.tensor_tensor(m2n[:], d4[:, 0:2], d4[:, 2:4],
                                      ALU.min)
                    TT(m1p[:, 1], m2n[:, 0], m2n[:, 1], ALU.min)

        def stage_B(pi):
            # u for both frames in one activation (const 2/sqrt(pi) cancels;
            # exp(-r2/2) lives in the scaled identity used by the acc matmuls)
            up = upool.tile([P, 2, DL, W], bf16, tag="up")
            up_t[pi] = up
            m1p = m1p_t.pop(pi)
            if USE_DERF:
                nc.scalar.activation(up[:], m1p[:], AF.Derivative_Erf,
                                     scale=SQ2I)
            else:
                sqp = trans.tile([P, 2, DL, W], bf16, tag="sqp")
                nc.scalar.activation(sqp[:], m1p[:], AF.Square)
                nc.scalar.activation(up[:], sqp[:], AF.Exp, scale=-0.5)

        def stage_P(pi):
            i, j, k = PAIRS[pi]
            up = up_t[pi]
            pr2 = trans.tile([P, 2, 3, DL, W], bf16, tag="prods",
                             name="pr2", bufs=C_LAG + 2)
            prods_t[pi] = pr2
            for fr, sgn in ((0, 1), (1, -1)):
                si, sj, sk = sgn * i, sgn * j, sgn * k
                mview = cv(msk_h[sj], si, sk)
                ub = up[:, fr:fr + 1].broadcast_to((P, 3, DL, W))
                eng = nc.gpsimd if (pi, fr) in T_PRODS_POOL else nc.vector
                eng.tensor_tensor(pr2[:, fr], ub, mview, ALU.mult)

        def stage_C(pi):
            i, j, k = PAIRS[pi]
            r2 = float(i * i + j * j + k * k)
            st, sp = (pi == PAIR_ORDER[0]), (pi == PAIR_ORDER[-1])
            up = up_t.pop(pi)
            pr2 = prods_t.pop(pi)
            for fr in range(2):
                nc.tensor.matmul(SP[:], ir2[r2], up[:, fr],
                                 start=(st and fr == 0), stop=(sp and fr == 1))
                for ci in range(3):
                    nc.tensor.matmul(accP[:, ci], ir2[r2], pr2[:, fr, ci],
                                     start=(st and fr == 0),
                                     stop=(sp and fr == 1))

        NP = len(PAIR_ORDER)
        for idx in range(NP + max(B_LAG, C_LAG)):
            if idx == CPHASE_AT:
                emit_cphase_front()
            if idx == 0:
                emit_masks()
            if idx < NP:
                stage_A(PAIR_ORDER[idx])
            if idx == 0:
                emit_hshifts()
            if B_LAG <= idx < NP + B_LAG:
                stage_B(PAIR_ORDER[idx - B_LAG])
            if P_LAG <= idx < NP + P_LAG:
                stage_P(PAIR_ORDER[idx - P_LAG])
            if C_LAG <= idx < NP + C_LAG:
                stage_C(PAIR_ORDER[idx - C_LAG])
            if idx == CACT_AT and not EXP_LATE:
                emit_cphase_act()
        if CPHASE_AT >= NP:
            emit_cphase_front()
        if not LSE_EARLY:
            emit_cphase_back()

        # ---- tail: p_w = sum(-0.5/S * sum_c dx_c*acc_c) ----
        dxb = cph['dxb']
        rS = cpool.tile([P, DL, W], f32, tag="rS")
        nc.vector.reciprocal_approx_fast(rS[:], SP[:])
        tp = cpool.tile([P, 3, DL, W], bf16, tag="tp")
        TT(tp[:], accP[:], dxb[:], ALU.mult)
        t1 = cpool.tile([P, DL, W], bf16, tag="t1")
        TT(t1[:], tp[:, 0], tp[:, 1], ALU.add)
        t2 = cpool.tile([P, DL, W], bf16, tag="t2")
        TT(t2[:], t1[:], tp[:, 2], ALU.add)
        nc.vector.scalar_tensor_tensor(scr3[:], t2[:], -0.5, rS[:],
                                       ALU.mult, ALU.mult,
                                       accum_out=pl[:, 2:3])
        nc.sync.dma_start(out_d[:, :], pl[:])

    nc.compile()
    return nc


def _get_nc():
    if "nc" not in _CACHED:
        _CACHED["nc"] = _build_nc()
    return _CACHED["nc"]


def make_in_maps(inputs, labels, images):
    """Host-side shard: full inputs -> per-core input dicts (layout prep:
    (b,h)->partition transpose, d/w halo padding, bf16 pre-cast)."""
    import ml_dtypes

    bf = ml_dtypes.bfloat16
    img = np.asarray(images, np.float32).astype(bf)
    lab = np.asarray(labels).astype(bf)  # values 0..3, exact in bf16
    lgt = np.ascontiguousarray(np.asarray(inputs, np.float32))

    img_p = np.pad(img, ((0, 0), (0, 0), (1, 1), (0, 0), (1, 1)), mode="edge")
    lab_p = np.pad(lab, ((0, 0), (1, 1), (0, 0), (1, 1)), mode="edge")

    in_maps = []
    for k in range(NCORES):
        d0 = k * DL
        ic = img_p[:, :, d0:d0 + DE]          # [2,4,10,64,66]
        lc = lab_p[:, d0:d0 + DE]             # [2,10,64,66]
        xc = lgt[:, :, d0:d0 + DL]            # [2,4,8,64,64]
        im = np.ascontiguousarray(ic.transpose(0, 3, 1, 2, 4)).reshape(P, -1)
        lm = np.ascontiguousarray(lc.transpose(0, 2, 1, 3)).reshape(P, -1)
        xm = np.ascontiguousarray(
            xc.transpose(0, 3, 1, 2, 4)).reshape(P, -1).astype(bf)
        in_maps.append({"img": im, "lab": lm, "logits": xm, "eye": _mats()})
    return in_maps


def _mats():
    """[-I, Sh(+1), Sh(-1), e^-.5 I, e^-1 I, e^-1.5 I] as one [P, 6P] bf16
    array. Sh(j)[k, m] = 1 iff k = b(m)*64 + clamp(h(m)+j, 0, 63)."""
    import ml_dtypes

    eye = np.eye(P, dtype=np.float32)
    sh = {}
    for jj in (1, -1):
        M = np.zeros((P, P), np.float32)
        for m in range(P):
            b, h = divmod(m, 64)
            M[b * 64 + min(max(h + jj, 0), 63), m] = 1.0
        sh[jj] = M
    blocks = [-eye, sh[1], sh[-1],
              np.exp(-0.5) * eye, np.exp(-1.0) * eye, np.exp(-1.5) * eye]
    out = np.concatenate(blocks, axis=1)
    return np.ascontiguousarray(out).astype(ml_dtypes.bfloat16)


def kernel(inputs, labels, images):
    from concourse.bass_utils import run_bass_kernel_spmd

    nc = _get_nc()
    in_maps = make_in_maps(inputs, labels, images)
    res = run_bass_kernel_spmd(nc, in_maps, core_ids=list(range(NCORES)))
    total = 0.0
    for k in range(NCORES):
        pl = res.results[k]["partials"].astype(np.float64)
        ym_scale = -1.0 if USE_XB else 0.5
        total += (pl[:, 0] - ym_scale * pl[:, 1] - pl[:, 3] + pl[:, 2]).sum()
    return np.float32(total / NVOX)



# revision 4
# speedup vs baseline: 1.0388x; 1.0006x over previous
"""Trainium2 Bass kernel for nn_CELossWithSVLS_VE (SVLS cross-entropy loss).

Math (derived + numerically validated vs reference):
  For the 26 non-center offsets n, with per-voxel
    u_n = exp(-0.5*(maxdiff_n^2 + r_n^2)),
    maxdiff_n(v) = max_c(img_c(v+n) - img_c(v))   (replicate-padded),
  the SVLS label weights reduce EXACTLY to w_center = 1/2, w_n = u_n/(2S),
  S = sum_n u_n.  Then
    loss(v) = lse(v) - 0.5*x_{l(v)}(v) - (1/(2S)) * sum_n u_n * x_{l(v+n)}(v)
  and the output is mean_v loss(v).

Engine plan (vs the 93.7us baseline):
  * u_n via ONE ScalarE activation: Derivative_Erf(m/sqrt2) = c*exp(-m^2/2);
    c cancels in T/S, and exp(-r2/2) moves into r2-scaled identity stationary
    matrices used by the PE accumulation matmuls (no bias/second activation).
  * most 4-channel stencil subtractions run on the PE as shift-matrix matmul
    pairs into PSUM; ScalarE copies PSUM->SBUF bf16 (the only engine that can
    get PSUM data back cheaply); DVE does only max/min trees + mask products.
  * loss folds into 3 per-partition accumulators (p_lse, p_yx, p_w) via
    accum_out side outputs; host combines  sum = p_lse - 0.5*p_yx + p_w.
    The T-dot reads accP straight out of PSUM (single-PSUM-operand TT).

Sharding: 8 cores, core k takes d-slab [8k, 8k+8) of both batches.
On-core layout: partition p = b*64 + h (128), free = (c?, d, w) with d,w
halos in SBUF.  h+-1 stencil shifts: PE shift-matrix matmuls (edge clamp
baked in) or partition-shifted SBUF DMA copies for the DVE-path frames.
"""
import sys
from contextlib import ExitStack

import numpy as np

if "/opt/trn_rl_repo" not in sys.path:
    sys.path.insert(0, "/opt/trn_rl_repo")

B, C, D, H, W = 2, 4, 64, 64, 64
NCORES = 8
DL = D // NCORES          # 8 local d-planes
DE, WE = DL + 2, W + 2    # 10, 66 (d/w halos)
P = 128                   # partitions = (b, h)
NVOX = B * D * H * W      # 524288

# 13 positive offsets; r2 = i*i+j*j+k*k.
PAIRS = [
    (1, 0, 0), (0, 0, 1), (1, 0, 1), (1, 0, -1),
    (0, 1, 0), (1, 1, 0), (1, -1, 0), (0, 1, 1), (0, 1, -1),
    (1, 1, 1), (1, 1, -1), (1, -1, 1), (1, -1, -1),
]

import os as _os, json as _json
_ov = _json.loads(_os.environ.get("KCONF", "{}"))
T_J0POOL = set(_ov.get("j0pool", []))     # j0 pairs: m2 on Pool
T_MINPOOL = set(_ov.get("minpool", []))   # DVE-min-path pairs: m2n on Pool
T_MASKS_DVE = _ov.get("masks_dve", 0)
T_EXPSUM_POOL = _ov.get("expsum_pool", 0)
T_PRODS_POOL = {tuple(t) for t in _ov.get("prods_pool",
                                           [[4, 1], [10, 1], [12, 1],
                                            [6, 1], [5, 1], [11, 1]])}
T_PSPLIT = _ov.get("psplit", 1)           # prods stage one slot early
T_MAXDVE = set(_ov.get("maxdve", [8, 9]))  # pairs: max-frame off PE
T_MINOFF = set(_ov.get("minoff", []))     # pairs removed from MIN_ON_PE
T_X0DVE = _ov.get("x0dve", 0)             # p_x0 accum via DVE tensor_scalar
T_YMPOOL = _ov.get("ympool", 0)           # ym STT on Pool
T_TAILSPLIT = _ov.get("tailsplit", 0)     # per-channel tail tp
T_DMAQ = _ov.get("dmaq", 0)               # mats/labf on Act DGE queue
# ---- schedule config ----
USE_DERF = True
# j!=0 pairs whose min-frame runs on PE (rest: DVE sub via h-shifted copies)
MIN_ON_PE = {10, 11, 12, 4, 5, 6, 7} - T_MINOFF
# emission order (j0 pairs interleaved between PE-heavy pairs)
PAIR_ORDER = _ov.get("order", [0, 4, 5, 1, 10, 11, 2, 12, 6, 3, 7, 8, 9])
CPHASE_AT = _ov.get("cphase", 5)
CACT_AT = _ov.get("cact", 3)
LSE_EARLY = False
EXP_LATE = False
USE_XB = False
USE_RSB = False
TTR_YM = False    # tensor_tensor_reduce compiles but faults at runtime
TTR_TAIL = False
USE_POOLOPS = True
CB_FULL = False
B_LAG = _ov.get("blag", 3)
C_LAG = _ov.get("clag", 3)

_CACHED = {}

SQ2I = 0.7071067811865476  # 1/sqrt(2)


def _build_nc():
    import concourse.bacc as bacc
    import concourse.mybir as mybir
    import concourse.tile as tile

    AF = mybir.ActivationFunctionType
    ALU = mybir.AluOpType
    dt = mybir.dt

    nc = bacc.Bacc("TRN2", target_bir_lowering=False, debug=False,
                   num_devices=NCORES)
    img_d = nc.dram_tensor("img", [P, C * DE * WE], dt.bfloat16,
                           kind="ExternalInput")
    lab_d = nc.dram_tensor("lab", [P, DE * WE], dt.bfloat16,
                           kind="ExternalInput")
    logit_d = nc.dram_tensor("logits", [P, C * DL * W], dt.bfloat16,
                             kind="ExternalInput")
    # mats: [-I, Sh(+1), Sh(-1), I*e^-.5, I*e^-1, I*e^-1.5,
    eye_d = nc.dram_tensor("eye", [P, 6 * P], dt.bfloat16,
                           kind="ExternalInput")
    out_d = nc.dram_tensor("partials", [P, 4], dt.float32,
                           kind="ExternalOutput")

    import concourse.bass as bass_mod

    with tile.TileContext(nc) as tc, ExitStack() as ctx:
        persist = ctx.enter_context(tc.tile_pool(name="persist", bufs=1))
        cpool = ctx.enter_context(tc.tile_pool(name="cpool", bufs=1))
        trans = ctx.enter_context(tc.tile_pool(name="trans", bufs=3))
        upool = ctx.enter_context(
            tc.tile_pool(name="upool", bufs=max(3, B_LAG + 1, C_LAG - B_LAG + 2)))
        psum = ctx.enter_context(
            tc.tile_pool(name="psum", bufs=1, space=bass_mod.MemorySpace.PSUM))
        psum2 = ctx.enter_context(
            tc.tile_pool(name="psum2", bufs=(1 if CB_FULL else 2),
                         space=bass_mod.MemorySpace.PSUM))

        f32, bf16 = dt.float32, dt.bfloat16
        TT = nc.vector.tensor_tensor

        # ---- loads (images/labels arrive pre-cast to bf16 from host) ----
        # The DMA engines serialize transfers, so order by first use: mats
        # (PE idles until it lands), labf (masks), imgb per-channel, then the
        # big f32 logits tensor (only needed once the Act exp work starts).
        mats = persist.tile([P, 6, P], bf16, tag="mats")
        (nc.scalar if T_DMAQ else nc.sync).dma_start(mats[:], eye_d[:, :])
        labf = persist.tile([P, DE, WE], bf16, tag="labf")
        imgb = persist.tile([P, C, DE, WE], bf16, tag="imgb")
        for c in range(C):
            nc.sync.dma_start(imgb[:, c],
                              img_d[:, c * DE * WE:(c + 1) * DE * WE])
        (nc.scalar if T_DMAQ else nc.sync).dma_start(labf[:], lab_d[:, :])
        x = persist.tile([P, C, DL, W], bf16, tag="x")
        nc.sync.dma_start(x[:], logit_d[:, :])

        negI = mats[:, 0]
        shm = {1: mats[:, 1], -1: mats[:, 2]}
        ir2 = {1.0: mats[:, 3], 2.0: mats[:, 4], 3.0: mats[:, 5]}

        masks = persist.tile([P, 3, DE, WE], bf16, tag="masks")

        def emit_masks():
            eng = nc.vector if T_MASKS_DVE else (
                nc.gpsimd if USE_POOLOPS else nc.vector)
            for ci, cval in enumerate((1.0, 2.0, 3.0)):
                eng.tensor_scalar(masks[:, ci], labf[:], cval, None,
                                  ALU.is_equal)

        # ---- h-shifted copies (partition shift via SBUF->SBUF DMA).
        def hshift_copies(dst_p, dst_m, src, eng):
            eng.dma_start(dst_p[0:63], src[1:64])
            eng.dma_start(dst_p[64:127], src[65:128])
            eng.dma_start(dst_p[63:64], src[63:64])
            eng.dma_start(dst_p[127:128], src[127:128])
            eng.dma_start(dst_m[1:64], src[0:63])
            eng.dma_start(dst_m[65:128], src[64:127])
            eng.dma_start(dst_m[0:1], src[0:1])
            eng.dma_start(dst_m[64:65], src[64:65])

        # masks_h before imgb_h: first mask-product use is much earlier than
        # the first DVE-path min-frame. SP ring so Act SEQ never blocks.
        masks_hp = persist.tile([P, 3, DE, WE], bf16, tag="masks_hp")
        masks_hm = persist.tile([P, 3, DE, WE], bf16, tag="masks_hm")
        msk_h = {1: masks_hp, 0: masks, -1: masks_hm}
        need_imgb_h = len(MIN_ON_PE) < 9 or len(T_MAXDVE) > 0
        if need_imgb_h:
            imgb_hp = persist.tile([P, C, DE, WE], bf16, tag="imgb_hp")
            imgb_hm = persist.tile([P, C, DE, WE], bf16, tag="imgb_hm")
            img_h = {1: imgb_hp, 0: imgb, -1: imgb_hm}

        def emit_hshifts():
            hshift_copies(masks_hp, masks_hm, masks, nc.sync)
            if need_imgb_h:
                hshift_copies(imgb_hp, imgb_hm, imgb, nc.sync)

        def cv(tile_, i, k):
            """center view shifted by (i, ., k) of a [..., DE, WE] tile."""
            return tile_[:, :, 1 + i:1 + i + DL, 1 + k:1 + k + W]

        # ---- PSUM accumulators; PE accumulates via r2-scaled identities ----
        accP = psum.tile([P, 3, DL, W], f32, tag="accP")
        SP = psum.tile([P, DL, W], f32, tag="SP")

        pl = cpool.tile([P, 4], f32, tag="pl")
        scr1 = cpool.tile([P, DL, W], f32, tag="scr1")
        scr2 = cpool.tile([P, DL, W], f32, tag="scr2")
        scr3 = cpool.tile([P, DL, W], f32, tag="scr3")

        cph = {}

        def emit_cphase_act():
            # exp-set work up front while PE/DVE wind up; p_x0 on the side
            if USE_XB:
                xb = cpool.tile([P, C, DL, W], bf16, tag="xb")
                nc.scalar.activation(xb[:], x[:], AF.Copy, scale=-0.5)
                cph.update(xb=xb)
            expx = cpool.tile([P, C, DL, W], bf16, tag="expx")
            nc.scalar.activation(expx[:], x[:], AF.Exp)
            if T_X0DVE:
                nc.vector.tensor_scalar(scr1[:], x[:, 0], 1.0, None,
                                        ALU.mult, accum_out=pl[:, 3:4])
            else:
                nc.scalar.activation(scr1[:], x[:, 0], AF.Copy,
                                     accum_out=pl[:, 3:4])
            cph.update(expx=expx)

        def emit_expsum():
            expx = cph['expx']
            e2 = cpool.tile([P, 2, DL, W], bf16, tag="e2")
            ene = nc.gpsimd if T_EXPSUM_POOL else nc.vector
            ene.tensor_tensor(e2[:], expx[:, 0:2], expx[:, 2:4], ALU.add)
            esum = cpool.tile([P, DL, W], bf16, tag="esum")
            ene.tensor_tensor(esum[:], e2[:, 0], e2[:, 1], ALU.add)
            cph.update(esum=esum)

        def emit_cphase_front():
            # DVE part: dxb; fused p_ym = sum(m_c * dx_c)
            if not EXP_LATE:
                emit_expsum()
            xs = cph['xb'] if USE_XB else x
            dxb = cpool.tile([P, 3, DL, W], bf16, tag="dxb")
            TT(dxb[:], xs[:, 1:4], xs[:, 0:1].broadcast_to((P, 3, DL, W)),
               ALU.subtract)
            # p_ym = sum over (c,d,w) of m_c*dx_c: one stt dot with accum
            # (needs a contiguous mask-center copy; Pool makes it for free)
            mc = cpool.tile([P, 3, DL, W], bf16, tag="mc")
            nc.gpsimd.tensor_copy(mc[:], cv(masks, 0, 0))
            ym = cpool.tile([P, 3, DL, W], bf16, tag="ym")
            yme = nc.gpsimd if T_YMPOOL else nc.vector
            yme.scalar_tensor_tensor(ym[:], mc[:], 1.0, dxb[:],
                                     ALU.mult, ALU.mult,
                                     accum_out=pl[:, 1:2])
            cph.update(dxb=dxb)
            if LSE_EARLY:
                emit_cphase_back()

        def emit_cphase_back():
            # p_lse: one act-table switch back to the ln/exp set
            if EXP_LATE:
                emit_cphase_act()
                emit_expsum()
            nc.scalar.activation(scr2[:], cph['esum'], AF.Ln,
                                 accum_out=pl[:, 0:1])

        def pe_frame(jj, ii, kk, mop, m1p_slot):
            """d4 = Sh_jj.T@view(ii,kk) - center on PE; Act copyback halves;
            DVE tree into m1p_slot."""
            cb = trans.tile([P, C, DL, W], bf16, tag="cb")
            if CB_FULL:
                d4p = psum2.tile([P, C, DL, W], f32, tag="d4p")
                for c in range(C):
                    nc.tensor.matmul(d4p[:, c], shm[jj],
                                     imgb[:, c, 1 + ii:1 + ii + DL,
                                          1 + kk:1 + kk + W],
                                     start=True, stop=False)
                    nc.tensor.matmul(d4p[:, c], negI,
                                     imgb[:, c, 1:1 + DL, 1:1 + W],
                                     start=False, stop=True)
                nc.scalar.copy(cb[:], d4p[:])
            else:
                for half in range(2):
                    d4p = psum2.tile([P, 2, DL, W], f32, tag="d4p")
                    for cc in range(2):
                        c = 2 * half + cc
                        nc.tensor.matmul(d4p[:, cc], shm[jj],
                                         imgb[:, c, 1 + ii:1 + ii + DL,
                                              1 + kk:1 + kk + W],
                                         start=True, stop=False)
                        nc.tensor.matmul(d4p[:, cc], negI,
                                         imgb[:, c, 1:1 + DL, 1:1 + W],
                                         start=False, stop=True)
                    nc.scalar.copy(cb[:, 2 * half:2 * half + 2], d4p[:])
            m2 = trans.tile([P, 2, DL, W], bf16, tag="m2")
            TT(m2[:], cb[:, 0:2], cb[:, 2:4], mop)
            TT(m1p_slot, m2[:, 0], m2[:, 1], mop)

        # ---- software-pipelined main loop over offset pairs ----
        m1p_t, up_t, prods_t = {}, {}, {}
        P_LAG = max(B_LAG, C_LAG - T_PSPLIT)

        def stage_A(pi):
            i, j, k = PAIRS[pi]
            m1p = upool.tile([P, 2, DL, W], bf16, tag="m1p")
            m1p_t[pi] = m1p
            if j == 0:
                # single sub on an extended box serves both frames as views
                nd, nw = (9 if i else 8), (65 if k else 64)
                d0, w0 = (0 if i == 1 else 1), (0 if k == 1 else 1)
                dpe = trans.tile([P, C, nd, nw], bf16, tag="dpe")
                if pi == PAIR_ORDER[0]:
                    for ch in range(0, C, 2):
                        TT(dpe[:, ch:ch + 2],
                           imgb[:, ch:ch + 2, d0 + i:d0 + i + nd,
                                w0 + k:w0 + k + nw],
                           imgb[:, ch:ch + 2, d0:d0 + nd, w0:w0 + nw],
                           ALU.subtract)
                else:
                    TT(dpe[:],
                       imgb[:, :, d0 + i:d0 + i + nd, w0 + k:w0 + k + nw],
                       imgb[:, :, d0:d0 + nd, w0:w0 + nw], ALU.subtract)
                for fr in range(2):
                    ds = 1 - d0 - (i if fr else 0)
                    ws = 1 - w0 - (k if fr else 0)
                    mop = ALU.max if fr == 0 else ALU.min
                    dv = dpe[:, :, ds:ds + DL, ws:ws + W]
                    m2 = trans.tile([P, 2, DL, W], bf16, tag="m2")
                    TT(m2[:], dv[:, 0:2], dv[:, 2:4], mop)
                    TT(m1p[:, fr], m2[:, 0], m2[:, 1], mop)
            elif pi in T_MAXDVE:
                d4x = trans.tile([P, C, DL, W], bf16, tag="d4x", name="d4x")
                TT(d4x[:], cv(img_h[j], i, k), cv(imgb, 0, 0), ALU.subtract)
                m2x = trans.tile([P, 2, DL, W], bf16, tag="m2x", name="m2x")
                TT(m2x[:], d4x[:, 0:2], d4x[:, 2:4], ALU.max)
                TT(m1p[:, 0], m2x[:, 0], m2x[:, 1], ALU.max)
                if pi in MIN_ON_PE:
                    pe_frame(-j, -i, -k, ALU.min, m1p[:, 1])
                else:
                    d4 = trans.tile([P, C, DL, W], bf16, tag="d4")
                    TT(d4[:], cv(imgb, 0, 0), cv(img_h[-j], -i, -k),
                       ALU.subtract)
                    m2n = trans.tile([P, 2, DL, W], bf16, tag="m2n")
                    TT(m2n[:], d4[:, 0:2], d4[:, 2:4], ALU.min)
                    TT(m1p[:, 1], m2n[:, 0], m2n[:, 1], ALU.min)
            else:
                pe_frame(j, i, k, ALU.max, m1p[:, 0])
                if pi in MIN_ON_PE:
                    # sign-free: min tree of Sh_-j view(-i,-k) - center
                    pe_frame(-j, -i, -k, ALU.min, m1p[:, 1])
                else:
                    d4 = trans.tile([P, C, DL, W], bf16, tag="d4")
                    TT(d4[:], cv(imgb, 0, 0), cv(img_h[-j], -i, -k),
                       ALU.subtract)
                    m2n = trans.tile([P, 2, DL, W], bf16, tag="m2n")
                    TT(m2n[:], d4[:, 0:2], d4[:, 2:4], ALU.min)
                    TT(m1p[:, 1], m2n[:, 0], m2n[:, 1], ALU.min)

        def stage_B(pi):
            # u for both frames in one activation (const 2/sqrt(pi) cancels;
            # exp(-r2/2) lives in the scaled identity used by the acc matmuls)
            up = upool.tile([P, 2, DL, W], bf16, tag="up")
            up_t[pi] = up
            m1p = m1p_t.pop(pi)
            if USE_DERF:
                nc.scalar.activation(up[:], m1p[:], AF.Derivative_Erf,
                                     scale=SQ2I)
            else:
                sqp = trans.tile([P, 2, DL, W], bf16, tag="sqp")
                nc.scalar.activation(sqp[:], m1p[:], AF.Square)
                nc.scalar.activation(up[:], sqp[:], AF.Exp, scale=-0.5)

        def stage_P(pi):
            i, j, k = PAIRS[pi]
            up = up_t[pi]
            pr2 = trans.tile([P, 2, 3, DL, W], bf16, tag="prods",
                             name="pr2", bufs=C_LAG + 2)
            prods_t[pi] = pr2
            for fr, sgn in ((0, 1), (1, -1)):
                si, sj, sk = sgn * i, sgn * j, sgn * k
                mview = cv(msk_h[sj], si, sk)
                ub = up[:, fr:fr + 1].broadcast_to((P, 3, DL, W))
                eng = nc.gpsimd if (pi, fr) in T_PRODS_POOL else nc.vector
                eng.tensor_tensor(pr2[:, fr], ub, mview, ALU.mult)

        def stage_C(pi):
            i, j, k = PAIRS[pi]
            r2 = float(i * i + j * j + k * k)
            st, sp = (pi == PAIR_ORDER[0]), (pi == PAIR_ORDER[-1])
            up = up_t.pop(pi)
            pr2 = prods_t.pop(pi)
            for fr in range(2):
                nc.tensor.matmul(SP[:], ir2[r2], up[:, fr],
                                 start=(st and fr == 0), stop=(sp and fr == 1))
                for ci in range(3):
                    nc.tensor.matmul(accP[:, ci], ir2[r2], pr2[:, fr, ci],
                                     start=(st and fr == 0),
                                     stop=(sp and fr == 1))

        NP = len(PAIR_ORDER)
        for idx in range(NP + max(B_LAG, C_LAG)):
            if idx == CPHASE_AT:
                emit_cphase_front()
            if idx == 0:
                emit_masks()
            if idx < NP:
                stage_A(PAIR_ORDER[idx])
            if idx == 0:
                emit_hshifts()
            if B_LAG <= idx < NP + B_LAG:
                stage_B(PAIR_ORDER[idx - B_LAG])
            if P_LAG <= idx < NP + P_LAG:
                stage_P(PAIR_ORDER[idx - P_LAG])
            if C_LAG <= idx < NP + C_LAG:
                stage_C(PAIR_ORDER[idx - C_LAG])
            if idx == CACT_AT and not EXP_LATE:
                emit_cphase_act()
        if CPHASE_AT >= NP:
            emit_cphase_front()
        if not LSE_EARLY:
            emit_cphase_back()

        # ---- tail: p_w = sum(-0.5/S * sum_c dx_c*acc_c) ----
        dxb = cph['dxb']
        rS = cpool.tile([P, DL, W], f32, tag="rS")
        nc.vector.reciprocal_approx_fast(rS[:], SP[:])
        tp = cpool.tile([P, 3, DL, W], bf16, tag="tp")
        if T_TAILSPLIT:
            for ci in range(3):
                TT(tp[:, ci], accP[:, ci], dxb[:, ci], ALU.mult)
        else:
            TT(tp[:], accP[:], dxb[:], ALU.mult)
        t1 = cpool.tile([P, DL, W], bf16, tag="t1")
        TT(t1[:], tp[:, 0], tp[:, 1], ALU.add)
        t2 = cpool.tile([P, DL, W], bf16, tag="t2")
        TT(t2[:], t1[:], tp[:, 2], ALU.add)
        nc.vector.scalar_tensor_tensor(scr3[:], t2[:], -0.5, rS[:],
                                       ALU.mult, ALU.mult,
                                       accum_out=pl[:, 2:3])
        nc.sync.dma_start(out_d[:, :], pl[:])

    nc.compile()
    return nc


def _get_nc():
    if "nc" not in _CACHED:
        _CACHED["nc"] = _build_nc()
    return _CACHED["nc"]


def make_in_maps(inputs, labels, images):
    """Host-side shard: full inputs -> per-core input dicts (layout prep:
    (b,h)->partition transpose, d/w halo padding, bf16 pre-cast)."""
    import ml_dtypes

    bf = ml_dtypes.bfloat16
    img = np.asarray(images, np.float32).astype(bf)
    lab = np.asarray(labels).astype(bf)  # values 0..3, exact in bf16
    lgt = np.ascontiguousarray(np.asarray(inputs, np.float32))

    img_p = np.pad(img, ((0, 0), (0, 0), (1, 1), (0, 0), (1, 1)), mode="edge")
    lab_p = np.pad(lab, ((0, 0), (1, 1), (0, 0), (1, 1)), mode="edge")

    in_maps = []
    for k in range(NCORES):
        d0 = k * DL
        ic = img_p[:, :, d0:d0 + DE]          # [2,4,10,64,66]
        lc = lab_p[:, d0:d0 + DE]             # [2,10,64,66]
        xc = lgt[:, :, d0:d0 + DL]            # [2,4,8,64,64]
        im = np.ascontiguousarray(ic.transpose(0, 3, 1, 2, 4)).reshape(P, -1)
        lm = np.ascontiguousarray(lc.transpose(0, 2, 1, 3)).reshape(P, -1)
        xm = np.ascontiguousarray(
            xc.transpose(0, 3, 1, 2, 4)).reshape(P, -1).astype(bf)
        in_maps.append({"img": im, "lab": lm, "logits": xm, "eye": _mats()})
    return in_maps


def _mats():
    """[-I, Sh(+1), Sh(-1), e^-.5 I, e^-1 I, e^-1.5 I] as one [P, 6P] bf16
    array. Sh(j)[k, m] = 1 iff k = b(m)*64 + clamp(h(m)+j, 0, 63)."""
    import ml_dtypes

    eye = np.eye(P, dtype=np.float32)
    sh = {}
    for jj in (1, -1):
        M = np.zeros((P, P), np.float32)
        for m in range(P):
            b, h = divmod(m, 64)
            M[b * 64 + min(max(h + jj, 0), 63), m] = 1.0
        sh[jj] = M
    blocks = [-eye, sh[1], sh[-1],
              np.exp(-0.5) * eye, np.exp(-1.0) * eye, np.exp(-1.5) * eye]
    out = np.concatenate(blocks, axis=1)
    return np.ascontiguousarray(out).astype(ml_dtypes.bfloat16)


def kernel(inputs, labels, images):
    from concourse.bass_utils import run_bass_kernel_spmd

    nc = _get_nc()
    in_maps = make_in_maps(inputs, labels, images)
    res = run_bass_kernel_spmd(nc, in_maps, core_ids=list(range(NCORES)))
    total = 0.0
    for k in range(NCORES):
        pl = res.results[k]["partials"].astype(np.float64)
        ym_scale = -1.0 if USE_XB else 0.5
        total += (pl[:, 0] - ym_scale * pl[:, 1] - pl[:, 3] + pl[:, 2]).sum()
    return np.float32(total / NVOX)



# revision 6
# speedup vs baseline: 1.0833x; 1.0429x over previous
"""Trainium2 Bass kernel for nn_CELossWithSVLS_VE (SVLS cross-entropy loss).

Math (derived + numerically validated vs reference):
  For the 26 non-center offsets n, with per-voxel
    u_n = exp(-0.5*(maxdiff_n^2 + r_n^2)),
    maxdiff_n(v) = max_c(img_c(v+n) - img_c(v))   (replicate-padded),
  the SVLS label weights reduce EXACTLY to w_center = 1/2, w_n = u_n/(2S),
  S = sum_n u_n.  Then
    loss(v) = lse(v) - 0.5*x_{l(v)}(v) - (1/(2S)) * sum_n u_n * x_{l(v+n)}(v)
  and the output is mean_v loss(v).

Engine plan (vs the 93.7us baseline):
  * u_n via ONE ScalarE activation: Derivative_Erf(m/sqrt2) = c*exp(-m^2/2);
    c cancels in T/S, and exp(-r2/2) moves into r2-scaled identity stationary
    matrices used by the PE accumulation matmuls (no bias/second activation).
  * most 4-channel stencil subtractions run on the PE as shift-matrix matmul
    pairs into PSUM; ScalarE copies PSUM->SBUF bf16 (the only engine that can
    get PSUM data back cheaply); DVE does only max/min trees + mask products.
  * loss folds into 3 per-partition accumulators (p_lse, p_yx, p_w) via
    accum_out side outputs; host combines  sum = p_lse - 0.5*p_yx + p_w.
    The T-dot reads accP straight out of PSUM (single-PSUM-operand TT).

Sharding: 8 cores, core k takes d-slab [8k, 8k+8) of both batches.
On-core layout: partition p = b*64 + h (128), free = (c?, d, w) with d,w
halos in SBUF.  h+-1 stencil shifts: PE shift-matrix matmuls (edge clamp
baked in) or partition-shifted SBUF DMA copies for the DVE-path frames.
"""
import sys
from contextlib import ExitStack

import numpy as np

if "/opt/trn_rl_repo" not in sys.path:
    sys.path.insert(0, "/opt/trn_rl_repo")

B, C, D, H, W = 2, 4, 64, 64, 64
NCORES = 8
DL = D // NCORES          # 8 local d-planes
DE, WE = DL + 2, W + 2    # 10, 66 (d/w halos)
P = 128                   # partitions = (b, h)
NVOX = B * D * H * W      # 524288

# 13 positive offsets; r2 = i*i+j*j+k*k.
PAIRS = [
    (1, 0, 0), (0, 0, 1), (1, 0, 1), (1, 0, -1),
    (0, 1, 0), (1, 1, 0), (1, -1, 0), (0, 1, 1), (0, 1, -1),
    (1, 1, 1), (1, 1, -1), (1, -1, 1), (1, -1, -1),
]

import os as _os, json as _json
_ov = _json.loads(_os.environ.get("KCONF", "{}"))
T_J0POOL = set(_ov.get("j0pool", []))     # j0 pairs: m2 on Pool
T_MINPOOL = set(_ov.get("minpool", []))   # DVE-min-path pairs: m2n on Pool
T_MASKS_DVE = _ov.get("masks_dve", 0)
T_EXPSUM_POOL = _ov.get("expsum_pool", 0)
T_PRODS_POOL = {tuple(t) for t in _ov.get("prods_pool",
                                           [[4, 1], [10, 1], [12, 1],
                                            [6, 1], [5, 1], [11, 1],
                                            [1, 1]])}
T_PSPLIT = _ov.get("psplit", 1)           # prods stage one slot early
T_MAXDVE = set(_ov.get("maxdve", [8, 9]))  # pairs: max-frame off PE
T_MINOFF = set(_ov.get("minoff", []))     # pairs removed from MIN_ON_PE
T_X0DVE = _ov.get("x0dve", 0)             # p_x0 accum via DVE tensor_scalar
T_YMPOOL = _ov.get("ympool", 0)           # ym STT on Pool (NO: won't compile)
T_YMSPLIT = _ov.get("ymsplit", 0)         # ym: Pool mult + DVE 4x ts-accum
T_TAILSPLIT = _ov.get("tailsplit", 0)     # per-channel tail tp
T_DMAQ = _ov.get("dmaq", 0)               # mats/labf on Act DGE queue
T_DMAORD = _ov.get("dmaord", 0)           # 1: labf,img,mats,x  2: img01 first
T_TREEBATCH = _ov.get("treebatch", 0)     # pair-batched trees on PE pairs
T_EXPTOKEN = _ov.get("exptoken", 1)       # gate exp/ln after last DErf
T_PRODS_SPLIT = {tuple(t) for t in _ov.get("prods_split",
                                            [[7, 0], [8, 0], [9, 0],
                                             [2, 0], [3, 0]])}
# ---- schedule config ----
USE_DERF = True
# j!=0 pairs whose min-frame runs on PE (rest: DVE sub via h-shifted copies)
MIN_ON_PE = {10, 11, 12, 4, 5, 6, 7} - T_MINOFF
# emission order (j0 pairs interleaved between PE-heavy pairs)
PAIR_ORDER = _ov.get("order", [0, 4, 5, 1, 10, 11, 2, 12, 6, 3, 7, 8, 9])
CPHASE_AT = _ov.get("cphase", 5)
CACT_AT = _ov.get("cact", 3)
LSE_EARLY = False
EXP_LATE = False  # overridden to True by T_EXPTOKEN at build time
USE_XB = False
USE_RSB = False
TTR_YM = False    # tensor_tensor_reduce compiles but faults at runtime
TTR_TAIL = False
USE_POOLOPS = True
CB_FULL = False
B_LAG = _ov.get("blag", 3)
C_LAG = _ov.get("clag", 3)

_CACHED = {}

SQ2I = 0.7071067811865476  # 1/sqrt(2)


def _build_nc():
    import concourse.bacc as bacc
    import concourse.mybir as mybir
    import concourse.tile as tile

    AF = mybir.ActivationFunctionType
    ALU = mybir.AluOpType
    dt = mybir.dt

    nc = bacc.Bacc("TRN2", target_bir_lowering=False, debug=False,
                   num_devices=NCORES)
    img_d = nc.dram_tensor("img", [P, C * DE * WE], dt.bfloat16,
                           kind="ExternalInput")
    lab_d = nc.dram_tensor("lab", [P, DE * WE], dt.bfloat16,
                           kind="ExternalInput")
    logit_d = nc.dram_tensor("logits", [P, C * DL * W], dt.bfloat16,
                             kind="ExternalInput")
    # mats: [-I, Sh(+1), Sh(-1), I*e^-.5, I*e^-1, I*e^-1.5,
    eye_d = nc.dram_tensor("eye", [P, (9 if T_TREEBATCH else 6) * P],
                           dt.bfloat16, kind="ExternalInput")
    out_d = nc.dram_tensor("partials", [P, 4], dt.float32,
                           kind="ExternalOutput")

    import concourse.bass as bass_mod

    exp_late = EXP_LATE or bool(T_EXPTOKEN)
    with tile.TileContext(nc) as tc, ExitStack() as ctx:
        persist = ctx.enter_context(tc.tile_pool(name="persist", bufs=1))
        cpool = ctx.enter_context(tc.tile_pool(name="cpool", bufs=1))
        trans = ctx.enter_context(tc.tile_pool(name="trans", bufs=3))
        upool = ctx.enter_context(
            tc.tile_pool(name="upool", bufs=max(3, B_LAG + 1, C_LAG - B_LAG + 2)))
        psum = ctx.enter_context(
            tc.tile_pool(name="psum", bufs=1, space=bass_mod.MemorySpace.PSUM))
        psum2 = ctx.enter_context(
            tc.tile_pool(name="psum2", bufs=(1 if CB_FULL else 2),
                         space=bass_mod.MemorySpace.PSUM))

        f32, bf16 = dt.float32, dt.bfloat16
        TT = nc.vector.tensor_tensor

        # ---- loads (images/labels arrive pre-cast to bf16 from host) ----
        # The DMA engines serialize transfers, so order by first use: mats
        # (PE idles until it lands), labf (masks), imgb per-channel, then the
        # big f32 logits tensor (only needed once the Act exp work starts).
        mats = persist.tile([P, 9 if T_TREEBATCH else 6, P], bf16,
                            tag="mats")
        labf = persist.tile([P, DE, WE], bf16, tag="labf")
        imgb = persist.tile([P, C, DE, WE], bf16, tag="imgb")
        x = persist.tile([P, C, DL, W], bf16, tag="x")

        def dma_img(c0, c1):
            for c in range(c0, c1):
                nc.sync.dma_start(imgb[:, c],
                                  img_d[:, c * DE * WE:(c + 1) * DE * WE])

        if T_DMAORD == 0:
            (nc.scalar if T_DMAQ else nc.sync).dma_start(mats[:], eye_d[:, :])
            dma_img(0, C)
            (nc.scalar if T_DMAQ else nc.sync).dma_start(labf[:], lab_d[:, :])
            nc.sync.dma_start(x[:], logit_d[:, :])
        elif T_DMAORD == 1:
            nc.sync.dma_start(labf[:], lab_d[:, :])
            dma_img(0, C)
            nc.sync.dma_start(mats[:], eye_d[:, :])
            nc.sync.dma_start(x[:], logit_d[:, :])
        else:
            dma_img(0, 2)
            nc.sync.dma_start(labf[:], lab_d[:, :])
            dma_img(2, C)
            nc.sync.dma_start(mats[:], eye_d[:, :])
            nc.sync.dma_start(x[:], logit_d[:, :])

        negI = mats[:, 0]
        shm = {1: mats[:, 1], -1: mats[:, 2]}
        ir2 = {1.0: mats[:, 3], 2.0: mats[:, 4], 3.0: mats[:, 5]}
        if T_TREEBATCH:
            nshm = {1: mats[:, 6], -1: mats[:, 7]}
            posI = mats[:, 8]

        masks = persist.tile([P, 3, DE, WE], bf16, tag="masks")

        def emit_masks():
            eng = nc.vector if T_MASKS_DVE else (
                nc.gpsimd if USE_POOLOPS else nc.vector)
            for ci, cval in enumerate((1.0, 2.0, 3.0)):
                eng.tensor_scalar(masks[:, ci], labf[:], cval, None,
                                  ALU.is_equal)

        # ---- h-shifted copies (partition shift via SBUF->SBUF DMA).
        def hshift_copies(dst_p, dst_m, src, eng):
            eng.dma_start(dst_p[0:63], src[1:64])
            eng.dma_start(dst_p[64:127], src[65:128])
            eng.dma_start(dst_p[63:64], src[63:64])
            eng.dma_start(dst_p[127:128], src[127:128])
            eng.dma_start(dst_m[1:64], src[0:63])
            eng.dma_start(dst_m[65:128], src[64:127])
            eng.dma_start(dst_m[0:1], src[0:1])
            eng.dma_start(dst_m[64:65], src[64:65])

        # masks_h before imgb_h: first mask-product use is much earlier than
        # the first DVE-path min-frame. SP ring so Act SEQ never blocks.
        masks_hp = persist.tile([P, 3, DE, WE], bf16, tag="masks_hp")
        masks_hm = persist.tile([P, 3, DE, WE], bf16, tag="masks_hm")
        msk_h = {1: masks_hp, 0: masks, -1: masks_hm}
        need_imgb_h = len(MIN_ON_PE) < 9 or len(T_MAXDVE) > 0
        if need_imgb_h:
            imgb_hp = persist.tile([P, C, DE, WE], bf16, tag="imgb_hp")
            imgb_hm = persist.tile([P, C, DE, WE], bf16, tag="imgb_hm")
            img_h = {1: imgb_hp, 0: imgb, -1: imgb_hm}

        def emit_hshifts():
            hshift_copies(masks_hp, masks_hm, masks, nc.sync)
            if need_imgb_h:
                hshift_copies(imgb_hp, imgb_hm, imgb, nc.sync)

        def cv(tile_, i, k):
            """center view shifted by (i, ., k) of a [..., DE, WE] tile."""
            return tile_[:, :, 1 + i:1 + i + DL, 1 + k:1 + k + W]

        # ---- PSUM accumulators; PE accumulates via r2-scaled identities ----
        accP = psum.tile([P, 3, DL, W], f32, tag="accP")
        SP = psum.tile([P, DL, W], f32, tag="SP")

        pl = cpool.tile([P, 4], f32, tag="pl")
        scr1 = cpool.tile([P, DL, W], f32, tag="scr1")
        scr2 = cpool.tile([P, DL, W], f32, tag="scr2")
        scr3 = cpool.tile([P, DL, W], f32, tag="scr3")

        cph = {}

        def emit_cphase_act():
            # exp-set work up front while PE/DVE wind up; p_x0 on the side
            if T_EXPTOKEN:
                # 1-element in-place bypass on x gated by the last pair's
                # DErf output: orders every x-reader emitted below (exp,
                # x0-accum) after the whole DErf block, so the act table
                # never leaves set 17 mid-stream (the readiness scheduler
                # would otherwise hoist exp into the DErf stream).
                TT(x[:, 0:1, 0:1, 0:1], x[:, 0:1, 0:1, 0:1],
                   cph['gate'][:, 0:1, 0:1, 0:1], ALU.bypass)
            if USE_XB:
                xb = cpool.tile([P, C, DL, W], bf16, tag="xb")
                nc.scalar.activation(xb[:], x[:], AF.Copy, scale=-0.5)
                cph.update(xb=xb)
            expx = cpool.tile([P, C, DL, W], bf16, tag="expx")
            nc.scalar.activation(expx[:], x[:], AF.Exp)
            if T_X0DVE:
                nc.vector.tensor_scalar(scr1[:], x[:, 0], 1.0, None,
                                        ALU.mult, accum_out=pl[:, 3:4])
            else:
                nc.scalar.activation(scr1[:], x[:, 0], AF.Copy,
                                     accum_out=pl[:, 3:4])
            cph.update(expx=expx)

        def emit_expsum():
            expx = cph['expx']
            e2 = cpool.tile([P, 2, DL, W], bf16, tag="e2")
            ene = nc.gpsimd if T_EXPSUM_POOL else nc.vector
            ene.tensor_tensor(e2[:], expx[:, 0:2], expx[:, 2:4], ALU.add)
            esum = cpool.tile([P, DL, W], bf16, tag="esum")
            ene.tensor_tensor(esum[:], e2[:, 0], e2[:, 1], ALU.add)
            cph.update(esum=esum)

        def emit_cphase_front():
            # DVE part: dxb; fused p_ym = sum(m_c * dx_c)
            if not exp_late:
                emit_expsum()
            xs = cph['xb'] if USE_XB else x
            dxb = cpool.tile([P, 3, DL, W], bf16, tag="dxb")
            TT(dxb[:], xs[:, 1:4], xs[:, 0:1].broadcast_to((P, 3, DL, W)),
               ALU.subtract)
            # p_ym = sum over (c,d,w) of m_c*dx_c: one stt dot with accum
            # (needs a contiguous mask-center copy; Pool makes it for free)
            mc = cpool.tile([P, 3, DL, W], bf16, tag="mc")
            nc.gpsimd.tensor_copy(mc[:], cv(masks, 0, 0))
            ym = cpool.tile([P, 3, DL, W], bf16, tag="ym")
            if T_YMSPLIT:
                nc.gpsimd.tensor_tensor(ym[:], mc[:], dxb[:], ALU.mult)
                scry = cpool.tile([P, 3, DL, W], bf16, tag="scry")
                nc.vector.tensor_scalar(scry[:], ym[:], 1.0, None,
                                        ALU.mult, accum_out=pl[:, 1:2])
            else:
                yme = nc.gpsimd if T_YMPOOL else nc.vector
                yme.scalar_tensor_tensor(ym[:], mc[:], 1.0, dxb[:],
                                         ALU.mult, ALU.mult,
                                         accum_out=pl[:, 1:2])
            cph.update(dxb=dxb)
            if LSE_EARLY:
                emit_cphase_back()

        def emit_cphase_back():
            # p_lse: one act-table switch back to the ln/exp set
            if exp_late:
                emit_cphase_act()
                emit_expsum()
            nc.scalar.activation(scr2[:], cph['esum'], AF.Ln,
                                 accum_out=pl[:, 0:1])

        def pe_frame_pair(i, j, k, m1p):
            """Both frames of a PE pair with batched trees: the min-frame
            uses negated stationaries (-Sh, +I) so BOTH trees are max-trees
            (Derivative_Erf is even), letting m2/m1p batch across frames."""
            cb2 = trans.tile([P, 2, C, DL, W], bf16, tag="cb2", name="cb2")
            for fr, sgn in ((0, 1), (1, -1)):
                si, sk = sgn * i, sgn * k
                st1 = shm[sgn * j] if fr == 0 else nshm[sgn * j]
                st2 = negI if fr == 0 else posI
                for half in range(2):
                    d4p = psum2.tile([P, 2, DL, W], f32, tag="d4p",
                                     name="d4p")
                    for cc in range(2):
                        c = 2 * half + cc
                        nc.tensor.matmul(d4p[:, cc], st1,
                                         imgb[:, c, 1 + si:1 + si + DL,
                                              1 + sk:1 + sk + W],
                                         start=True, stop=False)
                        nc.tensor.matmul(d4p[:, cc], st2,
                                         imgb[:, c, 1:1 + DL, 1:1 + W],
                                         start=False, stop=True)
                    nc.scalar.copy(cb2[:, fr, 2 * half:2 * half + 2],
                                   d4p[:])
            m2b = trans.tile([P, 2, 2, DL, W], bf16, tag="m2b", name="m2b")
            TT(m2b[:], cb2[:, :, 0:2], cb2[:, :, 2:4], ALU.max)
            TT(m1p[:], m2b[:, :, 0], m2b[:, :, 1], ALU.max)

        def pe_frame(jj, ii, kk, mop, m1p_slot):
            """d4 = Sh_jj.T@view(ii,kk) - center on PE; Act copyback halves;
            DVE tree into m1p_slot."""
            cb = trans.tile([P, C, DL, W], bf16, tag="cb")
            if CB_FULL:
                d4p = psum2.tile([P, C, DL, W], f32, tag="d4p")
                for c in range(C):
                    nc.tensor.matmul(d4p[:, c], shm[jj],
                                     imgb[:, c, 1 + ii:1 + ii + DL,
                                          1 + kk:1 + kk + W],
                                     start=True, stop=False)
                    nc.tensor.matmul(d4p[:, c], negI,
                                     imgb[:, c, 1:1 + DL, 1:1 + W],
                                     start=False, stop=True)
                nc.scalar.copy(cb[:], d4p[:])
            else:
                for half in range(2):
                    d4p = psum2.tile([P, 2, DL, W], f32, tag="d4p")
                    for cc in range(2):
                        c = 2 * half + cc
                        nc.tensor.matmul(d4p[:, cc], shm[jj],
                                         imgb[:, c, 1 + ii:1 + ii + DL,
                                              1 + kk:1 + kk + W],
                                         start=True, stop=False)
                        nc.tensor.matmul(d4p[:, cc], negI,
                                         imgb[:, c, 1:1 + DL, 1:1 + W],
                                         start=False, stop=True)
                    nc.scalar.copy(cb[:, 2 * half:2 * half + 2], d4p[:])
            m2 = trans.tile([P, 2, DL, W], bf16, tag="m2")
            TT(m2[:], cb[:, 0:2], cb[:, 2:4], mop)
            TT(m1p_slot, m2[:, 0], m2[:, 1], mop)

        # ---- software-pipelined main loop over offset pairs ----
        m1p_t, up_t, prods_t = {}, {}, {}
        P_LAG = max(B_LAG, C_LAG - T_PSPLIT)

        def stage_A(pi):
            i, j, k = PAIRS[pi]
            m1p = upool.tile([P, 2, DL, W], bf16, tag="m1p")
            m1p_t[pi] = m1p
            if j == 0:
                # single sub on an extended box serves both frames as views
                nd, nw = (9 if i else 8), (65 if k else 64)
                d0, w0 = (0 if i == 1 else 1), (0 if k == 1 else 1)
                dpe = trans.tile([P, C, nd, nw], bf16, tag="dpe")
                if pi == PAIR_ORDER[0]:
                    for ch in range(0, C, 2):
                        TT(dpe[:, ch:ch + 2],
                           imgb[:, ch:ch + 2, d0 + i:d0 + i + nd,
                                w0 + k:w0 + k + nw],
                           imgb[:, ch:ch + 2, d0:d0 + nd, w0:w0 + nw],
                           ALU.subtract)
                else:
                    TT(dpe[:],
                       imgb[:, :, d0 + i:d0 + i + nd, w0 + k:w0 + k + nw],
                       imgb[:, :, d0:d0 + nd, w0:w0 + nw], ALU.subtract)
                for fr in range(2):
                    ds = 1 - d0 - (i if fr else 0)
                    ws = 1 - w0 - (k if fr else 0)
                    mop = ALU.max if fr == 0 else ALU.min
                    dv = dpe[:, :, ds:ds + DL, ws:ws + W]
                    m2 = trans.tile([P, 2, DL, W], bf16, tag="m2")
                    TT(m2[:], dv[:, 0:2], dv[:, 2:4], mop)
                    TT(m1p[:, fr], m2[:, 0], m2[:, 1], mop)
            elif pi in T_MAXDVE:
                d4x = trans.tile([P, C, DL, W], bf16, tag="d4x", name="d4x")
                TT(d4x[:], cv(img_h[j], i, k), cv(imgb, 0, 0), ALU.subtract)
                m2x = trans.tile([P, 2, DL, W], bf16, tag="m2x", name="m2x")
                TT(m2x[:], d4x[:, 0:2], d4x[:, 2:4], ALU.max)
                TT(m1p[:, 0], m2x[:, 0], m2x[:, 1], ALU.max)
                if pi in MIN_ON_PE:
                    pe_frame(-j, -i, -k, ALU.min, m1p[:, 1])
                else:
                    d4 = trans.tile([P, C, DL, W], bf16, tag="d4")
                    TT(d4[:], cv(imgb, 0, 0), cv(img_h[-j], -i, -k),
                       ALU.subtract)
                    m2n = trans.tile([P, 2, DL, W], bf16, tag="m2n")
                    TT(m2n[:], d4[:, 0:2], d4[:, 2:4], ALU.min)
                    TT(m1p[:, 1], m2n[:, 0], m2n[:, 1], ALU.min)
            elif T_TREEBATCH and pi in MIN_ON_PE:
                pe_frame_pair(i, j, k, m1p)
            else:
                pe_frame(j, i, k, ALU.max, m1p[:, 0])
                if pi in MIN_ON_PE:
                    # sign-free: min tree of Sh_-j view(-i,-k) - center
                    pe_frame(-j, -i, -k, ALU.min, m1p[:, 1])
                else:
                    d4 = trans.tile([P, C, DL, W], bf16, tag="d4")
                    TT(d4[:], cv(imgb, 0, 0), cv(img_h[-j], -i, -k),
                       ALU.subtract)
                    m2n = trans.tile([P, 2, DL, W], bf16, tag="m2n")
                    TT(m2n[:], d4[:, 0:2], d4[:, 2:4], ALU.min)
                    TT(m1p[:, 1], m2n[:, 0], m2n[:, 1], ALU.min)

        def stage_B(pi):
            # u for both frames in one activation (const 2/sqrt(pi) cancels;
            # exp(-r2/2) lives in the scaled identity used by the acc matmuls)
            up = upool.tile([P, 2, DL, W], bf16, tag="up")
            up_t[pi] = up
            if pi == PAIR_ORDER[-1]:
                cph['gate'] = up
            m1p = m1p_t.pop(pi)
            if USE_DERF:
                nc.scalar.activation(up[:], m1p[:], AF.Derivative_Erf,
                                     scale=SQ2I)
            else:
                sqp = trans.tile([P, 2, DL, W], bf16, tag="sqp")
                nc.scalar.activation(sqp[:], m1p[:], AF.Square)
                nc.scalar.activation(up[:], sqp[:], AF.Exp, scale=-0.5)

        def stage_P(pi):
            i, j, k = PAIRS[pi]
            up = up_t[pi]
            pr2 = trans.tile([P, 2, 3, DL, W], bf16, tag="prods",
                             name="pr2", bufs=C_LAG + 2)
            prods_t[pi] = pr2
            for fr, sgn in ((0, 1), (1, -1)):
                si, sj, sk = sgn * i, sgn * j, sgn * k
                mview = cv(msk_h[sj], si, sk)
                ub = up[:, fr:fr + 1].broadcast_to((P, 3, DL, W))
                if (pi, fr) in T_PRODS_SPLIT:
                    # channel-split: 2ch on DVE, 1ch on Pool (finer quanta)
                    ub2 = up[:, fr:fr + 1].broadcast_to((P, 2, DL, W))
                    TT(pr2[:, fr, 0:2], ub2, mview[:, 0:2], ALU.mult)
                    ub1 = up[:, fr:fr + 1].broadcast_to((P, 1, DL, W))
                    nc.gpsimd.tensor_tensor(pr2[:, fr, 2:3], ub1,
                                            mview[:, 2:3], ALU.mult)
                else:
                    eng = (nc.gpsimd if (pi, fr) in T_PRODS_POOL
                           else nc.vector)
                    eng.tensor_tensor(pr2[:, fr], ub, mview, ALU.mult)

        def stage_C(pi):
            i, j, k = PAIRS[pi]
            r2 = float(i * i + j * j + k * k)
            st, sp = (pi == PAIR_ORDER[0]), (pi == PAIR_ORDER[-1])
            up = up_t.pop(pi)
            pr2 = prods_t.pop(pi)
            for fr in range(2):
                nc.tensor.matmul(SP[:], ir2[r2], up[:, fr],
                                 start=(st and fr == 0), stop=(sp and fr == 1))
                for ci in range(3):
                    nc.tensor.matmul(accP[:, ci], ir2[r2], pr2[:, fr, ci],
                                     start=(st and fr == 0),
                                     stop=(sp and fr == 1))

        NP = len(PAIR_ORDER)
        for idx in range(NP + max(B_LAG, C_LAG)):
            if idx == CPHASE_AT:
                emit_cphase_front()
            if idx == 0:
                emit_masks()
            if idx < NP:
                stage_A(PAIR_ORDER[idx])
            if idx == 0:
                emit_hshifts()
            if B_LAG <= idx < NP + B_LAG:
                stage_B(PAIR_ORDER[idx - B_LAG])
            if P_LAG <= idx < NP + P_LAG:
                stage_P(PAIR_ORDER[idx - P_LAG])
            if C_LAG <= idx < NP + C_LAG:
                stage_C(PAIR_ORDER[idx - C_LAG])
            if idx == CACT_AT and not exp_late:
                emit_cphase_act()
        if CPHASE_AT >= NP:
            emit_cphase_front()
        if not LSE_EARLY:
            emit_cphase_back()

        # ---- tail: p_w = sum(-0.5/S * sum_c dx_c*acc_c) ----
        dxb = cph['dxb']
        rS = cpool.tile([P, DL, W], f32, tag="rS")
        nc.vector.reciprocal_approx_fast(rS[:], SP[:])
        tp = cpool.tile([P, 3, DL, W], bf16, tag="tp")
        if T_TAILSPLIT:
            for ci in range(3):
                TT(tp[:, ci], accP[:, ci], dxb[:, ci], ALU.mult)
        else:
            TT(tp[:], accP[:], dxb[:], ALU.mult)
        t1 = cpool.tile([P, DL, W], bf16, tag="t1")
        TT(t1[:], tp[:, 0], tp[:, 1], ALU.add)
        t2 = cpool.tile([P, DL, W], bf16, tag="t2")
        TT(t2[:], t1[:], tp[:, 2], ALU.add)
        nc.vector.scalar_tensor_tensor(scr3[:], t2[:], -0.5, rS[:],
                                       ALU.mult, ALU.mult,
                                       accum_out=pl[:, 2:3])
        nc.sync.dma_start(out_d[:, :], pl[:])

    nc.compile()
    return nc


def _get_nc():
    if "nc" not in _CACHED:
        _CACHED["nc"] = _build_nc()
    return _CACHED["nc"]


def make_in_maps(inputs, labels, images):
    """Host-side shard: full inputs -> per-core input dicts (layout prep:
    (b,h)->partition transpose, d/w halo padding, bf16 pre-cast)."""
    import ml_dtypes

    bf = ml_dtypes.bfloat16
    img = np.asarray(images, np.float32).astype(bf)
    lab = np.asarray(labels).astype(bf)  # values 0..3, exact in bf16
    lgt = np.ascontiguousarray(np.asarray(inputs, np.float32))

    img_p = np.pad(img, ((0, 0), (0, 0), (1, 1), (0, 0), (1, 1)), mode="edge")
    lab_p = np.pad(lab, ((0, 0), (1, 1), (0, 0), (1, 1)), mode="edge")

    in_maps = []
    for k in range(NCORES):
        d0 = k * DL
        ic = img_p[:, :, d0:d0 + DE]          # [2,4,10,64,66]
        lc = lab_p[:, d0:d0 + DE]             # [2,10,64,66]
        xc = lgt[:, :, d0:d0 + DL]            # [2,4,8,64,64]
        im = np.ascontiguousarray(ic.transpose(0, 3, 1, 2, 4)).reshape(P, -1)
        lm = np.ascontiguousarray(lc.transpose(0, 2, 1, 3)).reshape(P, -1)
        xm = np.ascontiguousarray(
            xc.transpose(0, 3, 1, 2, 4)).reshape(P, -1).astype(bf)
        in_maps.append({"img": im, "lab": lm, "logits": xm, "eye": _mats()})
    return in_maps


def _mats():
    """[-I, Sh+-1, 3x scaled I, -Sh+-1, I] as one [P, 9P] bf16
    array. Sh(j)[k, m] = 1 iff k = b(m)*64 + clamp(h(m)+j, 0, 63)."""
    import ml_dtypes

    eye = np.eye(P, dtype=np.float32)
    sh = {}
    for jj in (1, -1):
        M = np.zeros((P, P), np.float32)
        for m in range(P):
            b, h = divmod(m, 64)
            M[b * 64 + min(max(h + jj, 0), 63), m] = 1.0
        sh[jj] = M
    blocks = [-eye, sh[1], sh[-1],
              np.exp(-0.5) * eye, np.exp(-1.0) * eye, np.exp(-1.5) * eye]
    if T_TREEBATCH:
        blocks += [-sh[1], -sh[-1], eye]
    out = np.concatenate(blocks, axis=1)
    return np.ascontiguousarray(out).astype(ml_dtypes.bfloat16)


def kernel(inputs, labels, images):
    from concourse.bass_utils import run_bass_kernel_spmd

    nc = _get_nc()
    in_maps = make_in_maps(inputs, labels, images)
    res = run_bass_kernel_spmd(nc, in_maps, core_ids=list(range(NCORES)))
    total = 0.0
    for k in range(NCORES):
        pl = res.results[k]["partials"].astype(np.float64)
        ym_scale = -1.0 if USE_XB else 0.5
        total += (pl[:, 0] - ym_scale * pl[:, 1] - pl[:, 3] + pl[:, 2]).sum()
    return np.float32(total / NVOX)



# revision 7
# speedup vs baseline: 1.0838x; 1.0004x over previous
"""Trainium2 Bass kernel for nn_CELossWithSVLS_VE (SVLS cross-entropy loss).

Math (derived + numerically validated vs reference):
  For the 26 non-center offsets n, with per-voxel
    u_n = exp(-0.5*(maxdiff_n^2 + r_n^2)),
    maxdiff_n(v) = max_c(img_c(v+n) - img_c(v))   (replicate-padded),
  the SVLS label weights reduce EXACTLY to w_center = 1/2, w_n = u_n/(2S),
  S = sum_n u_n.  Then
    loss(v) = lse(v) - 0.5*x_{l(v)}(v) - (1/(2S)) * sum_n u_n * x_{l(v+n)}(v)
  and the output is mean_v loss(v).

Engine plan (vs the 93.7us baseline):
  * u_n via ONE ScalarE activation: Derivative_Erf(m/sqrt2) = c*exp(-m^2/2);
    c cancels in T/S, and exp(-r2/2) moves into r2-scaled identity stationary
    matrices used by the PE accumulation matmuls (no bias/second activation).
  * most 4-channel stencil subtractions run on the PE as shift-matrix matmul
    pairs into PSUM; ScalarE copies PSUM->SBUF bf16 (the only engine that can
    get PSUM data back cheaply); DVE does only max/min trees + mask products.
  * loss folds into 3 per-partition accumulators (p_lse, p_yx, p_w) via
    accum_out side outputs; host combines  sum = p_lse - 0.5*p_yx + p_w.
    The T-dot reads accP straight out of PSUM (single-PSUM-operand TT).

Sharding: 8 cores, core k takes d-slab [8k, 8k+8) of both batches.
On-core layout: partition p = b*64 + h (128), free = (c?, d, w) with d,w
halos in SBUF.  h+-1 stencil shifts: PE shift-matrix matmuls (edge clamp
baked in) or partition-shifted SBUF DMA copies for the DVE-path frames.
"""
import sys
from contextlib import ExitStack

import numpy as np

if "/opt/trn_rl_repo" not in sys.path:
    sys.path.insert(0, "/opt/trn_rl_repo")

B, C, D, H, W = 2, 4, 64, 64, 64
NCORES = 8
DL = D // NCORES          # 8 local d-planes
DE, WE = DL + 2, W + 2    # 10, 66 (d/w halos)
P = 128                   # partitions = (b, h)
NVOX = B * D * H * W      # 524288

# 13 positive offsets; r2 = i*i+j*j+k*k.
PAIRS = [
    (1, 0, 0), (0, 0, 1), (1, 0, 1), (1, 0, -1),
    (0, 1, 0), (1, 1, 0), (1, -1, 0), (0, 1, 1), (0, 1, -1),
    (1, 1, 1), (1, 1, -1), (1, -1, 1), (1, -1, -1),
]

import os as _os, json as _json
_ov = _json.loads(_os.environ.get("KCONF", "{}"))
T_J0POOL = set(_ov.get("j0pool", []))     # j0 pairs: m2 on Pool
T_MINPOOL = set(_ov.get("minpool", []))   # DVE-min-path pairs: m2n on Pool
T_MASKS_DVE = _ov.get("masks_dve", 0)
T_EXPSUM_POOL = _ov.get("expsum_pool", 0)
T_PRODS_POOL = {tuple(t) for t in _ov.get("prods_pool",
                                           [[4, 1], [10, 1], [12, 1],
                                            [6, 1], [5, 1], [11, 1],
                                            [1, 1], [0, 1], [2, 1]])}
T_PSPLIT = _ov.get("psplit", 1)           # prods stage one slot early
T_MAXDVE = set(_ov.get("maxdve", [8, 9]))  # pairs: max-frame off PE
T_MINOFF = set(_ov.get("minoff", []))     # pairs removed from MIN_ON_PE
T_X0DVE = _ov.get("x0dve", 0)             # p_x0 accum via DVE tensor_scalar
T_YMPOOL = _ov.get("ympool", 0)           # ym STT on Pool (NO: won't compile)
T_YMSPLIT = _ov.get("ymsplit", 0)         # ym: Pool mult + DVE 4x ts-accum
T_TAILSPLIT = _ov.get("tailsplit", 0)     # per-channel tail tp
T_DMAQ = _ov.get("dmaq", 0)               # mats/labf on Act DGE queue
T_DMAORD = _ov.get("dmaord", 0)           # 1: labf,img,mats,x  2: img01 first
T_TREEBATCH = _ov.get("treebatch", 0)     # pair-batched trees on PE pairs
T_EXPTOKEN = _ov.get("exptoken", 1)       # gate exp/ln after last DErf
T_PRODS_SPLIT = {tuple(t) for t in _ov.get("prods_split",
                                            [[7, 0], [8, 0], [9, 0],
                                             [2, 0], [3, 0], [7, 1]])}
# ---- schedule config ----
USE_DERF = True
# j!=0 pairs whose min-frame runs on PE (rest: DVE sub via h-shifted copies)
MIN_ON_PE = {10, 11, 12, 4, 5, 6, 7} - T_MINOFF
# emission order (j0 pairs interleaved between PE-heavy pairs)
PAIR_ORDER = _ov.get("order", [0, 4, 5, 1, 10, 11, 2, 12, 6, 3, 7, 8, 9])
CPHASE_AT = _ov.get("cphase", 5)
CACT_AT = _ov.get("cact", 3)
LSE_EARLY = False
EXP_LATE = False  # overridden to True by T_EXPTOKEN at build time
USE_XB = False
USE_RSB = False
TTR_YM = False    # tensor_tensor_reduce compiles but faults at runtime
TTR_TAIL = False
USE_POOLOPS = True
CB_FULL = False
B_LAG = _ov.get("blag", 3)
C_LAG = _ov.get("clag", 3)

_CACHED = {}

SQ2I = 0.7071067811865476  # 1/sqrt(2)


def _build_nc():
    import concourse.bacc as bacc
    import concourse.mybir as mybir
    import concourse.tile as tile

    AF = mybir.ActivationFunctionType
    ALU = mybir.AluOpType
    dt = mybir.dt

    nc = bacc.Bacc("TRN2", target_bir_lowering=False, debug=False,
                   num_devices=NCORES)
    img_d = nc.dram_tensor("img", [P, C * DE * WE], dt.bfloat16,
                           kind="ExternalInput")
    lab_d = nc.dram_tensor("lab", [P, DE * WE], dt.bfloat16,
                           kind="ExternalInput")
    logit_d = nc.dram_tensor("logits", [P, C * DL * W], dt.bfloat16,
                             kind="ExternalInput")
    # mats: [-I, Sh(+1), Sh(-1), I*e^-.5, I*e^-1, I*e^-1.5,
    eye_d = nc.dram_tensor("eye", [P, (9 if T_TREEBATCH else 6) * P],
                           dt.bfloat16, kind="ExternalInput")
    out_d = nc.dram_tensor("partials", [P, 4], dt.float32,
                           kind="ExternalOutput")

    import concourse.bass as bass_mod

    exp_late = EXP_LATE or bool(T_EXPTOKEN)
    with tile.TileContext(nc) as tc, ExitStack() as ctx:
        persist = ctx.enter_context(tc.tile_pool(name="persist", bufs=1))
        cpool = ctx.enter_context(tc.tile_pool(name="cpool", bufs=1))
        trans = ctx.enter_context(tc.tile_pool(name="trans", bufs=3))
        upool = ctx.enter_context(
            tc.tile_pool(name="upool", bufs=max(3, B_LAG + 1, C_LAG - B_LAG + 2)))
        psum = ctx.enter_context(
            tc.tile_pool(name="psum", bufs=1, space=bass_mod.MemorySpace.PSUM))
        psum2 = ctx.enter_context(
            tc.tile_pool(name="psum2", bufs=(1 if CB_FULL else 2),
                         space=bass_mod.MemorySpace.PSUM))

        f32, bf16 = dt.float32, dt.bfloat16
        TT = nc.vector.tensor_tensor

        # ---- loads (images/labels arrive pre-cast to bf16 from host) ----
        # The DMA engines serialize transfers, so order by first use: mats
        # (PE idles until it lands), labf (masks), imgb per-channel, then the
        # big f32 logits tensor (only needed once the Act exp work starts).
        mats = persist.tile([P, 9 if T_TREEBATCH else 6, P], bf16,
                            tag="mats")
        labf = persist.tile([P, DE, WE], bf16, tag="labf")
        imgb = persist.tile([P, C, DE, WE], bf16, tag="imgb")
        x = persist.tile([P, C, DL, W], bf16, tag="x")

        def dma_img(c0, c1):
            for c in range(c0, c1):
                nc.sync.dma_start(imgb[:, c],
                                  img_d[:, c * DE * WE:(c + 1) * DE * WE])

        if T_DMAORD == 0:
            (nc.scalar if T_DMAQ else nc.sync).dma_start(mats[:], eye_d[:, :])
            dma_img(0, C)
            (nc.scalar if T_DMAQ else nc.sync).dma_start(labf[:], lab_d[:, :])
            nc.sync.dma_start(x[:], logit_d[:, :])
        elif T_DMAORD == 1:
            nc.sync.dma_start(labf[:], lab_d[:, :])
            dma_img(0, C)
            nc.sync.dma_start(mats[:], eye_d[:, :])
            nc.sync.dma_start(x[:], logit_d[:, :])
        else:
            dma_img(0, 2)
            nc.sync.dma_start(labf[:], lab_d[:, :])
            dma_img(2, C)
            nc.sync.dma_start(mats[:], eye_d[:, :])
            nc.sync.dma_start(x[:], logit_d[:, :])

        negI = mats[:, 0]
        shm = {1: mats[:, 1], -1: mats[:, 2]}
        ir2 = {1.0: mats[:, 3], 2.0: mats[:, 4], 3.0: mats[:, 5]}
        if T_TREEBATCH:
            nshm = {1: mats[:, 6], -1: mats[:, 7]}
            posI = mats[:, 8]

        masks = persist.tile([P, 3, DE, WE], bf16, tag="masks")

        def emit_masks():
            eng = nc.vector if T_MASKS_DVE else (
                nc.gpsimd if USE_POOLOPS else nc.vector)
            for ci, cval in enumerate((1.0, 2.0, 3.0)):
                eng.tensor_scalar(masks[:, ci], labf[:], cval, None,
                                  ALU.is_equal)

        # ---- h-shifted copies (partition shift via SBUF->SBUF DMA).
        def hshift_copies(dst_p, dst_m, src, eng):
            eng.dma_start(dst_p[0:63], src[1:64])
            eng.dma_start(dst_p[64:127], src[65:128])
            eng.dma_start(dst_p[63:64], src[63:64])
            eng.dma_start(dst_p[127:128], src[127:128])
            eng.dma_start(dst_m[1:64], src[0:63])
            eng.dma_start(dst_m[65:128], src[64:127])
            eng.dma_start(dst_m[0:1], src[0:1])
            eng.dma_start(dst_m[64:65], src[64:65])

        # masks_h before imgb_h: first mask-product use is much earlier than
        # the first DVE-path min-frame. SP ring so Act SEQ never blocks.
        masks_hp = persist.tile([P, 3, DE, WE], bf16, tag="masks_hp")
        masks_hm = persist.tile([P, 3, DE, WE], bf16, tag="masks_hm")
        msk_h = {1: masks_hp, 0: masks, -1: masks_hm}
        need_imgb_h = len(MIN_ON_PE) < 9 or len(T_MAXDVE) > 0
        if need_imgb_h:
            imgb_hp = persist.tile([P, C, DE, WE], bf16, tag="imgb_hp")
            imgb_hm = persist.tile([P, C, DE, WE], bf16, tag="imgb_hm")
            img_h = {1: imgb_hp, 0: imgb, -1: imgb_hm}

        def emit_hshifts():
            hshift_copies(masks_hp, masks_hm, masks, nc.sync)
            if need_imgb_h:
                hshift_copies(imgb_hp, imgb_hm, imgb, nc.sync)

        def cv(tile_, i, k):
            """center view shifted by (i, ., k) of a [..., DE, WE] tile."""
            return tile_[:, :, 1 + i:1 + i + DL, 1 + k:1 + k + W]

        # ---- PSUM accumulators; PE accumulates via r2-scaled identities ----
        accP = psum.tile([P, 3, DL, W], f32, tag="accP")
        SP = psum.tile([P, DL, W], f32, tag="SP")

        pl = cpool.tile([P, 4], f32, tag="pl")
        scr1 = cpool.tile([P, DL, W], f32, tag="scr1")
        scr2 = cpool.tile([P, DL, W], f32, tag="scr2")
        scr3 = cpool.tile([P, DL, W], f32, tag="scr3")

        cph = {}

        def emit_cphase_act():
            # exp-set work up front while PE/DVE wind up; p_x0 on the side
            if T_EXPTOKEN:
                # 1-element in-place bypass on x gated by the last pair's
                # DErf output: orders every x-reader emitted below (exp,
                # x0-accum) after the whole DErf block, so the act table
                # never leaves set 17 mid-stream (the readiness scheduler
                # would otherwise hoist exp into the DErf stream).
                TT(x[:, 0:1, 0:1, 0:1], x[:, 0:1, 0:1, 0:1],
                   cph['gate'][:, 0:1, 0:1, 0:1], ALU.bypass)
            if USE_XB:
                xb = cpool.tile([P, C, DL, W], bf16, tag="xb")
                nc.scalar.activation(xb[:], x[:], AF.Copy, scale=-0.5)
                cph.update(xb=xb)
            expx = cpool.tile([P, C, DL, W], bf16, tag="expx")
            nc.scalar.activation(expx[:], x[:], AF.Exp)
            if T_X0DVE:
                nc.vector.tensor_scalar(scr1[:], x[:, 0], 1.0, None,
                                        ALU.mult, accum_out=pl[:, 3:4])
            else:
                nc.scalar.activation(scr1[:], x[:, 0], AF.Copy,
                                     accum_out=pl[:, 3:4])
            cph.update(expx=expx)

        def emit_expsum():
            expx = cph['expx']
            e2 = cpool.tile([P, 2, DL, W], bf16, tag="e2")
            ene = nc.gpsimd if T_EXPSUM_POOL else nc.vector
            ene.tensor_tensor(e2[:], expx[:, 0:2], expx[:, 2:4], ALU.add)
            esum = cpool.tile([P, DL, W], bf16, tag="esum")
            ene.tensor_tensor(esum[:], e2[:, 0], e2[:, 1], ALU.add)
            cph.update(esum=esum)

        def emit_cphase_front():
            # DVE part: dxb; fused p_ym = sum(m_c * dx_c)
            if not exp_late:
                emit_expsum()
            xs = cph['xb'] if USE_XB else x
            dxb = cpool.tile([P, 3, DL, W], bf16, tag="dxb")
            TT(dxb[:], xs[:, 1:4], xs[:, 0:1].broadcast_to((P, 3, DL, W)),
               ALU.subtract)
            # p_ym = sum over (c,d,w) of m_c*dx_c: one stt dot with accum
            # (needs a contiguous mask-center copy; Pool makes it for free)
            mc = cpool.tile([P, 3, DL, W], bf16, tag="mc")
            nc.gpsimd.tensor_copy(mc[:], cv(masks, 0, 0))
            ym = cpool.tile([P, 3, DL, W], bf16, tag="ym")
            if T_YMSPLIT:
                nc.gpsimd.tensor_tensor(ym[:], mc[:], dxb[:], ALU.mult)
                scry = cpool.tile([P, 3, DL, W], bf16, tag="scry")
                nc.vector.tensor_scalar(scry[:], ym[:], 1.0, None,
                                        ALU.mult, accum_out=pl[:, 1:2])
            else:
                yme = nc.gpsimd if T_YMPOOL else nc.vector
                yme.scalar_tensor_tensor(ym[:], mc[:], 1.0, dxb[:],
                                         ALU.mult, ALU.mult,
                                         accum_out=pl[:, 1:2])
            cph.update(dxb=dxb)
            if LSE_EARLY:
                emit_cphase_back()

        def emit_cphase_back():
            # p_lse: one act-table switch back to the ln/exp set
            if exp_late:
                emit_cphase_act()
                emit_expsum()
            nc.scalar.activation(scr2[:], cph['esum'], AF.Ln,
                                 accum_out=pl[:, 0:1])

        def pe_frame_pair(i, j, k, m1p):
            """Both frames of a PE pair with batched trees: the min-frame
            uses negated stationaries (-Sh, +I) so BOTH trees are max-trees
            (Derivative_Erf is even), letting m2/m1p batch across frames."""
            cb2 = trans.tile([P, 2, C, DL, W], bf16, tag="cb2", name="cb2")
            for fr, sgn in ((0, 1), (1, -1)):
                si, sk = sgn * i, sgn * k
                st1 = shm[sgn * j] if fr == 0 else nshm[sgn * j]
                st2 = negI if fr == 0 else posI
                for half in range(2):
                    d4p = psum2.tile([P, 2, DL, W], f32, tag="d4p",
                                     name="d4p")
                    for cc in range(2):
                        c = 2 * half + cc
                        nc.tensor.matmul(d4p[:, cc], st1,
                                         imgb[:, c, 1 + si:1 + si + DL,
                                              1 + sk:1 + sk + W],
                                         start=True, stop=False)
                        nc.tensor.matmul(d4p[:, cc], st2,
                                         imgb[:, c, 1:1 + DL, 1:1 + W],
                                         start=False, stop=True)
                    nc.scalar.copy(cb2[:, fr, 2 * half:2 * half + 2],
                                   d4p[:])
            m2b = trans.tile([P, 2, 2, DL, W], bf16, tag="m2b", name="m2b")
            TT(m2b[:], cb2[:, :, 0:2], cb2[:, :, 2:4], ALU.max)
            TT(m1p[:], m2b[:, :, 0], m2b[:, :, 1], ALU.max)

        def pe_frame(jj, ii, kk, mop, m1p_slot):
            """d4 = Sh_jj.T@view(ii,kk) - center on PE; Act copyback halves;
            DVE tree into m1p_slot."""
            cb = trans.tile([P, C, DL, W], bf16, tag="cb")
            if CB_FULL:
                d4p = psum2.tile([P, C, DL, W], f32, tag="d4p")
                for c in range(C):
                    nc.tensor.matmul(d4p[:, c], shm[jj],
                                     imgb[:, c, 1 + ii:1 + ii + DL,
                                          1 + kk:1 + kk + W],
                                     start=True, stop=False)
                    nc.tensor.matmul(d4p[:, c], negI,
                                     imgb[:, c, 1:1 + DL, 1:1 + W],
                                     start=False, stop=True)
                nc.scalar.copy(cb[:], d4p[:])
            else:
                for half in range(2):
                    d4p = psum2.tile([P, 2, DL, W], f32, tag="d4p")
                    for cc in range(2):
                        c = 2 * half + cc
                        nc.tensor.matmul(d4p[:, cc], shm[jj],
                                         imgb[:, c, 1 + ii:1 + ii + DL,
                                              1 + kk:1 + kk + W],
                                         start=True, stop=False)
                        nc.tensor.matmul(d4p[:, cc], negI,
                                         imgb[:, c, 1:1 + DL, 1:1 + W],
                                         start=False, stop=True)
                    nc.scalar.copy(cb[:, 2 * half:2 * half + 2], d4p[:])
            m2 = trans.tile([P, 2, DL, W], bf16, tag="m2")
            TT(m2[:], cb[:, 0:2], cb[:, 2:4], mop)
            TT(m1p_slot, m2[:, 0], m2[:, 1], mop)

        # ---- software-pipelined main loop over offset pairs ----
        m1p_t, up_t, prods_t = {}, {}, {}
        P_LAG = max(B_LAG, C_LAG - T_PSPLIT)

        def stage_A(pi):
            i, j, k = PAIRS[pi]
            m1p = upool.tile([P, 2, DL, W], bf16, tag="m1p")
            m1p_t[pi] = m1p
            if j == 0:
                # single sub on an extended box serves both frames as views
                nd, nw = (9 if i else 8), (65 if k else 64)
                d0, w0 = (0 if i == 1 else 1), (0 if k == 1 else 1)
                dpe = trans.tile([P, C, nd, nw], bf16, tag="dpe")
                if pi == PAIR_ORDER[0]:
                    for ch in range(0, C, 2):
                        TT(dpe[:, ch:ch + 2],
                           imgb[:, ch:ch + 2, d0 + i:d0 + i + nd,
                                w0 + k:w0 + k + nw],
                           imgb[:, ch:ch + 2, d0:d0 + nd, w0:w0 + nw],
                           ALU.subtract)
                else:
                    TT(dpe[:],
                       imgb[:, :, d0 + i:d0 + i + nd, w0 + k:w0 + k + nw],
                       imgb[:, :, d0:d0 + nd, w0:w0 + nw], ALU.subtract)
                for fr in range(2):
                    ds = 1 - d0 - (i if fr else 0)
                    ws = 1 - w0 - (k if fr else 0)
                    mop = ALU.max if fr == 0 else ALU.min
                    dv = dpe[:, :, ds:ds + DL, ws:ws + W]
                    m2 = trans.tile([P, 2, DL, W], bf16, tag="m2")
                    TT(m2[:], dv[:, 0:2], dv[:, 2:4], mop)
                    TT(m1p[:, fr], m2[:, 0], m2[:, 1], mop)
            elif pi in T_MAXDVE:
                d4x = trans.tile([P, C, DL, W], bf16, tag="d4x", name="d4x")
                TT(d4x[:], cv(img_h[j], i, k), cv(imgb, 0, 0), ALU.subtract)
                m2x = trans.tile([P, 2, DL, W], bf16, tag="m2x", name="m2x")
                TT(m2x[:], d4x[:, 0:2], d4x[:, 2:4], ALU.max)
                TT(m1p[:, 0], m2x[:, 0], m2x[:, 1], ALU.max)
                if pi in MIN_ON_PE:
                    pe_frame(-j, -i, -k, ALU.min, m1p[:, 1])
                else:
                    d4 = trans.tile([P, C, DL, W], bf16, tag="d4")
                    TT(d4[:], cv(imgb, 0, 0), cv(img_h[-j], -i, -k),
                       ALU.subtract)
                    m2n = trans.tile([P, 2, DL, W], bf16, tag="m2n")
                    TT(m2n[:], d4[:, 0:2], d4[:, 2:4], ALU.min)
                    TT(m1p[:, 1], m2n[:, 0], m2n[:, 1], ALU.min)
            elif T_TREEBATCH and pi in MIN_ON_PE:
                pe_frame_pair(i, j, k, m1p)
            else:
                pe_frame(j, i, k, ALU.max, m1p[:, 0])
                if pi in MIN_ON_PE:
                    # sign-free: min tree of Sh_-j view(-i,-k) - center
                    pe_frame(-j, -i, -k, ALU.min, m1p[:, 1])
                else:
                    d4 = trans.tile([P, C, DL, W], bf16, tag="d4")
                    TT(d4[:], cv(imgb, 0, 0), cv(img_h[-j], -i, -k),
                       ALU.subtract)
                    m2n = trans.tile([P, 2, DL, W], bf16, tag="m2n")
                    TT(m2n[:], d4[:, 0:2], d4[:, 2:4], ALU.min)
                    TT(m1p[:, 1], m2n[:, 0], m2n[:, 1], ALU.min)

        def stage_B(pi):
            # u for both frames in one activation (const 2/sqrt(pi) cancels;
            # exp(-r2/2) lives in the scaled identity used by the acc matmuls)
            up = upool.tile([P, 2, DL, W], bf16, tag="up")
            up_t[pi] = up
            if pi == PAIR_ORDER[-1]:
                cph['gate'] = up
            m1p = m1p_t.pop(pi)
            if USE_DERF:
                nc.scalar.activation(up[:], m1p[:], AF.Derivative_Erf,
                                     scale=SQ2I)
            else:
                sqp = trans.tile([P, 2, DL, W], bf16, tag="sqp")
                nc.scalar.activation(sqp[:], m1p[:], AF.Square)
                nc.scalar.activation(up[:], sqp[:], AF.Exp, scale=-0.5)

        def stage_P(pi):
            i, j, k = PAIRS[pi]
            up = up_t[pi]
            pr2 = trans.tile([P, 2, 3, DL, W], bf16, tag="prods",
                             name="pr2", bufs=C_LAG + 2)
            prods_t[pi] = pr2
            for fr, sgn in ((0, 1), (1, -1)):
                si, sj, sk = sgn * i, sgn * j, sgn * k
                mview = cv(msk_h[sj], si, sk)
                ub = up[:, fr:fr + 1].broadcast_to((P, 3, DL, W))
                if (pi, fr) in T_PRODS_SPLIT:
                    # channel-split: 2ch on DVE, 1ch on Pool (finer quanta)
                    ub2 = up[:, fr:fr + 1].broadcast_to((P, 2, DL, W))
                    TT(pr2[:, fr, 0:2], ub2, mview[:, 0:2], ALU.mult)
                    ub1 = up[:, fr:fr + 1].broadcast_to((P, 1, DL, W))
                    nc.gpsimd.tensor_tensor(pr2[:, fr, 2:3], ub1,
                                            mview[:, 2:3], ALU.mult)
                else:
                    eng = (nc.gpsimd if (pi, fr) in T_PRODS_POOL
                           else nc.vector)
                    eng.tensor_tensor(pr2[:, fr], ub, mview, ALU.mult)

        def stage_C(pi):
            i, j, k = PAIRS[pi]
            r2 = float(i * i + j * j + k * k)
            st, sp = (pi == PAIR_ORDER[0]), (pi == PAIR_ORDER[-1])
            up = up_t.pop(pi)
            pr2 = prods_t.pop(pi)
            for fr in range(2):
                nc.tensor.matmul(SP[:], ir2[r2], up[:, fr],
                                 start=(st and fr == 0), stop=(sp and fr == 1))
                for ci in range(3):
                    nc.tensor.matmul(accP[:, ci], ir2[r2], pr2[:, fr, ci],
                                     start=(st and fr == 0),
                                     stop=(sp and fr == 1))

        NP = len(PAIR_ORDER)
        for idx in range(NP + max(B_LAG, C_LAG)):
            if idx == CPHASE_AT:
                emit_cphase_front()
            if idx == 0:
                emit_masks()
            if idx < NP:
                stage_A(PAIR_ORDER[idx])
            if idx == 0:
                emit_hshifts()
            if B_LAG <= idx < NP + B_LAG:
                stage_B(PAIR_ORDER[idx - B_LAG])
            if P_LAG <= idx < NP + P_LAG:
                stage_P(PAIR_ORDER[idx - P_LAG])
            if C_LAG <= idx < NP + C_LAG:
                stage_C(PAIR_ORDER[idx - C_LAG])
            if idx == CACT_AT and not exp_late:
                emit_cphase_act()
        if CPHASE_AT >= NP:
            emit_cphase_front()
        if not LSE_EARLY:
            emit_cphase_back()

        # ---- tail: p_w = sum(-0.5/S * sum_c dx_c*acc_c) ----
        dxb = cph['dxb']
        rS = cpool.tile([P, DL, W], f32, tag="rS")
        nc.vector.reciprocal_approx_fast(rS[:], SP[:])
        tp = cpool.tile([P, 3, DL, W], bf16, tag="tp")
        if T_TAILSPLIT:
            for ci in range(3):
                TT(tp[:, ci], accP[:, ci], dxb[:, ci], ALU.mult)
        else:
            TT(tp[:], accP[:], dxb[:], ALU.mult)
        t1 = cpool.tile([P, DL, W], bf16, tag="t1")
        TT(t1[:], tp[:, 0], tp[:, 1], ALU.add)
        t2 = cpool.tile([P, DL, W], bf16, tag="t2")
        TT(t2[:], t1[:], tp[:, 2], ALU.add)
        nc.vector.scalar_tensor_tensor(scr3[:], t2[:], -0.5, rS[:],
                                       ALU.mult, ALU.mult,
                                       accum_out=pl[:, 2:3])
        nc.sync.dma_start(out_d[:, :], pl[:])

    nc.compile()
    return nc


def _get_nc():
    if "nc" not in _CACHED:
        _CACHED["nc"] = _build_nc()
    return _CACHED["nc"]


def make_in_maps(inputs, labels, images):
    """Host-side shard: full inputs -> per-core input dicts (layout prep:
    (b,h)->partition transpose, d/w halo padding, bf16 pre-cast)."""
    import ml_dtypes

    bf = ml_dtypes.bfloat16
    img = np.asarray(images, np.float32).astype(bf)
    lab = np.asarray(labels).astype(bf)  # values 0..3, exact in bf16
    lgt = np.ascontiguousarray(np.asarray(inputs, np.float32))

    img_p = np.pad(img, ((0, 0), (0, 0), (1, 1), (0, 0), (1, 1)), mode="edge")
    lab_p = np.pad(lab, ((0, 0), (1, 1), (0, 0), (1, 1)), mode="edge")

    in_maps = []
    for k in range(NCORES):
        d0 = k * DL
        ic = img_p[:, :, d0:d0 + DE]          # [2,4,10,64,66]
        lc = lab_p[:, d0:d0 + DE]             # [2,10,64,66]
        xc = lgt[:, :, d0:d0 + DL]            # [2,4,8,64,64]
        im = np.ascontiguousarray(ic.transpose(0, 3, 1, 2, 4)).reshape(P, -1)
        lm = np.ascontiguousarray(lc.transpose(0, 2, 1, 3)).reshape(P, -1)
        xm = np.ascontiguousarray(
            xc.transpose(0, 3, 1, 2, 4)).reshape(P, -1).astype(bf)
        in_maps.append({"img": im, "lab": lm, "logits": xm, "eye": _mats()})
    return in_maps


def _mats():
    """[-I, Sh+-1, 3x scaled I, -Sh+-1, I] as one [P, 9P] bf16
    array. Sh(j)[k, m] = 1 iff k = b(m)*64 + clamp(h(m)+j, 0, 63)."""
    import ml_dtypes

    eye = np.eye(P, dtype=np.float32)
    sh = {}
    for jj in (1, -1):
        M = np.zeros((P, P), np.float32)
        for m in range(P):
            b, h = divmod(m, 64)
            M[b * 64 + min(max(h + jj, 0), 63), m] = 1.0
        sh[jj] = M
    blocks = [-eye, sh[1], sh[-1],
              np.exp(-0.5) * eye, np.exp(-1.0) * eye, np.exp(-1.5) * eye]
    if T_TREEBATCH:
        blocks += [-sh[1], -sh[-1], eye]
    out = np.concatenate(blocks, axis=1)
    return np.ascontiguousarray(out).astype(ml_dtypes.bfloat16)


def kernel(inputs, labels, images):
    from concourse.bass_utils import run_bass_kernel_spmd

    nc = _get_nc()
    in_maps = make_in_maps(inputs, labels, images)
    res = run_bass_kernel_spmd(nc, in_maps, core_ids=list(range(NCORES)))
    total = 0.0
    for k in range(NCORES):
        pl = res.results[k]["partials"].astype(np.float64)
        ym_scale = -1.0 if USE_XB else 0.5
        total += (pl[:, 0] - ym_scale * pl[:, 1] - pl[:, 3] + pl[:, 2]).sum()
    return np.float32(total / NVOX)



# revision 8
# speedup vs baseline: 1.0840x; 1.0002x over previous
"""Trainium2 Bass kernel for nn_CELossWithSVLS_VE (SVLS cross-entropy loss).

Math (derived + numerically validated vs reference):
  For the 26 non-center offsets n, with per-voxel
    u_n = exp(-0.5*(maxdiff_n^2 + r_n^2)),
    maxdiff_n(v) = max_c(img_c(v+n) - img_c(v))   (replicate-padded),
  the SVLS label weights reduce EXACTLY to w_center = 1/2, w_n = u_n/(2S),
  S = sum_n u_n.  Then
    loss(v) = lse(v) - 0.5*x_{l(v)}(v) - (1/(2S)) * sum_n u_n * x_{l(v+n)}(v)
  and the output is mean_v loss(v).

Engine plan (vs the 93.7us baseline):
  * u_n via ONE ScalarE activation: Derivative_Erf(m/sqrt2) = c*exp(-m^2/2);
    c cancels in T/S, and exp(-r2/2) moves into r2-scaled identity stationary
    matrices used by the PE accumulation matmuls (no bias/second activation).
  * most 4-channel stencil subtractions run on the PE as shift-matrix matmul
    pairs into PSUM; ScalarE copies PSUM->SBUF bf16 (the only engine that can
    get PSUM data back cheaply); DVE does only max/min trees + mask products.
  * loss folds into 3 per-partition accumulators (p_lse, p_yx, p_w) via
    accum_out side outputs; host combines  sum = p_lse - 0.5*p_yx + p_w.
    The T-dot reads accP straight out of PSUM (single-PSUM-operand TT).

Sharding: 8 cores, core k takes d-slab [8k, 8k+8) of both batches.
On-core layout: partition p = b*64 + h (128), free = (c?, d, w) with d,w
halos in SBUF.  h+-1 stencil shifts: PE shift-matrix matmuls (edge clamp
baked in) or partition-shifted SBUF DMA copies for the DVE-path frames.
"""
import sys
from contextlib import ExitStack

import numpy as np

if "/opt/trn_rl_repo" not in sys.path:
    sys.path.insert(0, "/opt/trn_rl_repo")

B, C, D, H, W = 2, 4, 64, 64, 64
NCORES = 8
DL = D // NCORES          # 8 local d-planes
DE, WE = DL + 2, W + 2    # 10, 66 (d/w halos)
P = 128                   # partitions = (b, h)
NVOX = B * D * H * W      # 524288

# 13 positive offsets; r2 = i*i+j*j+k*k.
PAIRS = [
    (1, 0, 0), (0, 0, 1), (1, 0, 1), (1, 0, -1),
    (0, 1, 0), (1, 1, 0), (1, -1, 0), (0, 1, 1), (0, 1, -1),
    (1, 1, 1), (1, 1, -1), (1, -1, 1), (1, -1, -1),
]

import os as _os, json as _json
_ov = _json.loads(_os.environ.get("KCONF", "{}"))
T_J0POOL = set(_ov.get("j0pool", []))     # j0 pairs: m2 on Pool
T_MINPOOL = set(_ov.get("minpool", []))   # DVE-min-path pairs: m2n on Pool
T_MASKS_DVE = _ov.get("masks_dve", 0)
T_EXPSUM_POOL = _ov.get("expsum_pool", 1)
T_PRODS_POOL = {tuple(t) for t in _ov.get("prods_pool",
                                           [[4, 1], [10, 1], [12, 1],
                                            [6, 1], [5, 1], [11, 1],
                                            [1, 1], [0, 1], [2, 1]])}
T_PSPLIT = _ov.get("psplit", 1)           # prods stage one slot early
T_MAXDVE = set(_ov.get("maxdve", [8, 9]))  # pairs: max-frame off PE
T_MINOFF = set(_ov.get("minoff", []))     # pairs removed from MIN_ON_PE
T_X0DVE = _ov.get("x0dve", 0)             # p_x0 accum via DVE tensor_scalar
T_YMPOOL = _ov.get("ympool", 0)           # ym STT on Pool (NO: won't compile)
T_YMSPLIT = _ov.get("ymsplit", 0)         # ym: Pool mult + DVE 4x ts-accum
T_TAILSPLIT = _ov.get("tailsplit", 0)     # per-channel tail tp
T_DMAQ = _ov.get("dmaq", 0)               # mats/labf on Act DGE queue
T_DMAORD = _ov.get("dmaord", 0)           # 1: labf,img,mats,x  2: img01 first
T_TREEBATCH = _ov.get("treebatch", 0)     # pair-batched trees on PE pairs
T_EXPTOKEN = _ov.get("exptoken", 1)       # gate exp/ln after last DErf
T_PRODS_SPLIT = {tuple(t) for t in _ov.get("prods_split",
                                            [[7, 0], [8, 0], [9, 0],
                                             [2, 0], [3, 0], [7, 1]])}
# ---- schedule config ----
USE_DERF = True
# j!=0 pairs whose min-frame runs on PE (rest: DVE sub via h-shifted copies)
MIN_ON_PE = {10, 11, 12, 4, 5, 6, 7} - T_MINOFF
# emission order (j0 pairs interleaved between PE-heavy pairs)
PAIR_ORDER = _ov.get("order", [0, 4, 5, 1, 10, 11, 2, 12, 6, 3, 7, 8, 9])
CPHASE_AT = _ov.get("cphase", 5)
CACT_AT = _ov.get("cact", 3)
LSE_EARLY = False
EXP_LATE = False  # overridden to True by T_EXPTOKEN at build time
USE_XB = False
USE_RSB = False
TTR_YM = False    # tensor_tensor_reduce compiles but faults at runtime
TTR_TAIL = False
USE_POOLOPS = True
CB_FULL = False
B_LAG = _ov.get("blag", 3)
C_LAG = _ov.get("clag", 3)

_CACHED = {}

SQ2I = 0.7071067811865476  # 1/sqrt(2)


def _build_nc():
    import concourse.bacc as bacc
    import concourse.mybir as mybir
    import concourse.tile as tile

    AF = mybir.ActivationFunctionType
    ALU = mybir.AluOpType
    dt = mybir.dt

    nc = bacc.Bacc("TRN2", target_bir_lowering=False, debug=False,
                   num_devices=NCORES)
    img_d = nc.dram_tensor("img", [P, C * DE * WE], dt.bfloat16,
                           kind="ExternalInput")
    lab_d = nc.dram_tensor("lab", [P, DE * WE], dt.bfloat16,
                           kind="ExternalInput")
    logit_d = nc.dram_tensor("logits", [P, C * DL * W], dt.bfloat16,
                             kind="ExternalInput")
    # mats: [-I, Sh(+1), Sh(-1), I*e^-.5, I*e^-1, I*e^-1.5,
    eye_d = nc.dram_tensor("eye", [P, (9 if T_TREEBATCH else 6) * P],
                           dt.bfloat16, kind="ExternalInput")
    out_d = nc.dram_tensor("partials", [P, 4], dt.float32,
                           kind="ExternalOutput")

    import concourse.bass as bass_mod

    exp_late = EXP_LATE or bool(T_EXPTOKEN)
    with tile.TileContext(nc) as tc, ExitStack() as ctx:
        persist = ctx.enter_context(tc.tile_pool(name="persist", bufs=1))
        cpool = ctx.enter_context(tc.tile_pool(name="cpool", bufs=1))
        trans = ctx.enter_context(tc.tile_pool(name="trans", bufs=3))
        upool = ctx.enter_context(
            tc.tile_pool(name="upool", bufs=max(3, B_LAG + 1, C_LAG - B_LAG + 2)))
        psum = ctx.enter_context(
            tc.tile_pool(name="psum", bufs=1, space=bass_mod.MemorySpace.PSUM))
        psum2 = ctx.enter_context(
            tc.tile_pool(name="psum2", bufs=(1 if CB_FULL else 2),
                         space=bass_mod.MemorySpace.PSUM))

        f32, bf16 = dt.float32, dt.bfloat16
        TT = nc.vector.tensor_tensor

        # ---- loads (images/labels arrive pre-cast to bf16 from host) ----
        # The DMA engines serialize transfers, so order by first use: mats
        # (PE idles until it lands), labf (masks), imgb per-channel, then the
        # big f32 logits tensor (only needed once the Act exp work starts).
        mats = persist.tile([P, 9 if T_TREEBATCH else 6, P], bf16,
                            tag="mats")
        labf = persist.tile([P, DE, WE], bf16, tag="labf")
        imgb = persist.tile([P, C, DE, WE], bf16, tag="imgb")
        x = persist.tile([P, C, DL, W], bf16, tag="x")

        def dma_img(c0, c1):
            for c in range(c0, c1):
                nc.sync.dma_start(imgb[:, c],
                                  img_d[:, c * DE * WE:(c + 1) * DE * WE])

        if T_DMAORD == 0:
            (nc.scalar if T_DMAQ else nc.sync).dma_start(mats[:], eye_d[:, :])
            dma_img(0, C)
            (nc.scalar if T_DMAQ else nc.sync).dma_start(labf[:], lab_d[:, :])
            nc.sync.dma_start(x[:], logit_d[:, :])
        elif T_DMAORD == 1:
            nc.sync.dma_start(labf[:], lab_d[:, :])
            dma_img(0, C)
            nc.sync.dma_start(mats[:], eye_d[:, :])
            nc.sync.dma_start(x[:], logit_d[:, :])
        else:
            dma_img(0, 2)
            nc.sync.dma_start(labf[:], lab_d[:, :])
            dma_img(2, C)
            nc.sync.dma_start(mats[:], eye_d[:, :])
            nc.sync.dma_start(x[:], logit_d[:, :])

        negI = mats[:, 0]
        shm = {1: mats[:, 1], -1: mats[:, 2]}
        ir2 = {1.0: mats[:, 3], 2.0: mats[:, 4], 3.0: mats[:, 5]}
        if T_TREEBATCH:
            nshm = {1: mats[:, 6], -1: mats[:, 7]}
            posI = mats[:, 8]

        masks = persist.tile([P, 3, DE, WE], bf16, tag="masks")

        def emit_masks():
            eng = nc.vector if T_MASKS_DVE else (
                nc.gpsimd if USE_POOLOPS else nc.vector)
            for ci, cval in enumerate((1.0, 2.0, 3.0)):
                eng.tensor_scalar(masks[:, ci], labf[:], cval, None,
                                  ALU.is_equal)

        # ---- h-shifted copies (partition shift via SBUF->SBUF DMA).
        def hshift_copies(dst_p, dst_m, src, eng):
            eng.dma_start(dst_p[0:63], src[1:64])
            eng.dma_start(dst_p[64:127], src[65:128])
            eng.dma_start(dst_p[63:64], src[63:64])
            eng.dma_start(dst_p[127:128], src[127:128])
            eng.dma_start(dst_m[1:64], src[0:63])
            eng.dma_start(dst_m[65:128], src[64:127])
            eng.dma_start(dst_m[0:1], src[0:1])
            eng.dma_start(dst_m[64:65], src[64:65])

        # masks_h before imgb_h: first mask-product use is much earlier than
        # the first DVE-path min-frame. SP ring so Act SEQ never blocks.
        masks_hp = persist.tile([P, 3, DE, WE], bf16, tag="masks_hp")
        masks_hm = persist.tile([P, 3, DE, WE], bf16, tag="masks_hm")
        msk_h = {1: masks_hp, 0: masks, -1: masks_hm}
        need_imgb_h = len(MIN_ON_PE) < 9 or len(T_MAXDVE) > 0
        if need_imgb_h:
            imgb_hp = persist.tile([P, C, DE, WE], bf16, tag="imgb_hp")
            imgb_hm = persist.tile([P, C, DE, WE], bf16, tag="imgb_hm")
            img_h = {1: imgb_hp, 0: imgb, -1: imgb_hm}

        def emit_hshifts():
            hshift_copies(masks_hp, masks_hm, masks, nc.sync)
            if need_imgb_h:
                hshift_copies(imgb_hp, imgb_hm, imgb, nc.sync)

        def cv(tile_, i, k):
            """center view shifted by (i, ., k) of a [..., DE, WE] tile."""
            return tile_[:, :, 1 + i:1 + i + DL, 1 + k:1 + k + W]

        # ---- PSUM accumulators; PE accumulates via r2-scaled identities ----
        accP = psum.tile([P, 3, DL, W], f32, tag="accP")
        SP = psum.tile([P, DL, W], f32, tag="SP")

        pl = cpool.tile([P, 4], f32, tag="pl")
        scr1 = cpool.tile([P, DL, W], f32, tag="scr1")
        scr2 = cpool.tile([P, DL, W], f32, tag="scr2")
        scr3 = cpool.tile([P, DL, W], f32, tag="scr3")

        cph = {}

        def emit_cphase_act():
            # exp-set work up front while PE/DVE wind up; p_x0 on the side
            if T_EXPTOKEN:
                # 1-element in-place bypass on x gated by the last pair's
                # DErf output: orders every x-reader emitted below (exp,
                # x0-accum) after the whole DErf block, so the act table
                # never leaves set 17 mid-stream (the readiness scheduler
                # would otherwise hoist exp into the DErf stream).
                TT(x[:, 0:1, 0:1, 0:1], x[:, 0:1, 0:1, 0:1],
                   cph['gate'][:, 0:1, 0:1, 0:1], ALU.bypass)
            if USE_XB:
                xb = cpool.tile([P, C, DL, W], bf16, tag="xb")
                nc.scalar.activation(xb[:], x[:], AF.Copy, scale=-0.5)
                cph.update(xb=xb)
            expx = cpool.tile([P, C, DL, W], bf16, tag="expx")
            nc.scalar.activation(expx[:], x[:], AF.Exp)
            if T_X0DVE:
                nc.vector.tensor_scalar(scr1[:], x[:, 0], 1.0, None,
                                        ALU.mult, accum_out=pl[:, 3:4])
            else:
                nc.scalar.activation(scr1[:], x[:, 0], AF.Copy,
                                     accum_out=pl[:, 3:4])
            cph.update(expx=expx)

        def emit_expsum():
            expx = cph['expx']
            e2 = cpool.tile([P, 2, DL, W], bf16, tag="e2")
            ene = nc.gpsimd if T_EXPSUM_POOL else nc.vector
            ene.tensor_tensor(e2[:], expx[:, 0:2], expx[:, 2:4], ALU.add)
            esum = cpool.tile([P, DL, W], bf16, tag="esum")
            ene.tensor_tensor(esum[:], e2[:, 0], e2[:, 1], ALU.add)
            cph.update(esum=esum)

        def emit_cphase_front():
            # DVE part: dxb; fused p_ym = sum(m_c * dx_c)
            if not exp_late:
                emit_expsum()
            xs = cph['xb'] if USE_XB else x
            dxb = cpool.tile([P, 3, DL, W], bf16, tag="dxb")
            TT(dxb[:], xs[:, 1:4], xs[:, 0:1].broadcast_to((P, 3, DL, W)),
               ALU.subtract)
            # p_ym = sum over (c,d,w) of m_c*dx_c: one stt dot with accum
            # (needs a contiguous mask-center copy; Pool makes it for free)
            mc = cpool.tile([P, 3, DL, W], bf16, tag="mc")
            nc.gpsimd.tensor_copy(mc[:], cv(masks, 0, 0))
            ym = cpool.tile([P, 3, DL, W], bf16, tag="ym")
            if T_YMSPLIT:
                nc.gpsimd.tensor_tensor(ym[:], mc[:], dxb[:], ALU.mult)
                scry = cpool.tile([P, 3, DL, W], bf16, tag="scry")
                nc.vector.tensor_scalar(scry[:], ym[:], 1.0, None,
                                        ALU.mult, accum_out=pl[:, 1:2])
            else:
                yme = nc.gpsimd if T_YMPOOL else nc.vector
                yme.scalar_tensor_tensor(ym[:], mc[:], 1.0, dxb[:],
                                         ALU.mult, ALU.mult,
                                         accum_out=pl[:, 1:2])
            cph.update(dxb=dxb)
            if LSE_EARLY:
                emit_cphase_back()

        def emit_cphase_back():
            # p_lse: one act-table switch back to the ln/exp set
            if exp_late:
                emit_cphase_act()
                emit_expsum()
            nc.scalar.activation(scr2[:], cph['esum'], AF.Ln,
                                 accum_out=pl[:, 0:1])

        def pe_frame_pair(i, j, k, m1p):
            """Both frames of a PE pair with batched trees: the min-frame
            uses negated stationaries (-Sh, +I) so BOTH trees are max-trees
            (Derivative_Erf is even), letting m2/m1p batch across frames."""
            cb2 = trans.tile([P, 2, C, DL, W], bf16, tag="cb2", name="cb2")
            for fr, sgn in ((0, 1), (1, -1)):
                si, sk = sgn * i, sgn * k
                st1 = shm[sgn * j] if fr == 0 else nshm[sgn * j]
                st2 = negI if fr == 0 else posI
                for half in range(2):
                    d4p = psum2.tile([P, 2, DL, W], f32, tag="d4p",
                                     name="d4p")
                    for cc in range(2):
                        c = 2 * half + cc
                        nc.tensor.matmul(d4p[:, cc], st1,
                                         imgb[:, c, 1 + si:1 + si + DL,
                                              1 + sk:1 + sk + W],
                                         start=True, stop=False)
                        nc.tensor.matmul(d4p[:, cc], st2,
                                         imgb[:, c, 1:1 + DL, 1:1 + W],
                                         start=False, stop=True)
                    nc.scalar.copy(cb2[:, fr, 2 * half:2 * half + 2],
                                   d4p[:])
            m2b = trans.tile([P, 2, 2, DL, W], bf16, tag="m2b", name="m2b")
            TT(m2b[:], cb2[:, :, 0:2], cb2[:, :, 2:4], ALU.max)
            TT(m1p[:], m2b[:, :, 0], m2b[:, :, 1], ALU.max)

        def pe_frame(jj, ii, kk, mop, m1p_slot):
            """d4 = Sh_jj.T@view(ii,kk) - center on PE; Act copyback halves;
            DVE tree into m1p_slot."""
            cb = trans.tile([P, C, DL, W], bf16, tag="cb")
            if CB_FULL:
                d4p = psum2.tile([P, C, DL, W], f32, tag="d4p")
                for c in range(C):
                    nc.tensor.matmul(d4p[:, c], shm[jj],
                                     imgb[:, c, 1 + ii:1 + ii + DL,
                                          1 + kk:1 + kk + W],
                                     start=True, stop=False)
                    nc.tensor.matmul(d4p[:, c], negI,
                                     imgb[:, c, 1:1 + DL, 1:1 + W],
                                     start=False, stop=True)
                nc.scalar.copy(cb[:], d4p[:])
            else:
                for half in range(2):
                    d4p = psum2.tile([P, 2, DL, W], f32, tag="d4p")
                    for cc in range(2):
                        c = 2 * half + cc
                        nc.tensor.matmul(d4p[:, cc], shm[jj],
                                         imgb[:, c, 1 + ii:1 + ii + DL,
                                              1 + kk:1 + kk + W],
                                         start=True, stop=False)
                        nc.tensor.matmul(d4p[:, cc], negI,
                                         imgb[:, c, 1:1 + DL, 1:1 + W],
                                         start=False, stop=True)
                    nc.scalar.copy(cb[:, 2 * half:2 * half + 2], d4p[:])
            m2 = trans.tile([P, 2, DL, W], bf16, tag="m2")
            TT(m2[:], cb[:, 0:2], cb[:, 2:4], mop)
            TT(m1p_slot, m2[:, 0], m2[:, 1], mop)

        # ---- software-pipelined main loop over offset pairs ----
        m1p_t, up_t, prods_t = {}, {}, {}
        P_LAG = max(B_LAG, C_LAG - T_PSPLIT)

        def stage_A(pi):
            i, j, k = PAIRS[pi]
            m1p = upool.tile([P, 2, DL, W], bf16, tag="m1p")
            m1p_t[pi] = m1p
            if j == 0:
                # single sub on an extended box serves both frames as views
                nd, nw = (9 if i else 8), (65 if k else 64)
                d0, w0 = (0 if i == 1 else 1), (0 if k == 1 else 1)
                dpe = trans.tile([P, C, nd, nw], bf16, tag="dpe")
                if pi == PAIR_ORDER[0]:
                    for ch in range(0, C, 2):
                        TT(dpe[:, ch:ch + 2],
                           imgb[:, ch:ch + 2, d0 + i:d0 + i + nd,
                                w0 + k:w0 + k + nw],
                           imgb[:, ch:ch + 2, d0:d0 + nd, w0:w0 + nw],
                           ALU.subtract)
                else:
                    TT(dpe[:],
                       imgb[:, :, d0 + i:d0 + i + nd, w0 + k:w0 + k + nw],
                       imgb[:, :, d0:d0 + nd, w0:w0 + nw], ALU.subtract)
                for fr in range(2):
                    ds = 1 - d0 - (i if fr else 0)
                    ws = 1 - w0 - (k if fr else 0)
                    mop = ALU.max if fr == 0 else ALU.min
                    dv = dpe[:, :, ds:ds + DL, ws:ws + W]
                    m2 = trans.tile([P, 2, DL, W], bf16, tag="m2")
                    TT(m2[:], dv[:, 0:2], dv[:, 2:4], mop)
                    TT(m1p[:, fr], m2[:, 0], m2[:, 1], mop)
            elif pi in T_MAXDVE:
                d4x = trans.tile([P, C, DL, W], bf16, tag="d4x", name="d4x")
                TT(d4x[:], cv(img_h[j], i, k), cv(imgb, 0, 0), ALU.subtract)
                m2x = trans.tile([P, 2, DL, W], bf16, tag="m2x", name="m2x")
                TT(m2x[:], d4x[:, 0:2], d4x[:, 2:4], ALU.max)
                TT(m1p[:, 0], m2x[:, 0], m2x[:, 1], ALU.max)
                if pi in MIN_ON_PE:
                    pe_frame(-j, -i, -k, ALU.min, m1p[:, 1])
                else:
                    d4 = trans.tile([P, C, DL, W], bf16, tag="d4")
                    TT(d4[:], cv(imgb, 0, 0), cv(img_h[-j], -i, -k),
                       ALU.subtract)
                    m2n = trans.tile([P, 2, DL, W], bf16, tag="m2n")
                    TT(m2n[:], d4[:, 0:2], d4[:, 2:4], ALU.min)
                    TT(m1p[:, 1], m2n[:, 0], m2n[:, 1], ALU.min)
            elif T_TREEBATCH and pi in MIN_ON_PE:
                pe_frame_pair(i, j, k, m1p)
            else:
                pe_frame(j, i, k, ALU.max, m1p[:, 0])
                if pi in MIN_ON_PE:
                    # sign-free: min tree of Sh_-j view(-i,-k) - center
                    pe_frame(-j, -i, -k, ALU.min, m1p[:, 1])
                else:
                    d4 = trans.tile([P, C, DL, W], bf16, tag="d4")
                    TT(d4[:], cv(imgb, 0, 0), cv(img_h[-j], -i, -k),
                       ALU.subtract)
                    m2n = trans.tile([P, 2, DL, W], bf16, tag="m2n")
                    TT(m2n[:], d4[:, 0:2], d4[:, 2:4], ALU.min)
                    TT(m1p[:, 1], m2n[:, 0], m2n[:, 1], ALU.min)

        def stage_B(pi):
            # u for both frames in one activation (const 2/sqrt(pi) cancels;
            # exp(-r2/2) lives in the scaled identity used by the acc matmuls)
            up = upool.tile([P, 2, DL, W], bf16, tag="up")
            up_t[pi] = up
            if pi == PAIR_ORDER[-1]:
                cph['gate'] = up
            m1p = m1p_t.pop(pi)
            if USE_DERF:
                nc.scalar.activation(up[:], m1p[:], AF.Derivative_Erf,
                                     scale=SQ2I)
            else:
                sqp = trans.tile([P, 2, DL, W], bf16, tag="sqp")
                nc.scalar.activation(sqp[:], m1p[:], AF.Square)
                nc.scalar.activation(up[:], sqp[:], AF.Exp, scale=-0.5)

        def stage_P(pi):
            i, j, k = PAIRS[pi]
            up = up_t[pi]
            pr2 = trans.tile([P, 2, 3, DL, W], bf16, tag="prods",
                             name="pr2", bufs=C_LAG + 2)
            prods_t[pi] = pr2
            for fr, sgn in ((0, 1), (1, -1)):
                si, sj, sk = sgn * i, sgn * j, sgn * k
                mview = cv(msk_h[sj], si, sk)
                ub = up[:, fr:fr + 1].broadcast_to((P, 3, DL, W))
                if (pi, fr) in T_PRODS_SPLIT:
                    # channel-split: 2ch on DVE, 1ch on Pool (finer quanta)
                    ub2 = up[:, fr:fr + 1].broadcast_to((P, 2, DL, W))
                    TT(pr2[:, fr, 0:2], ub2, mview[:, 0:2], ALU.mult)
                    ub1 = up[:, fr:fr + 1].broadcast_to((P, 1, DL, W))
                    nc.gpsimd.tensor_tensor(pr2[:, fr, 2:3], ub1,
                                            mview[:, 2:3], ALU.mult)
                else:
                    eng = (nc.gpsimd if (pi, fr) in T_PRODS_POOL
                           else nc.vector)
                    eng.tensor_tensor(pr2[:, fr], ub, mview, ALU.mult)

        def stage_C(pi):
            i, j, k = PAIRS[pi]
            r2 = float(i * i + j * j + k * k)
            st, sp = (pi == PAIR_ORDER[0]), (pi == PAIR_ORDER[-1])
            up = up_t.pop(pi)
            pr2 = prods_t.pop(pi)
            for fr in range(2):
                nc.tensor.matmul(SP[:], ir2[r2], up[:, fr],
                                 start=(st and fr == 0), stop=(sp and fr == 1))
                for ci in range(3):
                    nc.tensor.matmul(accP[:, ci], ir2[r2], pr2[:, fr, ci],
                                     start=(st and fr == 0),
                                     stop=(sp and fr == 1))

        NP = len(PAIR_ORDER)
        for idx in range(NP + max(B_LAG, C_LAG)):
            if idx == CPHASE_AT:
                emit_cphase_front()
            if idx == 0:
                emit_masks()
            if idx < NP:
                stage_A(PAIR_ORDER[idx])
            if idx == 0:
                emit_hshifts()
            if B_LAG <= idx < NP + B_LAG:
                stage_B(PAIR_ORDER[idx - B_LAG])
            if P_LAG <= idx < NP + P_LAG:
                stage_P(PAIR_ORDER[idx - P_LAG])
            if C_LAG <= idx < NP + C_LAG:
                stage_C(PAIR_ORDER[idx - C_LAG])
            if idx == CACT_AT and not exp_late:
                emit_cphase_act()
        if CPHASE_AT >= NP:
            emit_cphase_front()
        if not LSE_EARLY:
            emit_cphase_back()

        # ---- tail: p_w = sum(-0.5/S * sum_c dx_c*acc_c) ----
        dxb = cph['dxb']
        rS = cpool.tile([P, DL, W], f32, tag="rS")
        nc.vector.reciprocal_approx_fast(rS[:], SP[:])
        tp = cpool.tile([P, 3, DL, W], bf16, tag="tp")
        if T_TAILSPLIT:
            for ci in range(3):
                TT(tp[:, ci], accP[:, ci], dxb[:, ci], ALU.mult)
        else:
            TT(tp[:], accP[:], dxb[:], ALU.mult)
        t1 = cpool.tile([P, DL, W], bf16, tag="t1")
        TT(t1[:], tp[:, 0], tp[:, 1], ALU.add)
        t2 = cpool.tile([P, DL, W], bf16, tag="t2")
        TT(t2[:], t1[:], tp[:, 2], ALU.add)
        nc.vector.scalar_tensor_tensor(scr3[:], t2[:], -0.5, rS[:],
                                       ALU.mult, ALU.mult,
                                       accum_out=pl[:, 2:3])
        nc.sync.dma_start(out_d[:, :], pl[:])

    nc.compile()
    return nc


def _get_nc():
    if "nc" not in _CACHED:
        _CACHED["nc"] = _build_nc()
    return _CACHED["nc"]


def make_in_maps(inputs, labels, images):
    """Host-side shard: full inputs -> per-core input dicts (layout prep:
    (b,h)->partition transpose, d/w halo padding, bf16 pre-cast)."""
    import ml_dtypes

    bf = ml_dtypes.bfloat16
    img = np.asarray(images, np.float32).astype(bf)
    lab = np.asarray(labels).astype(bf)  # values 0..3, exact in bf16
    lgt = np.ascontiguousarray(np.asarray(inputs, np.float32))

    img_p = np.pad(img, ((0, 0), (0, 0), (1, 1), (0, 0), (1, 1)), mode="edge")
    lab_p = np.pad(lab, ((0, 0), (1, 1), (0, 0), (1, 1)), mode="edge")

    in_maps = []
    for k in range(NCORES):
        d0 = k * DL
        ic = img_p[:, :, d0:d0 + DE]          # [2,4,10,64,66]
        lc = lab_p[:, d0:d0 + DE]             # [2,10,64,66]
        xc = lgt[:, :, d0:d0 + DL]            # [2,4,8,64,64]
        im = np.ascontiguousarray(ic.transpose(0, 3, 1, 2, 4)).reshape(P, -1)
        lm = np.ascontiguousarray(lc.transpose(0, 2, 1, 3)).reshape(P, -1)
        xm = np.ascontiguousarray(
            xc.transpose(0, 3, 1, 2, 4)).reshape(P, -1).astype(bf)
        in_maps.append({"img": im, "lab": lm, "logits": xm, "eye": _mats()})
    return in_maps


def _mats():
    """[-I, Sh+-1, 3x scaled I, -Sh+-1, I] as one [P, 9P] bf16
    array. Sh(j)[k, m] = 1 iff k = b(m)*64 + clamp(h(m)+j, 0, 63)."""
    import ml_dtypes

    eye = np.eye(P, dtype=np.float32)
    sh = {}
    for jj in (1, -1):
        M = np.zeros((P, P), np.float32)
        for m in range(P):
            b, h = divmod(m, 64)
            M[b * 64 + min(max(h + jj, 0), 63), m] = 1.0
        sh[jj] = M
    blocks = [-eye, sh[1], sh[-1],
              np.exp(-0.5) * eye, np.exp(-1.0) * eye, np.exp(-1.5) * eye]
    if T_TREEBATCH:
        blocks += [-sh[1], -sh[-1], eye]
    out = np.concatenate(blocks, axis=1)
    return np.ascontiguousarray(out).astype(ml_dtypes.bfloat16)


def kernel(inputs, labels, images):
    from concourse.bass_utils import run_bass_kernel_spmd

    nc = _get_nc()
    in_maps = make_in_maps(inputs, labels, images)
    res = run_bass_kernel_spmd(nc, in_maps, core_ids=list(range(NCORES)))
    total = 0.0
    for k in range(NCORES):
        pl = res.results[k]["partials"].astype(np.float64)
        ym_scale = -1.0 if USE_XB else 0.5
        total += (pl[:, 0] - ym_scale * pl[:, 1] - pl[:, 3] + pl[:, 2]).sum()
    return np.float32(total / NVOX)



# revision 9
# speedup vs baseline: 1.0879x; 1.0036x over previous
"""Trainium2 Bass kernel for nn_CELossWithSVLS_VE (SVLS cross-entropy loss).

Math (derived + numerically validated vs reference):
  For the 26 non-center offsets n, with per-voxel
    u_n = exp(-0.5*(maxdiff_n^2 + r_n^2)),
    maxdiff_n(v) = max_c(img_c(v+n) - img_c(v))   (replicate-padded),
  the SVLS label weights reduce EXACTLY to w_center = 1/2, w_n = u_n/(2S),
  S = sum_n u_n.  Then
    loss(v) = lse(v) - 0.5*x_{l(v)}(v) - (1/(2S)) * sum_n u_n * x_{l(v+n)}(v)
  and the output is mean_v loss(v).

Engine plan (vs the 93.7us baseline):
  * u_n via ONE ScalarE activation: Derivative_Erf(m/sqrt2) = c*exp(-m^2/2);
    c cancels in T/S, and exp(-r2/2) moves into r2-scaled identity stationary
    matrices used by the PE accumulation matmuls (no bias/second activation).
  * most 4-channel stencil subtractions run on the PE as shift-matrix matmul
    pairs into PSUM; ScalarE copies PSUM->SBUF bf16 (the only engine that can
    get PSUM data back cheaply); DVE does only max/min trees + mask products.
  * loss folds into 3 per-partition accumulators (p_lse, p_yx, p_w) via
    accum_out side outputs; host combines  sum = p_lse - 0.5*p_yx + p_w.
    The T-dot reads accP straight out of PSUM (single-PSUM-operand TT).

Sharding: 8 cores, core k takes d-slab [8k, 8k+8) of both batches.
On-core layout: partition p = b*64 + h (128), free = (c?, d, w) with d,w
halos in SBUF.  h+-1 stencil shifts: PE shift-matrix matmuls (edge clamp
baked in) or partition-shifted SBUF DMA copies for the DVE-path frames.
"""
import sys
from contextlib import ExitStack

import numpy as np

if "/opt/trn_rl_repo" not in sys.path:
    sys.path.insert(0, "/opt/trn_rl_repo")

B, C, D, H, W = 2, 4, 64, 64, 64
NCORES = 8
DL = D // NCORES          # 8 local d-planes
DE, WE = DL + 2, W + 2    # 10, 66 (d/w halos)
P = 128                   # partitions = (b, h)
NVOX = B * D * H * W      # 524288

# 13 positive offsets; r2 = i*i+j*j+k*k.
PAIRS = [
    (1, 0, 0), (0, 0, 1), (1, 0, 1), (1, 0, -1),
    (0, 1, 0), (1, 1, 0), (1, -1, 0), (0, 1, 1), (0, 1, -1),
    (1, 1, 1), (1, 1, -1), (1, -1, 1), (1, -1, -1),
]

import os as _os, json as _json
_ov = _json.loads(_os.environ.get("KCONF", "{}"))
T_J0POOL = set(_ov.get("j0pool", []))     # j0 pairs: m2 on Pool
T_MINPOOL = set(_ov.get("minpool", []))   # DVE-min-path pairs: m2n on Pool
T_MASKS_DVE = _ov.get("masks_dve", 0)
T_EXPSUM_POOL = _ov.get("expsum_pool", 1)
T_PRODS_POOL = {tuple(t) for t in _ov.get("prods_pool",
                                           [[4, 1], [10, 1], [12, 1],
                                            [6, 1], [5, 1], [11, 1],
                                            [1, 1], [0, 1], [2, 1]])}
T_PSPLIT = _ov.get("psplit", 1)           # prods stage one slot early
T_MAXDVE = set(_ov.get("maxdve", [8, 9]))  # pairs: max-frame off PE
T_MINOFF = set(_ov.get("minoff", []))     # pairs removed from MIN_ON_PE
T_X0DVE = _ov.get("x0dve", 0)             # p_x0 accum via DVE tensor_scalar
T_YMPOOL = _ov.get("ympool", 0)           # ym STT on Pool (NO: won't compile)
T_YMSPLIT = _ov.get("ymsplit", 0)         # ym: Pool mult + DVE 4x ts-accum
T_TAILSPLIT = _ov.get("tailsplit", 0)     # per-channel tail tp
T_DMAQ = _ov.get("dmaq", 0)               # mats/labf on Act DGE queue
T_DMAORD = _ov.get("dmaord", 0)           # 1: labf,img,mats,x  2: img01 first
T_SUB1CH = _ov.get("sub1ch", 0)           # pair-0 sub split per channel
T_TBUFS = _ov.get("tbufs", 3)             # trans tile-pool ring depth
T_CBUFS = _ov.get("cbufs", 6)             # cb-ring depth override
T_M2BUFS = _ov.get("m2bufs", 0)           # m2-ring depth override
T_UBUFS = _ov.get("ubufs", 0)             # upool override (0: formula)
T_TREEBATCH = _ov.get("treebatch", 0)     # pair-batched trees on PE pairs
T_EXPTOKEN = _ov.get("exptoken", 1)       # gate exp/ln after last DErf
T_PRODS_SPLIT = {tuple(t) for t in _ov.get("prods_split",
                                            [[7, 0], [8, 0], [9, 0],
                                             [2, 0], [3, 0], [7, 1]])}
# ---- schedule config ----
USE_DERF = True
# j!=0 pairs whose min-frame runs on PE (rest: DVE sub via h-shifted copies)
MIN_ON_PE = {10, 11, 12, 4, 5, 6, 7} - T_MINOFF
# emission order (j0 pairs interleaved between PE-heavy pairs)
PAIR_ORDER = _ov.get("order", [0, 4, 5, 1, 10, 11, 2, 12, 6, 3, 7, 8, 9])
CPHASE_AT = _ov.get("cphase", 5)
CACT_AT = _ov.get("cact", 3)
LSE_EARLY = False
EXP_LATE = False  # overridden to True by T_EXPTOKEN at build time
USE_XB = False
USE_RSB = False
TTR_YM = False    # tensor_tensor_reduce compiles but faults at runtime
TTR_TAIL = False
USE_POOLOPS = True
CB_FULL = False
B_LAG = _ov.get("blag", 3)
C_LAG = _ov.get("clag", 3)

_CACHED = {}

SQ2I = 0.7071067811865476  # 1/sqrt(2)


def _build_nc():
    import concourse.bacc as bacc
    import concourse.mybir as mybir
    import concourse.tile as tile

    AF = mybir.ActivationFunctionType
    ALU = mybir.AluOpType
    dt = mybir.dt

    nc = bacc.Bacc("TRN2", target_bir_lowering=False, debug=False,
                   num_devices=NCORES)
    img_d = nc.dram_tensor("img", [P, C * DE * WE], dt.bfloat16,
                           kind="ExternalInput")
    lab_d = nc.dram_tensor("lab", [P, DE * WE], dt.bfloat16,
                           kind="ExternalInput")
    logit_d = nc.dram_tensor("logits", [P, C * DL * W], dt.bfloat16,
                             kind="ExternalInput")
    # mats: [-I, Sh(+1), Sh(-1), I*e^-.5, I*e^-1, I*e^-1.5,
    eye_d = nc.dram_tensor("eye", [P, (9 if T_TREEBATCH else 6) * P],
                           dt.bfloat16, kind="ExternalInput")
    out_d = nc.dram_tensor("partials", [P, 4], dt.float32,
                           kind="ExternalOutput")

    import concourse.bass as bass_mod

    exp_late = EXP_LATE or bool(T_EXPTOKEN)
    with tile.TileContext(nc) as tc, ExitStack() as ctx:
        persist = ctx.enter_context(tc.tile_pool(name="persist", bufs=1))
        cpool = ctx.enter_context(tc.tile_pool(name="cpool", bufs=1))
        trans = ctx.enter_context(tc.tile_pool(name="trans", bufs=T_TBUFS))
        upool = ctx.enter_context(
            tc.tile_pool(name="upool",
                         bufs=T_UBUFS or max(3, B_LAG + 1,
                                             C_LAG - B_LAG + 2)))
        psum = ctx.enter_context(
            tc.tile_pool(name="psum", bufs=1, space=bass_mod.MemorySpace.PSUM))
        psum2 = ctx.enter_context(
            tc.tile_pool(name="psum2", bufs=(1 if CB_FULL else 2),
                         space=bass_mod.MemorySpace.PSUM))

        f32, bf16 = dt.float32, dt.bfloat16
        TT = nc.vector.tensor_tensor

        # ---- loads (images/labels arrive pre-cast to bf16 from host) ----
        # The DMA engines serialize transfers, so order by first use: mats
        # (PE idles until it lands), labf (masks), imgb per-channel, then the
        # big f32 logits tensor (only needed once the Act exp work starts).
        mats = persist.tile([P, 9 if T_TREEBATCH else 6, P], bf16,
                            tag="mats")
        labf = persist.tile([P, DE, WE], bf16, tag="labf")
        imgb = persist.tile([P, C, DE, WE], bf16, tag="imgb")
        x = persist.tile([P, C, DL, W], bf16, tag="x")

        def dma_img(c0, c1):
            for c in range(c0, c1):
                nc.sync.dma_start(imgb[:, c],
                                  img_d[:, c * DE * WE:(c + 1) * DE * WE])

        if T_DMAORD == 0:
            (nc.scalar if T_DMAQ else nc.sync).dma_start(mats[:], eye_d[:, :])
            dma_img(0, C)
            (nc.scalar if T_DMAQ else nc.sync).dma_start(labf[:], lab_d[:, :])
            nc.sync.dma_start(x[:], logit_d[:, :])
        elif T_DMAORD == 1:
            nc.sync.dma_start(labf[:], lab_d[:, :])
            dma_img(0, C)
            nc.sync.dma_start(mats[:], eye_d[:, :])
            nc.sync.dma_start(x[:], logit_d[:, :])
        elif T_DMAORD == 2:
            dma_img(0, 2)
            nc.sync.dma_start(labf[:], lab_d[:, :])
            dma_img(2, C)
            nc.sync.dma_start(mats[:], eye_d[:, :])
            nc.sync.dma_start(x[:], logit_d[:, :])
        else:  # 3: ch0,ch1 first, mats third, then labf/ch23/x
            dma_img(0, 2)
            nc.sync.dma_start(mats[:], eye_d[:, :])
            nc.sync.dma_start(labf[:], lab_d[:, :])
            dma_img(2, C)
            nc.sync.dma_start(x[:], logit_d[:, :])

        negI = mats[:, 0]
        shm = {1: mats[:, 1], -1: mats[:, 2]}
        ir2 = {1.0: mats[:, 3], 2.0: mats[:, 4], 3.0: mats[:, 5]}
        if T_TREEBATCH:
            nshm = {1: mats[:, 6], -1: mats[:, 7]}
            posI = mats[:, 8]

        masks = persist.tile([P, 3, DE, WE], bf16, tag="masks")

        def emit_masks():
            eng = nc.vector if T_MASKS_DVE else (
                nc.gpsimd if USE_POOLOPS else nc.vector)
            for ci, cval in enumerate((1.0, 2.0, 3.0)):
                eng.tensor_scalar(masks[:, ci], labf[:], cval, None,
                                  ALU.is_equal)

        # ---- h-shifted copies (partition shift via SBUF->SBUF DMA).
        def hshift_copies(dst_p, dst_m, src, eng):
            eng.dma_start(dst_p[0:63], src[1:64])
            eng.dma_start(dst_p[64:127], src[65:128])
            eng.dma_start(dst_p[63:64], src[63:64])
            eng.dma_start(dst_p[127:128], src[127:128])
            eng.dma_start(dst_m[1:64], src[0:63])
            eng.dma_start(dst_m[65:128], src[64:127])
            eng.dma_start(dst_m[0:1], src[0:1])
            eng.dma_start(dst_m[64:65], src[64:65])

        # masks_h before imgb_h: first mask-product use is much earlier than
        # the first DVE-path min-frame. SP ring so Act SEQ never blocks.
        masks_hp = persist.tile([P, 3, DE, WE], bf16, tag="masks_hp")
        masks_hm = persist.tile([P, 3, DE, WE], bf16, tag="masks_hm")
        msk_h = {1: masks_hp, 0: masks, -1: masks_hm}
        need_imgb_h = len(MIN_ON_PE) < 9 or len(T_MAXDVE) > 0
        if need_imgb_h:
            imgb_hp = persist.tile([P, C, DE, WE], bf16, tag="imgb_hp")
            imgb_hm = persist.tile([P, C, DE, WE], bf16, tag="imgb_hm")
            img_h = {1: imgb_hp, 0: imgb, -1: imgb_hm}

        def emit_hshifts():
            hshift_copies(masks_hp, masks_hm, masks, nc.sync)
            if need_imgb_h:
                hshift_copies(imgb_hp, imgb_hm, imgb, nc.sync)

        def cv(tile_, i, k):
            """center view shifted by (i, ., k) of a [..., DE, WE] tile."""
            return tile_[:, :, 1 + i:1 + i + DL, 1 + k:1 + k + W]

        # ---- PSUM accumulators; PE accumulates via r2-scaled identities ----
        accP = psum.tile([P, 3, DL, W], f32, tag="accP")
        SP = psum.tile([P, DL, W], f32, tag="SP")

        pl = cpool.tile([P, 4], f32, tag="pl")
        scr1 = cpool.tile([P, DL, W], f32, tag="scr1")
        scr2 = cpool.tile([P, DL, W], f32, tag="scr2")
        scr3 = cpool.tile([P, DL, W], f32, tag="scr3")

        cph = {}

        def emit_cphase_act():
            # exp-set work up front while PE/DVE wind up; p_x0 on the side
            if T_EXPTOKEN:
                # 1-element in-place bypass on x gated by the last pair's
                # DErf output: orders every x-reader emitted below (exp,
                # x0-accum) after the whole DErf block, so the act table
                # never leaves set 17 mid-stream (the readiness scheduler
                # would otherwise hoist exp into the DErf stream).
                TT(x[:, 0:1, 0:1, 0:1], x[:, 0:1, 0:1, 0:1],
                   cph['gate'][:, 0:1, 0:1, 0:1], ALU.bypass)
            if USE_XB:
                xb = cpool.tile([P, C, DL, W], bf16, tag="xb")
                nc.scalar.activation(xb[:], x[:], AF.Copy, scale=-0.5)
                cph.update(xb=xb)
            expx = cpool.tile([P, C, DL, W], bf16, tag="expx")
            nc.scalar.activation(expx[:], x[:], AF.Exp)
            if T_X0DVE:
                nc.vector.tensor_scalar(scr1[:], x[:, 0], 1.0, None,
                                        ALU.mult, accum_out=pl[:, 3:4])
            else:
                nc.scalar.activation(scr1[:], x[:, 0], AF.Copy,
                                     accum_out=pl[:, 3:4])
            cph.update(expx=expx)

        def emit_expsum():
            expx = cph['expx']
            e2 = cpool.tile([P, 2, DL, W], bf16, tag="e2")
            ene = nc.gpsimd if T_EXPSUM_POOL else nc.vector
            ene.tensor_tensor(e2[:], expx[:, 0:2], expx[:, 2:4], ALU.add)
            esum = cpool.tile([P, DL, W], bf16, tag="esum")
            ene.tensor_tensor(esum[:], e2[:, 0], e2[:, 1], ALU.add)
            cph.update(esum=esum)

        def emit_cphase_front():
            # DVE part: dxb; fused p_ym = sum(m_c * dx_c)
            if not exp_late:
                emit_expsum()
            xs = cph['xb'] if USE_XB else x
            dxb = cpool.tile([P, 3, DL, W], bf16, tag="dxb")
            TT(dxb[:], xs[:, 1:4], xs[:, 0:1].broadcast_to((P, 3, DL, W)),
               ALU.subtract)
            # p_ym = sum over (c,d,w) of m_c*dx_c: one stt dot with accum
            # (needs a contiguous mask-center copy; Pool makes it for free)
            mc = cpool.tile([P, 3, DL, W], bf16, tag="mc")
            nc.gpsimd.tensor_copy(mc[:], cv(masks, 0, 0))
            ym = cpool.tile([P, 3, DL, W], bf16, tag="ym")
            if T_YMSPLIT:
                nc.gpsimd.tensor_tensor(ym[:], mc[:], dxb[:], ALU.mult)
                scry = cpool.tile([P, 3, DL, W], bf16, tag="scry")
                nc.vector.tensor_scalar(scry[:], ym[:], 1.0, None,
                                        ALU.mult, accum_out=pl[:, 1:2])
            else:
                yme = nc.gpsimd if T_YMPOOL else nc.vector
                yme.scalar_tensor_tensor(ym[:], mc[:], 1.0, dxb[:],
                                         ALU.mult, ALU.mult,
                                         accum_out=pl[:, 1:2])
            cph.update(dxb=dxb)
            if LSE_EARLY:
                emit_cphase_back()

        def emit_cphase_back():
            # p_lse: one act-table switch back to the ln/exp set
            if exp_late:
                emit_cphase_act()
                emit_expsum()
            nc.scalar.activation(scr2[:], cph['esum'], AF.Ln,
                                 accum_out=pl[:, 0:1])

        def pe_frame_pair(i, j, k, m1p):
            """Both frames of a PE pair with batched trees: the min-frame
            uses negated stationaries (-Sh, +I) so BOTH trees are max-trees
            (Derivative_Erf is even), letting m2/m1p batch across frames."""
            cb2 = trans.tile([P, 2, C, DL, W], bf16, tag="cb2", name="cb2")
            for fr, sgn in ((0, 1), (1, -1)):
                si, sk = sgn * i, sgn * k
                st1 = shm[sgn * j] if fr == 0 else nshm[sgn * j]
                st2 = negI if fr == 0 else posI
                for half in range(2):
                    d4p = psum2.tile([P, 2, DL, W], f32, tag="d4p",
                                     name="d4p")
                    for cc in range(2):
                        c = 2 * half + cc
                        nc.tensor.matmul(d4p[:, cc], st1,
                                         imgb[:, c, 1 + si:1 + si + DL,
                                              1 + sk:1 + sk + W],
                                         start=True, stop=False)
                        nc.tensor.matmul(d4p[:, cc], st2,
                                         imgb[:, c, 1:1 + DL, 1:1 + W],
                                         start=False, stop=True)
                    nc.scalar.copy(cb2[:, fr, 2 * half:2 * half + 2],
                                   d4p[:])
            m2b = trans.tile([P, 2, 2, DL, W], bf16, tag="m2b", name="m2b")
            TT(m2b[:], cb2[:, :, 0:2], cb2[:, :, 2:4], ALU.max)
            TT(m1p[:], m2b[:, :, 0], m2b[:, :, 1], ALU.max)

        def pe_frame(jj, ii, kk, mop, m1p_slot):
            """d4 = Sh_jj.T@view(ii,kk) - center on PE; Act copyback halves;
            DVE tree into m1p_slot."""
            cb = trans.tile([P, C, DL, W], bf16, tag="cb", name="cb",
                            bufs=T_CBUFS or None)
            if CB_FULL:
                d4p = psum2.tile([P, C, DL, W], f32, tag="d4p")
                for c in range(C):
                    nc.tensor.matmul(d4p[:, c], shm[jj],
                                     imgb[:, c, 1 + ii:1 + ii + DL,
                                          1 + kk:1 + kk + W],
                                     start=True, stop=False)
                    nc.tensor.matmul(d4p[:, c], negI,
                                     imgb[:, c, 1:1 + DL, 1:1 + W],
                                     start=False, stop=True)
                nc.scalar.copy(cb[:], d4p[:])
            else:
                for half in range(2):
                    d4p = psum2.tile([P, 2, DL, W], f32, tag="d4p")
                    for cc in range(2):
                        c = 2 * half + cc
                        nc.tensor.matmul(d4p[:, cc], shm[jj],
                                         imgb[:, c, 1 + ii:1 + ii + DL,
                                              1 + kk:1 + kk + W],
                                         start=True, stop=False)
                        nc.tensor.matmul(d4p[:, cc], negI,
                                         imgb[:, c, 1:1 + DL, 1:1 + W],
                                         start=False, stop=True)
                    nc.scalar.copy(cb[:, 2 * half:2 * half + 2], d4p[:])
            m2 = trans.tile([P, 2, DL, W], bf16, tag="m2")
            TT(m2[:], cb[:, 0:2], cb[:, 2:4], mop)
            TT(m1p_slot, m2[:, 0], m2[:, 1], mop)

        # ---- software-pipelined main loop over offset pairs ----
        m1p_t, up_t, prods_t = {}, {}, {}
        P_LAG = max(B_LAG, C_LAG - T_PSPLIT)

        def stage_A(pi):
            i, j, k = PAIRS[pi]
            m1p = upool.tile([P, 2, DL, W], bf16, tag="m1p")
            m1p_t[pi] = m1p
            if j == 0:
                # single sub on an extended box serves both frames as views
                nd, nw = (9 if i else 8), (65 if k else 64)
                d0, w0 = (0 if i == 1 else 1), (0 if k == 1 else 1)
                dpe = trans.tile([P, C, nd, nw], bf16, tag="dpe")
                if pi == PAIR_ORDER[0]:
                    step = 1 if T_SUB1CH else 2
                    for ch in range(0, C, step):
                        TT(dpe[:, ch:ch + step],
                           imgb[:, ch:ch + step, d0 + i:d0 + i + nd,
                                w0 + k:w0 + k + nw],
                           imgb[:, ch:ch + step, d0:d0 + nd, w0:w0 + nw],
                           ALU.subtract)
                else:
                    TT(dpe[:],
                       imgb[:, :, d0 + i:d0 + i + nd, w0 + k:w0 + k + nw],
                       imgb[:, :, d0:d0 + nd, w0:w0 + nw], ALU.subtract)
                for fr in range(2):
                    ds = 1 - d0 - (i if fr else 0)
                    ws = 1 - w0 - (k if fr else 0)
                    mop = ALU.max if fr == 0 else ALU.min
                    dv = dpe[:, :, ds:ds + DL, ws:ws + W]
                    m2 = trans.tile([P, 2, DL, W], bf16, tag="m2",
                                    name="m2", bufs=T_M2BUFS or None)
                    TT(m2[:], dv[:, 0:2], dv[:, 2:4], mop)
                    TT(m1p[:, fr], m2[:, 0], m2[:, 1], mop)
            elif pi in T_MAXDVE:
                d4x = trans.tile([P, C, DL, W], bf16, tag="d4x", name="d4x")
                TT(d4x[:], cv(img_h[j], i, k), cv(imgb, 0, 0), ALU.subtract)
                m2x = trans.tile([P, 2, DL, W], bf16, tag="m2x", name="m2x")
                TT(m2x[:], d4x[:, 0:2], d4x[:, 2:4], ALU.max)
                TT(m1p[:, 0], m2x[:, 0], m2x[:, 1], ALU.max)
                if pi in MIN_ON_PE:
                    pe_frame(-j, -i, -k, ALU.min, m1p[:, 1])
                else:
                    d4 = trans.tile([P, C, DL, W], bf16, tag="d4")
                    TT(d4[:], cv(imgb, 0, 0), cv(img_h[-j], -i, -k),
                       ALU.subtract)
                    m2n = trans.tile([P, 2, DL, W], bf16, tag="m2n")
                    TT(m2n[:], d4[:, 0:2], d4[:, 2:4], ALU.min)
                    TT(m1p[:, 1], m2n[:, 0], m2n[:, 1], ALU.min)
            elif T_TREEBATCH and pi in MIN_ON_PE:
                pe_frame_pair(i, j, k, m1p)
            else:
                pe_frame(j, i, k, ALU.max, m1p[:, 0])
                if pi in MIN_ON_PE:
                    # sign-free: min tree of Sh_-j view(-i,-k) - center
                    pe_frame(-j, -i, -k, ALU.min, m1p[:, 1])
                else:
                    d4 = trans.tile([P, C, DL, W], bf16, tag="d4")
                    TT(d4[:], cv(imgb, 0, 0), cv(img_h[-j], -i, -k),
                       ALU.subtract)
                    m2n = trans.tile([P, 2, DL, W], bf16, tag="m2n")
                    TT(m2n[:], d4[:, 0:2], d4[:, 2:4], ALU.min)
                    TT(m1p[:, 1], m2n[:, 0], m2n[:, 1], ALU.min)

        def stage_B(pi):
            # u for both frames in one activation (const 2/sqrt(pi) cancels;
            # exp(-r2/2) lives in the scaled identity used by the acc matmuls)
            up = upool.tile([P, 2, DL, W], bf16, tag="up")
            up_t[pi] = up
            if pi == PAIR_ORDER[-1]:
                cph['gate'] = up
            m1p = m1p_t.pop(pi)
            if USE_DERF:
                nc.scalar.activation(up[:], m1p[:], AF.Derivative_Erf,
                                     scale=SQ2I)
            else:
                sqp = trans.tile([P, 2, DL, W], bf16, tag="sqp")
                nc.scalar.activation(sqp[:], m1p[:], AF.Square)
                nc.scalar.activation(up[:], sqp[:], AF.Exp, scale=-0.5)

        def stage_P(pi):
            i, j, k = PAIRS[pi]
            up = up_t[pi]
            pr2 = trans.tile([P, 2, 3, DL, W], bf16, tag="prods",
                             name="pr2", bufs=C_LAG + 2)
            prods_t[pi] = pr2
            for fr, sgn in ((0, 1), (1, -1)):
                si, sj, sk = sgn * i, sgn * j, sgn * k
                mview = cv(msk_h[sj], si, sk)
                ub = up[:, fr:fr + 1].broadcast_to((P, 3, DL, W))
                if (pi, fr) in T_PRODS_SPLIT:
                    # channel-split: 2ch on DVE, 1ch on Pool (finer quanta)
                    ub2 = up[:, fr:fr + 1].broadcast_to((P, 2, DL, W))
                    TT(pr2[:, fr, 0:2], ub2, mview[:, 0:2], ALU.mult)
                    ub1 = up[:, fr:fr + 1].broadcast_to((P, 1, DL, W))
                    nc.gpsimd.tensor_tensor(pr2[:, fr, 2:3], ub1,
                                            mview[:, 2:3], ALU.mult)
                else:
                    eng = (nc.gpsimd if (pi, fr) in T_PRODS_POOL
                           else nc.vector)
                    eng.tensor_tensor(pr2[:, fr], ub, mview, ALU.mult)

        def stage_C(pi):
            i, j, k = PAIRS[pi]
            r2 = float(i * i + j * j + k * k)
            st, sp = (pi == PAIR_ORDER[0]), (pi == PAIR_ORDER[-1])
            up = up_t.pop(pi)
            pr2 = prods_t.pop(pi)
            for fr in range(2):
                nc.tensor.matmul(SP[:], ir2[r2], up[:, fr],
                                 start=(st and fr == 0), stop=(sp and fr == 1))
                for ci in range(3):
                    nc.tensor.matmul(accP[:, ci], ir2[r2], pr2[:, fr, ci],
                                     start=(st and fr == 0),
                                     stop=(sp and fr == 1))

        NP = len(PAIR_ORDER)
        for idx in range(NP + max(B_LAG, C_LAG)):
            if idx == CPHASE_AT:
                emit_cphase_front()
            if idx == 0:
                emit_masks()
            if idx < NP:
                stage_A(PAIR_ORDER[idx])
            if idx == 0:
                emit_hshifts()
            if B_LAG <= idx < NP + B_LAG:
                stage_B(PAIR_ORDER[idx - B_LAG])
            if P_LAG <= idx < NP + P_LAG:
                stage_P(PAIR_ORDER[idx - P_LAG])
            if C_LAG <= idx < NP + C_LAG:
                stage_C(PAIR_ORDER[idx - C_LAG])
            if idx == CACT_AT and not exp_late:
                emit_cphase_act()
        if CPHASE_AT >= NP:
            emit_cphase_front()
        if not LSE_EARLY:
            emit_cphase_back()

        # ---- tail: p_w = sum(-0.5/S * sum_c dx_c*acc_c) ----
        dxb = cph['dxb']
        rS = cpool.tile([P, DL, W], f32, tag="rS")
        nc.vector.reciprocal_approx_fast(rS[:], SP[:])
        tp = cpool.tile([P, 3, DL, W], bf16, tag="tp")
        if T_TAILSPLIT:
            for ci in range(3):
                TT(tp[:, ci], accP[:, ci], dxb[:, ci], ALU.mult)
        else:
            TT(tp[:], accP[:], dxb[:], ALU.mult)
        t1 = cpool.tile([P, DL, W], bf16, tag="t1")
        TT(t1[:], tp[:, 0], tp[:, 1], ALU.add)
        t2 = cpool.tile([P, DL, W], bf16, tag="t2")
        TT(t2[:], t1[:], tp[:, 2], ALU.add)
        nc.vector.scalar_tensor_tensor(scr3[:], t2[:], -0.5, rS[:],
                                       ALU.mult, ALU.mult,
                                       accum_out=pl[:, 2:3])
        nc.sync.dma_start(out_d[:, :], pl[:])

    nc.compile()
    return nc


def _get_nc():
    if "nc" not in _CACHED:
        _CACHED["nc"] = _build_nc()
    return _CACHED["nc"]


def make_in_maps(inputs, labels, images):
    """Host-side shard: full inputs -> per-core input dicts (layout prep:
    (b,h)->partition transpose, d/w halo padding, bf16 pre-cast)."""
    import ml_dtypes

    bf = ml_dtypes.bfloat16
    img = np.asarray(images, np.float32).astype(bf)
    lab = np.asarray(labels).astype(bf)  # values 0..3, exact in bf16
    lgt = np.ascontiguousarray(np.asarray(inputs, np.float32))

    img_p = np.pad(img, ((0, 0), (0, 0), (1, 1), (0, 0), (1, 1)), mode="edge")
    lab_p = np.pad(lab, ((0, 0), (1, 1), (0, 0), (1, 1)), mode="edge")

    in_maps = []
    for k in range(NCORES):
        d0 = k * DL
        ic = img_p[:, :, d0:d0 + DE]          # [2,4,10,64,66]
        lc = lab_p[:, d0:d0 + DE]             # [2,10,64,66]
        xc = lgt[:, :, d0:d0 + DL]            # [2,4,8,64,64]
        im = np.ascontiguousarray(ic.transpose(0, 3, 1, 2, 4)).reshape(P, -1)
        lm = np.ascontiguousarray(lc.transpose(0, 2, 1, 3)).reshape(P, -1)
        xm = np.ascontiguousarray(
            xc.transpose(0, 3, 1, 2, 4)).reshape(P, -1).astype(bf)
        in_maps.append({"img": im, "lab": lm, "logits": xm, "eye": _mats()})
    return in_maps


def _mats():
    """[-I, Sh+-1, 3x scaled I, -Sh+-1, I] as one [P, 9P] bf16
    array. Sh(j)[k, m] = 1 iff k = b(m)*64 + clamp(h(m)+j, 0, 63)."""
    import ml_dtypes

    eye = np.eye(P, dtype=np.float32)
    sh = {}
    for jj in (1, -1):
        M = np.zeros((P, P), np.float32)
        for m in range(P):
            b, h = divmod(m, 64)
            M[b * 64 + min(max(h + jj, 0), 63), m] = 1.0
        sh[jj] = M
    blocks = [-eye, sh[1], sh[-1],
              np.exp(-0.5) * eye, np.exp(-1.0) * eye, np.exp(-1.5) * eye]
    if T_TREEBATCH:
        blocks += [-sh[1], -sh[-1], eye]
    out = np.concatenate(blocks, axis=1)
    return np.ascontiguousarray(out).astype(ml_dtypes.bfloat16)


def kernel(inputs, labels, images):
    from concourse.bass_utils import run_bass_kernel_spmd

    nc = _get_nc()
    in_maps = make_in_maps(inputs, labels, images)
    res = run_bass_kernel_spmd(nc, in_maps, core_ids=list(range(NCORES)))
    total = 0.0
    for k in range(NCORES):
        pl = res.results[k]["partials"].astype(np.float64)
        ym_scale = -1.0 if USE_XB else 0.5
        total += (pl[:, 0] - ym_scale * pl[:, 1] - pl[:, 3] + pl[:, 2]).sum()
    return np.float32(total / NVOX)



# revision 10
# speedup vs baseline: 1.0895x; 1.0014x over previous
"""Trainium2 Bass kernel for nn_CELossWithSVLS_VE (SVLS cross-entropy loss).

Math (derived + numerically validated vs reference):
  For the 26 non-center offsets n, with per-voxel
    u_n = exp(-0.5*(maxdiff_n^2 + r_n^2)),
    maxdiff_n(v) = max_c(img_c(v+n) - img_c(v))   (replicate-padded),
  the SVLS label weights reduce EXACTLY to w_center = 1/2, w_n = u_n/(2S),
  S = sum_n u_n.  Then
    loss(v) = lse(v) - 0.5*x_{l(v)}(v) - (1/(2S)) * sum_n u_n * x_{l(v+n)}(v)
  and the output is mean_v loss(v).

Engine plan (vs the 93.7us baseline):
  * u_n via ONE ScalarE activation: Derivative_Erf(m/sqrt2) = c*exp(-m^2/2);
    c cancels in T/S, and exp(-r2/2) moves into r2-scaled identity stationary
    matrices used by the PE accumulation matmuls (no bias/second activation).
  * most 4-channel stencil subtractions run on the PE as shift-matrix matmul
    pairs into PSUM; ScalarE copies PSUM->SBUF bf16 (the only engine that can
    get PSUM data back cheaply); DVE does only max/min trees + mask products.
  * loss folds into 3 per-partition accumulators (p_lse, p_yx, p_w) via
    accum_out side outputs; host combines  sum = p_lse - 0.5*p_yx + p_w.
    The T-dot reads accP straight out of PSUM (single-PSUM-operand TT).

Sharding: 8 cores, core k takes d-slab [8k, 8k+8) of both batches.
On-core layout: partition p = b*64 + h (128), free = (c?, d, w) with d,w
halos in SBUF.  h+-1 stencil shifts: PE shift-matrix matmuls (edge clamp
baked in) or partition-shifted SBUF DMA copies for the DVE-path frames.
"""
import sys
from contextlib import ExitStack

import numpy as np

if "/opt/trn_rl_repo" not in sys.path:
    sys.path.insert(0, "/opt/trn_rl_repo")

B, C, D, H, W = 2, 4, 64, 64, 64
NCORES = 8
DL = D // NCORES          # 8 local d-planes
DE, WE = DL + 2, W + 2    # 10, 66 (d/w halos)
P = 128                   # partitions = (b, h)
NVOX = B * D * H * W      # 524288

# 13 positive offsets; r2 = i*i+j*j+k*k.
PAIRS = [
    (1, 0, 0), (0, 0, 1), (1, 0, 1), (1, 0, -1),
    (0, 1, 0), (1, 1, 0), (1, -1, 0), (0, 1, 1), (0, 1, -1),
    (1, 1, 1), (1, 1, -1), (1, -1, 1), (1, -1, -1),
]

import os as _os, json as _json
_ov = _json.loads(_os.environ.get("KCONF", "{}"))
T_J0POOL = set(_ov.get("j0pool", []))     # j0 pairs: m2 on Pool
T_MINPOOL = set(_ov.get("minpool", []))   # DVE-min-path pairs: m2n on Pool
T_MASKS_DVE = _ov.get("masks_dve", 0)
T_EXPSUM_POOL = _ov.get("expsum_pool", 1)
T_PRODS_POOL = {tuple(t) for t in _ov.get("prods_pool",
                                           [[4, 1], [10, 1], [12, 1],
                                            [6, 1], [5, 1], [11, 1],
                                            [1, 1], [0, 1], [2, 1]])}
T_PSPLIT = _ov.get("psplit", 1)           # prods stage one slot early
T_MAXDVE = set(_ov.get("maxdve", [8, 9]))  # pairs: max-frame off PE
T_MINOFF = set(_ov.get("minoff", []))     # pairs removed from MIN_ON_PE
T_X0DVE = _ov.get("x0dve", 0)             # p_x0 accum via DVE tensor_scalar
T_YMPOOL = _ov.get("ympool", 0)           # ym STT on Pool (NO: won't compile)
T_YMSPLIT = _ov.get("ymsplit", 0)         # ym: Pool mult + DVE 4x ts-accum
T_TAILSPLIT = _ov.get("tailsplit", 0)     # per-channel tail tp
T_DMAQ = _ov.get("dmaq", 0)               # mats/labf on Act DGE queue
T_DMAORD = _ov.get("dmaord", 0)           # 1: labf,img,mats,x  2: img01 first
T_SUB1CH = _ov.get("sub1ch", 0)           # pair-0 sub split per channel
T_TBUFS = _ov.get("tbufs", 3)             # trans tile-pool ring depth
T_CBUFS = _ov.get("cbufs", 8)             # cb-ring depth override
T_M2BUFS = _ov.get("m2bufs", 0)           # m2-ring depth override
T_BUFS = {k: int(v) for k, v in _ov.get("bufs", {}).items()}
T_UBUFS = _ov.get("ubufs", 0)             # upool override (0: formula)
T_TREEBATCH = _ov.get("treebatch", 0)     # pair-batched trees on PE pairs
T_EXPTOKEN = _ov.get("exptoken", 1)       # gate exp/ln after last DErf
T_PRODS_SPLIT = {tuple(t) for t in _ov.get("prods_split",
                                            [[7, 0], [8, 0], [9, 0],
                                             [2, 0], [3, 0], [7, 1]])}
# ---- schedule config ----
USE_DERF = True
# j!=0 pairs whose min-frame runs on PE (rest: DVE sub via h-shifted copies)
MIN_ON_PE = {10, 11, 12, 4, 5, 6, 7} - T_MINOFF
# emission order (j0 pairs interleaved between PE-heavy pairs)
PAIR_ORDER = _ov.get("order", [0, 4, 5, 1, 10, 11, 2, 12, 6, 3, 7, 8, 9])
CPHASE_AT = _ov.get("cphase", 5)
CACT_AT = _ov.get("cact", 3)
LSE_EARLY = False
EXP_LATE = False  # overridden to True by T_EXPTOKEN at build time
USE_XB = False
USE_RSB = False
TTR_YM = False    # tensor_tensor_reduce compiles but faults at runtime
TTR_TAIL = False
USE_POOLOPS = True
CB_FULL = False
B_LAG = _ov.get("blag", 3)
C_LAG = _ov.get("clag", 3)

_CACHED = {}

SQ2I = 0.7071067811865476  # 1/sqrt(2)


def _build_nc():
    import concourse.bacc as bacc
    import concourse.mybir as mybir
    import concourse.tile as tile

    AF = mybir.ActivationFunctionType
    ALU = mybir.AluOpType
    dt = mybir.dt

    nc = bacc.Bacc("TRN2", target_bir_lowering=False, debug=False,
                   num_devices=NCORES)
    img_d = nc.dram_tensor("img", [P, C * DE * WE], dt.bfloat16,
                           kind="ExternalInput")
    lab_d = nc.dram_tensor("lab", [P, DE * WE], dt.bfloat16,
                           kind="ExternalInput")
    logit_d = nc.dram_tensor("logits", [P, C * DL * W], dt.bfloat16,
                             kind="ExternalInput")
    # mats: [-I, Sh(+1), Sh(-1), I*e^-.5, I*e^-1, I*e^-1.5,
    eye_d = nc.dram_tensor("eye", [P, (9 if T_TREEBATCH else 6) * P],
                           dt.bfloat16, kind="ExternalInput")
    out_d = nc.dram_tensor("partials", [P, 4], dt.float32,
                           kind="ExternalOutput")

    import concourse.bass as bass_mod

    exp_late = EXP_LATE or bool(T_EXPTOKEN)
    with tile.TileContext(nc) as tc, ExitStack() as ctx:
        persist = ctx.enter_context(tc.tile_pool(name="persist", bufs=1))
        cpool = ctx.enter_context(tc.tile_pool(name="cpool", bufs=1))
        trans = ctx.enter_context(tc.tile_pool(name="trans", bufs=T_TBUFS))
        upool = ctx.enter_context(
            tc.tile_pool(name="upool",
                         bufs=T_UBUFS or max(3, B_LAG + 1,
                                             C_LAG - B_LAG + 2)))
        psum = ctx.enter_context(
            tc.tile_pool(name="psum", bufs=1, space=bass_mod.MemorySpace.PSUM))
        psum2 = ctx.enter_context(
            tc.tile_pool(name="psum2", bufs=(1 if CB_FULL else 2),
                         space=bass_mod.MemorySpace.PSUM))

        f32, bf16 = dt.float32, dt.bfloat16
        TT = nc.vector.tensor_tensor

        # ---- loads (images/labels arrive pre-cast to bf16 from host) ----
        # The DMA engines serialize transfers, so order by first use: mats
        # (PE idles until it lands), labf (masks), imgb per-channel, then the
        # big f32 logits tensor (only needed once the Act exp work starts).
        mats = persist.tile([P, 9 if T_TREEBATCH else 6, P], bf16,
                            tag="mats")
        labf = persist.tile([P, DE, WE], bf16, tag="labf")
        imgb = persist.tile([P, C, DE, WE], bf16, tag="imgb")
        x = persist.tile([P, C, DL, W], bf16, tag="x")

        def dma_img(c0, c1):
            for c in range(c0, c1):
                nc.sync.dma_start(imgb[:, c],
                                  img_d[:, c * DE * WE:(c + 1) * DE * WE])

        if T_DMAORD == 0:
            (nc.scalar if T_DMAQ else nc.sync).dma_start(mats[:], eye_d[:, :])
            dma_img(0, C)
            (nc.scalar if T_DMAQ else nc.sync).dma_start(labf[:], lab_d[:, :])
            nc.sync.dma_start(x[:], logit_d[:, :])
        elif T_DMAORD == 1:
            nc.sync.dma_start(labf[:], lab_d[:, :])
            dma_img(0, C)
            nc.sync.dma_start(mats[:], eye_d[:, :])
            nc.sync.dma_start(x[:], logit_d[:, :])
        elif T_DMAORD == 2:
            dma_img(0, 2)
            nc.sync.dma_start(labf[:], lab_d[:, :])
            dma_img(2, C)
            nc.sync.dma_start(mats[:], eye_d[:, :])
            nc.sync.dma_start(x[:], logit_d[:, :])
        else:  # 3: ch0,ch1 first, mats third, then labf/ch23/x
            dma_img(0, 2)
            nc.sync.dma_start(mats[:], eye_d[:, :])
            nc.sync.dma_start(labf[:], lab_d[:, :])
            dma_img(2, C)
            nc.sync.dma_start(x[:], logit_d[:, :])

        negI = mats[:, 0]
        shm = {1: mats[:, 1], -1: mats[:, 2]}
        ir2 = {1.0: mats[:, 3], 2.0: mats[:, 4], 3.0: mats[:, 5]}
        if T_TREEBATCH:
            nshm = {1: mats[:, 6], -1: mats[:, 7]}
            posI = mats[:, 8]

        masks = persist.tile([P, 3, DE, WE], bf16, tag="masks")

        def emit_masks():
            eng = nc.vector if T_MASKS_DVE else (
                nc.gpsimd if USE_POOLOPS else nc.vector)
            for ci, cval in enumerate((1.0, 2.0, 3.0)):
                eng.tensor_scalar(masks[:, ci], labf[:], cval, None,
                                  ALU.is_equal)

        # ---- h-shifted copies (partition shift via SBUF->SBUF DMA).
        def hshift_copies(dst_p, dst_m, src, eng):
            eng.dma_start(dst_p[0:63], src[1:64])
            eng.dma_start(dst_p[64:127], src[65:128])
            eng.dma_start(dst_p[63:64], src[63:64])
            eng.dma_start(dst_p[127:128], src[127:128])
            eng.dma_start(dst_m[1:64], src[0:63])
            eng.dma_start(dst_m[65:128], src[64:127])
            eng.dma_start(dst_m[0:1], src[0:1])
            eng.dma_start(dst_m[64:65], src[64:65])

        # masks_h before imgb_h: first mask-product use is much earlier than
        # the first DVE-path min-frame. SP ring so Act SEQ never blocks.
        masks_hp = persist.tile([P, 3, DE, WE], bf16, tag="masks_hp")
        masks_hm = persist.tile([P, 3, DE, WE], bf16, tag="masks_hm")
        msk_h = {1: masks_hp, 0: masks, -1: masks_hm}
        need_imgb_h = len(MIN_ON_PE) < 9 or len(T_MAXDVE) > 0
        if need_imgb_h:
            imgb_hp = persist.tile([P, C, DE, WE], bf16, tag="imgb_hp")
            imgb_hm = persist.tile([P, C, DE, WE], bf16, tag="imgb_hm")
            img_h = {1: imgb_hp, 0: imgb, -1: imgb_hm}

        def emit_hshifts():
            hshift_copies(masks_hp, masks_hm, masks, nc.sync)
            if need_imgb_h:
                hshift_copies(imgb_hp, imgb_hm, imgb, nc.sync)

        def cv(tile_, i, k):
            """center view shifted by (i, ., k) of a [..., DE, WE] tile."""
            return tile_[:, :, 1 + i:1 + i + DL, 1 + k:1 + k + W]

        # ---- PSUM accumulators; PE accumulates via r2-scaled identities ----
        accP = psum.tile([P, 3, DL, W], f32, tag="accP")
        SP = psum.tile([P, DL, W], f32, tag="SP")

        pl = cpool.tile([P, 4], f32, tag="pl")
        scr1 = cpool.tile([P, DL, W], f32, tag="scr1")
        scr2 = cpool.tile([P, DL, W], f32, tag="scr2")
        scr3 = cpool.tile([P, DL, W], f32, tag="scr3")

        cph = {}

        def emit_cphase_act():
            # exp-set work up front while PE/DVE wind up; p_x0 on the side
            if T_EXPTOKEN:
                # 1-element in-place bypass on x gated by the last pair's
                # DErf output: orders every x-reader emitted below (exp,
                # x0-accum) after the whole DErf block, so the act table
                # never leaves set 17 mid-stream (the readiness scheduler
                # would otherwise hoist exp into the DErf stream).
                TT(x[:, 0:1, 0:1, 0:1], x[:, 0:1, 0:1, 0:1],
                   cph['gate'][:, 0:1, 0:1, 0:1], ALU.bypass)
            if USE_XB:
                xb = cpool.tile([P, C, DL, W], bf16, tag="xb")
                nc.scalar.activation(xb[:], x[:], AF.Copy, scale=-0.5)
                cph.update(xb=xb)
            expx = cpool.tile([P, C, DL, W], bf16, tag="expx")
            nc.scalar.activation(expx[:], x[:], AF.Exp)
            if T_X0DVE:
                nc.vector.tensor_scalar(scr1[:], x[:, 0], 1.0, None,
                                        ALU.mult, accum_out=pl[:, 3:4])
            else:
                nc.scalar.activation(scr1[:], x[:, 0], AF.Copy,
                                     accum_out=pl[:, 3:4])
            cph.update(expx=expx)

        def emit_expsum():
            expx = cph['expx']
            e2 = cpool.tile([P, 2, DL, W], bf16, tag="e2")
            ene = nc.gpsimd if T_EXPSUM_POOL else nc.vector
            ene.tensor_tensor(e2[:], expx[:, 0:2], expx[:, 2:4], ALU.add)
            esum = cpool.tile([P, DL, W], bf16, tag="esum")
            ene.tensor_tensor(esum[:], e2[:, 0], e2[:, 1], ALU.add)
            cph.update(esum=esum)

        def emit_cphase_front():
            # DVE part: dxb; fused p_ym = sum(m_c * dx_c)
            if not exp_late:
                emit_expsum()
            xs = cph['xb'] if USE_XB else x
            dxb = cpool.tile([P, 3, DL, W], bf16, tag="dxb")
            TT(dxb[:], xs[:, 1:4], xs[:, 0:1].broadcast_to((P, 3, DL, W)),
               ALU.subtract)
            # p_ym = sum over (c,d,w) of m_c*dx_c: one stt dot with accum
            # (needs a contiguous mask-center copy; Pool makes it for free)
            mc = cpool.tile([P, 3, DL, W], bf16, tag="mc")
            nc.gpsimd.tensor_copy(mc[:], cv(masks, 0, 0))
            ym = cpool.tile([P, 3, DL, W], bf16, tag="ym")
            if T_YMSPLIT:
                nc.gpsimd.tensor_tensor(ym[:], mc[:], dxb[:], ALU.mult)
                scry = cpool.tile([P, 3, DL, W], bf16, tag="scry")
                nc.vector.tensor_scalar(scry[:], ym[:], 1.0, None,
                                        ALU.mult, accum_out=pl[:, 1:2])
            else:
                yme = nc.gpsimd if T_YMPOOL else nc.vector
                yme.scalar_tensor_tensor(ym[:], mc[:], 1.0, dxb[:],
                                         ALU.mult, ALU.mult,
                                         accum_out=pl[:, 1:2])
            cph.update(dxb=dxb)
            if LSE_EARLY:
                emit_cphase_back()

        def emit_cphase_back():
            # p_lse: one act-table switch back to the ln/exp set
            if exp_late:
                emit_cphase_act()
                emit_expsum()
            nc.scalar.activation(scr2[:], cph['esum'], AF.Ln,
                                 accum_out=pl[:, 0:1])

        def pe_frame_pair(i, j, k, m1p):
            """Both frames of a PE pair with batched trees: the min-frame
            uses negated stationaries (-Sh, +I) so BOTH trees are max-trees
            (Derivative_Erf is even), letting m2/m1p batch across frames."""
            cb2 = trans.tile([P, 2, C, DL, W], bf16, tag="cb2", name="cb2")
            for fr, sgn in ((0, 1), (1, -1)):
                si, sk = sgn * i, sgn * k
                st1 = shm[sgn * j] if fr == 0 else nshm[sgn * j]
                st2 = negI if fr == 0 else posI
                for half in range(2):
                    d4p = psum2.tile([P, 2, DL, W], f32, tag="d4p",
                                     name="d4p")
                    for cc in range(2):
                        c = 2 * half + cc
                        nc.tensor.matmul(d4p[:, cc], st1,
                                         imgb[:, c, 1 + si:1 + si + DL,
                                              1 + sk:1 + sk + W],
                                         start=True, stop=False)
                        nc.tensor.matmul(d4p[:, cc], st2,
                                         imgb[:, c, 1:1 + DL, 1:1 + W],
                                         start=False, stop=True)
                    nc.scalar.copy(cb2[:, fr, 2 * half:2 * half + 2],
                                   d4p[:])
            m2b = trans.tile([P, 2, 2, DL, W], bf16, tag="m2b", name="m2b")
            TT(m2b[:], cb2[:, :, 0:2], cb2[:, :, 2:4], ALU.max)
            TT(m1p[:], m2b[:, :, 0], m2b[:, :, 1], ALU.max)

        def pe_frame(jj, ii, kk, mop, m1p_slot):
            """d4 = Sh_jj.T@view(ii,kk) - center on PE; Act copyback halves;
            DVE tree into m1p_slot."""
            cb = trans.tile([P, C, DL, W], bf16, tag="cb", name="cb",
                            bufs=T_CBUFS or None)
            if CB_FULL:
                d4p = psum2.tile([P, C, DL, W], f32, tag="d4p")
                for c in range(C):
                    nc.tensor.matmul(d4p[:, c], shm[jj],
                                     imgb[:, c, 1 + ii:1 + ii + DL,
                                          1 + kk:1 + kk + W],
                                     start=True, stop=False)
                    nc.tensor.matmul(d4p[:, c], negI,
                                     imgb[:, c, 1:1 + DL, 1:1 + W],
                                     start=False, stop=True)
                nc.scalar.copy(cb[:], d4p[:])
            else:
                for half in range(2):
                    d4p = psum2.tile([P, 2, DL, W], f32, tag="d4p")
                    for cc in range(2):
                        c = 2 * half + cc
                        nc.tensor.matmul(d4p[:, cc], shm[jj],
                                         imgb[:, c, 1 + ii:1 + ii + DL,
                                              1 + kk:1 + kk + W],
                                         start=True, stop=False)
                        nc.tensor.matmul(d4p[:, cc], negI,
                                         imgb[:, c, 1:1 + DL, 1:1 + W],
                                         start=False, stop=True)
                    nc.scalar.copy(cb[:, 2 * half:2 * half + 2], d4p[:])
            m2 = trans.tile([P, 2, DL, W], bf16, tag="m2")
            TT(m2[:], cb[:, 0:2], cb[:, 2:4], mop)
            TT(m1p_slot, m2[:, 0], m2[:, 1], mop)

        # ---- software-pipelined main loop over offset pairs ----
        m1p_t, up_t, prods_t = {}, {}, {}
        P_LAG = max(B_LAG, C_LAG - T_PSPLIT)

        def stage_A(pi):
            i, j, k = PAIRS[pi]
            m1p = upool.tile([P, 2, DL, W], bf16, tag="m1p")
            m1p_t[pi] = m1p
            if j == 0:
                # single sub on an extended box serves both frames as views
                nd, nw = (9 if i else 8), (65 if k else 64)
                d0, w0 = (0 if i == 1 else 1), (0 if k == 1 else 1)
                dpe = trans.tile([P, C, nd, nw], bf16, tag="dpe",
                                 name="dpe", bufs=T_BUFS.get("dpe"))
                if pi == PAIR_ORDER[0]:
                    step = 1 if T_SUB1CH else 2
                    for ch in range(0, C, step):
                        TT(dpe[:, ch:ch + step],
                           imgb[:, ch:ch + step, d0 + i:d0 + i + nd,
                                w0 + k:w0 + k + nw],
                           imgb[:, ch:ch + step, d0:d0 + nd, w0:w0 + nw],
                           ALU.subtract)
                else:
                    TT(dpe[:],
                       imgb[:, :, d0 + i:d0 + i + nd, w0 + k:w0 + k + nw],
                       imgb[:, :, d0:d0 + nd, w0:w0 + nw], ALU.subtract)
                for fr in range(2):
                    ds = 1 - d0 - (i if fr else 0)
                    ws = 1 - w0 - (k if fr else 0)
                    mop = ALU.max if fr == 0 else ALU.min
                    dv = dpe[:, :, ds:ds + DL, ws:ws + W]
                    m2 = trans.tile([P, 2, DL, W], bf16, tag="m2",
                                    name="m2", bufs=T_M2BUFS or None)
                    TT(m2[:], dv[:, 0:2], dv[:, 2:4], mop)
                    TT(m1p[:, fr], m2[:, 0], m2[:, 1], mop)
            elif pi in T_MAXDVE:
                d4x = trans.tile([P, C, DL, W], bf16, tag="d4x", name="d4x",
                                 bufs=T_BUFS.get("d4x"))
                TT(d4x[:], cv(img_h[j], i, k), cv(imgb, 0, 0), ALU.subtract)
                m2x = trans.tile([P, 2, DL, W], bf16, tag="m2x", name="m2x",
                                 bufs=T_BUFS.get("m2x"))
                TT(m2x[:], d4x[:, 0:2], d4x[:, 2:4], ALU.max)
                TT(m1p[:, 0], m2x[:, 0], m2x[:, 1], ALU.max)
                if pi in MIN_ON_PE:
                    pe_frame(-j, -i, -k, ALU.min, m1p[:, 1])
                else:
                    d4 = trans.tile([P, C, DL, W], bf16, tag="d4")
                    TT(d4[:], cv(imgb, 0, 0), cv(img_h[-j], -i, -k),
                       ALU.subtract)
                    m2n = trans.tile([P, 2, DL, W], bf16, tag="m2n")
                    TT(m2n[:], d4[:, 0:2], d4[:, 2:4], ALU.min)
                    TT(m1p[:, 1], m2n[:, 0], m2n[:, 1], ALU.min)
            elif T_TREEBATCH and pi in MIN_ON_PE:
                pe_frame_pair(i, j, k, m1p)
            else:
                pe_frame(j, i, k, ALU.max, m1p[:, 0])
                if pi in MIN_ON_PE:
                    # sign-free: min tree of Sh_-j view(-i,-k) - center
                    pe_frame(-j, -i, -k, ALU.min, m1p[:, 1])
                else:
                    d4 = trans.tile([P, C, DL, W], bf16, tag="d4")
                    TT(d4[:], cv(imgb, 0, 0), cv(img_h[-j], -i, -k),
                       ALU.subtract)
                    m2n = trans.tile([P, 2, DL, W], bf16, tag="m2n")
                    TT(m2n[:], d4[:, 0:2], d4[:, 2:4], ALU.min)
                    TT(m1p[:, 1], m2n[:, 0], m2n[:, 1], ALU.min)

        def stage_B(pi):
            # u for both frames in one activation (const 2/sqrt(pi) cancels;
            # exp(-r2/2) lives in the scaled identity used by the acc matmuls)
            up = upool.tile([P, 2, DL, W], bf16, tag="up")
            up_t[pi] = up
            if pi == PAIR_ORDER[-1]:
                cph['gate'] = up
            m1p = m1p_t.pop(pi)
            if USE_DERF:
                nc.scalar.activation(up[:], m1p[:], AF.Derivative_Erf,
                                     scale=SQ2I)
            else:
                sqp = trans.tile([P, 2, DL, W], bf16, tag="sqp")
                nc.scalar.activation(sqp[:], m1p[:], AF.Square)
                nc.scalar.activation(up[:], sqp[:], AF.Exp, scale=-0.5)

        def stage_P(pi):
            i, j, k = PAIRS[pi]
            up = up_t[pi]
            pr2 = trans.tile([P, 2, 3, DL, W], bf16, tag="prods",
                             name="pr2", bufs=C_LAG + 2)
            prods_t[pi] = pr2
            for fr, sgn in ((0, 1), (1, -1)):
                si, sj, sk = sgn * i, sgn * j, sgn * k
                mview = cv(msk_h[sj], si, sk)
                ub = up[:, fr:fr + 1].broadcast_to((P, 3, DL, W))
                if (pi, fr) in T_PRODS_SPLIT:
                    # channel-split: 2ch on DVE, 1ch on Pool (finer quanta)
                    ub2 = up[:, fr:fr + 1].broadcast_to((P, 2, DL, W))
                    TT(pr2[:, fr, 0:2], ub2, mview[:, 0:2], ALU.mult)
                    ub1 = up[:, fr:fr + 1].broadcast_to((P, 1, DL, W))
                    nc.gpsimd.tensor_tensor(pr2[:, fr, 2:3], ub1,
                                            mview[:, 2:3], ALU.mult)
                else:
                    eng = (nc.gpsimd if (pi, fr) in T_PRODS_POOL
                           else nc.vector)
                    eng.tensor_tensor(pr2[:, fr], ub, mview, ALU.mult)

        def stage_C(pi):
            i, j, k = PAIRS[pi]
            r2 = float(i * i + j * j + k * k)
            st, sp = (pi == PAIR_ORDER[0]), (pi == PAIR_ORDER[-1])
            up = up_t.pop(pi)
            pr2 = prods_t.pop(pi)
            for fr in range(2):
                nc.tensor.matmul(SP[:], ir2[r2], up[:, fr],
                                 start=(st and fr == 0), stop=(sp and fr == 1))
                for ci in range(3):
                    nc.tensor.matmul(accP[:, ci], ir2[r2], pr2[:, fr, ci],
                                     start=(st and fr == 0),
                                     stop=(sp and fr == 1))

        NP = len(PAIR_ORDER)
        for idx in range(NP + max(B_LAG, C_LAG)):
            if idx == CPHASE_AT:
                emit_cphase_front()
            if idx == 0:
                emit_masks()
            if idx < NP:
                stage_A(PAIR_ORDER[idx])
            if idx == 0:
                emit_hshifts()
            if B_LAG <= idx < NP + B_LAG:
                stage_B(PAIR_ORDER[idx - B_LAG])
            if P_LAG <= idx < NP + P_LAG:
                stage_P(PAIR_ORDER[idx - P_LAG])
            if C_LAG <= idx < NP + C_LAG:
                stage_C(PAIR_ORDER[idx - C_LAG])
            if idx == CACT_AT and not exp_late:
                emit_cphase_act()
        if CPHASE_AT >= NP:
            emit_cphase_front()
        if not LSE_EARLY:
            emit_cphase_back()

        # ---- tail: p_w = sum(-0.5/S * sum_c dx_c*acc_c) ----
        dxb = cph['dxb']
        rS = cpool.tile([P, DL, W], f32, tag="rS")
        nc.vector.reciprocal_approx_fast(rS[:], SP[:])
        tp = cpool.tile([P, 3, DL, W], bf16, tag="tp")
        if T_TAILSPLIT:
            for ci in range(3):
                TT(tp[:, ci], accP[:, ci], dxb[:, ci], ALU.mult)
        else:
            TT(tp[:], accP[:], dxb[:], ALU.mult)
        t1 = cpool.tile([P, DL, W], bf16, tag="t1")
        TT(t1[:], tp[:, 0], tp[:, 1], ALU.add)
        t2 = cpool.tile([P, DL, W], bf16, tag="t2")
        TT(t2[:], t1[:], tp[:, 2], ALU.add)
        nc.vector.scalar_tensor_tensor(scr3[:], t2[:], -0.5, rS[:],
                                       ALU.mult, ALU.mult,
                                       accum_out=pl[:, 2:3])
        nc.sync.dma_start(out_d[:, :], pl[:])

    nc.compile()
    return nc


def _get_nc():
    if "nc" not in _CACHED:
        _CACHED["nc"] = _build_nc()
    return _CACHED["nc"]


def make_in_maps(inputs, labels, images):
    """Host-side shard: full inputs -> per-core input dicts (layout prep:
    (b,h)->partition transpose, d/w halo padding, bf16 pre-cast)."""
    import ml_dtypes

    bf = ml_dtypes.bfloat16
    img = np.asarray(images, np.float32).astype(bf)
    lab = np.asarray(labels).astype(bf)  # values 0..3, exact in bf16
    lgt = np.ascontiguousarray(np.asarray(inputs, np.float32))

    img_p = np.pad(img, ((0, 0), (0, 0), (1, 1), (0, 0), (1, 1)), mode="edge")
    lab_p = np.pad(lab, ((0, 0), (1, 1), (0, 0), (1, 1)), mode="edge")

    in_maps = []
    for k in range(NCORES):
        d0 = k * DL
        ic = img_p[:, :, d0:d0 + DE]          # [2,4,10,64,66]
        lc = lab_p[:, d0:d0 + DE]             # [2,10,64,66]
        xc = lgt[:, :, d0:d0 + DL]            # [2,4,8,64,64]
        im = np.ascontiguousarray(ic.transpose(0, 3, 1, 2, 4)).reshape(P, -1)
        lm = np.ascontiguousarray(lc.transpose(0, 2, 1, 3)).reshape(P, -1)
        xm = np.ascontiguousarray(
            xc.transpose(0, 3, 1, 2, 4)).reshape(P, -1).astype(bf)
        in_maps.append({"img": im, "lab": lm, "logits": xm, "eye": _mats()})
    return in_maps


def _mats():
    """[-I, Sh+-1, 3x scaled I, -Sh+-1, I] as one [P, 9P] bf16
    array. Sh(j)[k, m] = 1 iff k = b(m)*64 + clamp(h(m)+j, 0, 63)."""
    import ml_dtypes

    eye = np.eye(P, dtype=np.float32)
    sh = {}
    for jj in (1, -1):
        M = np.zeros((P, P), np.float32)
        for m in range(P):
            b, h = divmod(m, 64)
            M[b * 64 + min(max(h + jj, 0), 63), m] = 1.0
        sh[jj] = M
    blocks = [-eye, sh[1], sh[-1],
              np.exp(-0.5) * eye, np.exp(-1.0) * eye, np.exp(-1.5) * eye]
    if T_TREEBATCH:
        blocks += [-sh[1], -sh[-1], eye]
    out = np.concatenate(blocks, axis=1)
    return np.ascontiguousarray(out).astype(ml_dtypes.bfloat16)


def kernel(inputs, labels, images):
    from concourse.bass_utils import run_bass_kernel_spmd

    nc = _get_nc()
    in_maps = make_in_maps(inputs, labels, images)
    res = run_bass_kernel_spmd(nc, in_maps, core_ids=list(range(NCORES)))
    total = 0.0
    for k in range(NCORES):
        pl = res.results[k]["partials"].astype(np.float64)
        ym_scale = -1.0 if USE_XB else 0.5
        total += (pl[:, 0] - ym_scale * pl[:, 1] - pl[:, 3] + pl[:, 2]).sum()
    return np.float32(total / NVOX)

